# revision 29
# baseline (speedup 1.0000x reference)
"""MACE+Ewald forward on 8 Trainium2 NeuronCores.

Sharding: graph-per-core (8 graphs, 8 cores). Atoms balanced across 4 blocks
of <=128 slots (padded NL=512 per core); edges assigned to the core/block
owning their dst atom, packed into 128-edge tiles with per-block tile counts.

Key device-side structure per layer:
  1. hu = h @ Wup computed atom-major per block, AllGather (bf16, Shared out)
     kicked immediately so the Ewald block + radial-MLP prepass overlap it.
  2. Ewald: structure factors / he MLP, all bf16 matmuls.
  3. Radial MLP prepass for all edge tiles (gather-independent).
  4. Edge loop: batched indirect gather of hu rows per block; per tile the
     product-basis messages are scattered to dst atoms with matmuls whose
     moving operand is a host-precomputed segY matrix (the one-hot dst
     scatter matrix with the spherical harmonics Y and 1/avg_nei folded in),
     c-major output so no transposes are needed afterwards.
  5. Product basis (A^2 contractions) per block, h update, readout.

All heavy matmuls run with bf16 operands (4x PE throughput vs fp32; the
harness tolerance is 2e-2, measured error stays ~1e-3). segY ships as
fp8e4m3 to halve its DMA stream.
"""

import numpy as np
import ml_dtypes

C = 128
L = 2
NB = 8
NEL = 10
BG = 8
N_ATOMS = 3200
N_EDGES = 51200
R_MAX = 5.0
P_CUT = 5.0
AVG_NEI = 16.0
DELTA_K = 0.2
NKRBF = 128
DP = 8
SKIP = (2.0 + 1.0) ** -0.5
NL = 512            # padded atoms per core
NBLK = NL // 128    # atom groups per core (128 slots each)
SUB = 4             # scatter sub-blocks per group
ABLK = 32           # atoms per sub-block
NSUB = NBLK * SUB   # 16
KPAD = 128          # padded k-point count (real: 123)
LOFLM = np.repeat(np.arange(4), [1, 3, 5, 7])   # [16]
L_START = [0, 1, 4, 9]
L_WIDTH = [1, 3, 5, 7]
# scatter matmul chunks: (l, first lm, number of lm) with moving-free <= 512
CHUNKS = [(0, 0, 1), (1, 1, 3), (2, 4, 4), (2, 8, 1), (3, 9, 4), (3, 13, 3)]
SEGY_FP8 = True
HU_FP8 = True
SCAT_DR = True   # fp8 DoubleRow scatter (2 edge tiles per PE pass)      # layer>=1 hu AllGather + gather in fp8e4m3

_CACHE = {}


def _const_layouts(G4):
    """Constant packing: 3 fused SBUF-resident buffers loaded with one DMA
    each (HWDGE issue time for ~50 separate loads dominated kernel startup).
    cbA = layer-0 critical path, cbB = the rest, cf = fp32 smalls."""
    bfA = [('attrsT', NEL, NL), ('Wembed', NEL, C),
           ('rW1_0', NB, 64), ('rW2_0', 64, 64), ('rW3_0', 64, 64), ('rW4_0', 64, 4 * C),
           ('efTpack', 8, G4 * 128)]
    bfB = [('Wpre1_0', C, C), ('Wpre2_0', C, C), ('Wm1_0', C, C), ('Wm2_0', C, C),
           ('Wmix_0', C, C), ('Wup_1', C, C),
           ('cosdam', 128, NBLK * KPAD), ('sindam', 128, NBLK * KPAD),
           ('cosdkm', KPAD, NL), ('sindkm', KPAD, NL),
           ('ident', 128, 128), ('Wr0', C, 1), ('Wr1a', C, 16), ('Wr1b', 16, 1),
           ('Wpre1_1', C, C), ('Wpre2_1', C, C), ('Wm1_1', C, C), ('Wm2_1', C, C),
           ('Wmix_1', C, C),
           ('rW1_1', NB, 64), ('rW2_1', 64, 64), ('rW3_1', 64, 64), ('rW4_1', 64, 4 * C)]
    cf = ([('kfilt_0', KPAD, C), ('kfilt_1', KPAD, C)]
          + [(f'w{j}T_{i}', C, 4) for i in range(L) for j in (2, 3)]
          + [(f'{nm}_{i}', C, 1) for i in range(L) for nm in ('bpre1', 'bpre2', 'bm1', 'bm2')]
          + [(f'{nm}_{i}', 64, 1) for i in range(L) for nm in ('rb1', 'rb2', 'rb3')])
    return {'cbA': bfA, 'cbB': bfB, 'cf': cf}


def unpack_consts(m, G4):
    """Recover named f32 views from a core's fused const buffers (for host_sim)."""
    out = {}
    for buf, entries in _const_layouts(G4).items():
        c0 = 0
        for name, rows, cols in entries:
            out[name] = np.asarray(m[buf][0:rows, c0:c0 + cols], np.float32)
            c0 += cols
    return out


# ---------------------------------------------------------------- host math
def _sph_np(u):
    x, y, z = u[:, 0], u[:, 1], u[:, 2]
    s3, s5, s15 = 3.0 ** 0.5, 5.0 ** 0.5, 15.0 ** 0.5
    c70, c105, c42, c7 = 70.0 ** 0.5 / 4.0, 105.0 ** 0.5, 42.0 ** 0.5 / 4.0, 7.0 ** 0.5 / 2.0
    comps = [np.ones_like(x),
             s3 * x, s3 * y, s3 * z,
             s15 * x * y, s15 * y * z, 0.5 * s5 * (3 * z * z - 1.0), s15 * x * z,
             0.5 * s15 * (x * x - y * y),
             c70 * y * (3 * x * x - y * y), c105 * x * y * z, c42 * y * (5 * z * z - 1.0),
             c7 * z * (5 * z * z - 3.0), c42 * x * (5 * z * z - 1.0),
             0.5 * c105 * z * (x * x - y * y), c70 * x * (x * x - 3 * y * y)]
    return np.stack(comps, axis=-1).astype(np.float32)


def _radial_np(r):
    n = np.arange(1, NB + 1, dtype=np.float32)
    rb = np.sqrt(2.0 / R_MAX) * np.sin(n * np.pi * r[:, None] / R_MAX) / np.maximum(r, 1e-9)[:, None]
    uu = np.clip(r / R_MAX, 0.0, 1.0)
    p = P_CUT
    env = 1.0 - (p + 1.0) * (p + 2.0) / 2.0 * uu ** 5 + p * (p + 2.0) * uu ** 6 - p * (p + 1.0) / 2.0 * uu ** 7
    env = env * (r < R_MAX)
    return (rb * env[:, None]).astype(np.float32)


def host_prep(inputs):
    """Build per-core padded arrays. Returns (in_maps, T_list, G4, e0)."""
    f32 = np.float32
    bf16 = ml_dtypes.bfloat16
    segy_np = ml_dtypes.float8_e4m3 if SEGY_FP8 else bf16
    pos = np.asarray(inputs['positions'], f32)
    attrs = np.asarray(inputs['node_attrs'], f32)
    shifts = np.asarray(inputs['shifts'], f32)
    eidx = np.asarray(inputs['edge_index']).astype(np.int64)
    batch = np.asarray(inputs['batch']).astype(np.int64)
    kgrid = np.asarray(inputs['kgrid'], f32)
    krbf = np.asarray(inputs['krbf'], f32)
    K = kgrid.shape[0]

    # per-graph contiguous atom ranges (batch is sorted)
    starts = np.searchsorted(batch, np.arange(BG))
    ends = np.searchsorted(batch, np.arange(BG), side='right')
    counts = ends - starts
    assert counts.max() <= NL, counts

    # balanced split of each graph's atoms into NSUB sub-blocks of <=ABLK slots
    slot = np.zeros(N_ATOMS, np.int64)          # padded local slot per atom
    for b in range(BG):
        n = int(counts[b])
        base, rem = divmod(n, NSUB)
        sizes = [base + (k < rem) for k in range(NSUB)]
        assert max(sizes) <= ABLK
        cum = 0
        for k in range(NSUB):
            j = np.arange(cum, cum + sizes[k])
            slot[starts[b] + j] = k * ABLK + (j - cum)
            cum += sizes[k]
    pid = (batch * NL + slot).astype(np.int32)  # padded global id [N]

    # ---- edge geometry (host) ----
    src, dst = eidx[0], eidx[1]
    vec = pos[dst] - pos[src] + shifts
    r = np.linalg.norm(vec.astype(np.float64), axis=1).astype(f32)
    uvec = vec / np.maximum(r, 1e-9)[:, None]
    Y = _sph_np(uvec)                           # [E,16]
    ef = _radial_np(r)                          # [E,8]

    # ---- Ewald geometry (host) ----
    dot = pos @ kgrid.T                         # [N,K]
    sd = np.prod(np.sinc(0.5 * DELTA_K * pos), axis=1).astype(f32)   # [N]
    cosd = (sd[:, None] * np.cos(dot)).astype(f32)
    sind = (sd[:, None] * np.sin(dot)).astype(f32)

    kdown = krbf @ np.asarray(inputs['Wdown'], f32)      # [K,DP]

    # ---- edge -> (core, sub-block) assignment, per-sub tile counts ----
    gdst = batch[dst]
    kblk = slot[dst] // ABLK
    ecount = np.zeros((BG, NSUB), np.int64)
    np.add.at(ecount, (gdst, kblk), 1)
    T_list = [max(1, int(np.ceil(ecount[:, k].max() / 128))) for k in range(NSUB)]
    if SCAT_DR:
        T_list = [t + (t % 2) for t in T_list]
    O_list = np.concatenate([[0], np.cumsum(T_list)]).astype(int)
    NT = int(O_list[-1])
    G4 = ((NT + 3) // 4) * 4

    # ---- shared (replicated) weight arrays ----
    g = lambda k: np.asarray(inputs[k], f32)
    shared = {'Wembed': g('W_embed'),
              'ident': np.eye(128, dtype=f32),
              'Wr0': g('Wr0'), 'Wr1a': g('Wr1a'), 'Wr1b': g('Wr1b')}
    # layer-0 hu is weight-only (h0 = attrs @ Wembed): precompute the full
    # gathered table on the host, killing the first AllGather.
    h0_full = attrs @ g('W_embed')                       # [N, C]
    huG0 = np.zeros((BG * NL, C), f32)
    huG0[pid] = h0_full @ g('Wup')[0]
    huG0 = huG0.astype(bf16)
    for i in range(L):
        for nm in ('Wpre1', 'Wpre2', 'Wm1', 'Wm2', 'Wup', 'Wmix'):
            shared[f'{nm}_{i}'] = g(nm)[i]
        shared[f'rW1_{i}'] = g('rW1')[i]
        shared[f'rW2_{i}'] = g('rW2')[i]
        shared[f'rW3_{i}'] = g('rW3')[i]
        # rW4 reshaped l-major: [64, l*128 + c]
        shared[f'rW4_{i}'] = g('rW4')[i].reshape(64, C, 4).transpose(0, 2, 1).reshape(64, 4 * C)
        for nm in ('bpre1', 'bpre2', 'bm1', 'bm2'):
            shared[f'{nm}_{i}'] = g(nm)[i].reshape(C, 1)
        for nm in ('rb1', 'rb2', 'rb3'):
            shared[f'{nm}_{i}'] = g(nm)[i].reshape(64, 1)
        kf = np.zeros((KPAD, C), f32)
        kf[:K] = 0.01 * (kdown @ g('WupE')[i])
        shared[f'kfilt_{i}'] = kf
        shared[f'w2T_{i}'] = g('w2')[i].T.copy()             # [C,4] f32
        shared[f'w3T_{i}'] = g('w3')[i].T.copy()

    layouts = _const_layouts(G4)

    # ---- per-core arrays ----
    in_maps = []
    for b in range(BG):
        sl = slice(starts[b], ends[b])
        per = {}
        slot_b = slot[sl]
        at = np.zeros((NEL, NL), f32)
        at[:, slot_b] = attrs[sl].T
        per['attrsT'] = at
        cam = np.zeros((128, NBLK * KPAD), f32)   # atom-major cosd, per block
        sam = np.zeros((128, NBLK * KPAD), f32)
        ckm = np.zeros((KPAD, NL), f32)           # k-major
        skm = np.zeros((KPAD, NL), f32)
        pr, bb = slot_b % 128, slot_b // 128
        cam.reshape(128, NBLK, KPAD)[pr, bb, :K] = cosd[sl]
        sam.reshape(128, NBLK, KPAD)[pr, bb, :K] = sind[sl]
        ckm[:K, slot_b] = cosd[sl].T
        skm[:K, slot_b] = sind[sl].T
        per['cosdam'], per['sindam'] = cam, sam
        per['cosdkm'], per['sindkm'] = ckm, skm

        efp = np.zeros((8, G4 * 128), f32)
        sip = np.zeros((128, NT), np.int32)
        segY = np.zeros((128, NT * 16 * ABLK), f32)
        emask = gdst == b
        for k in range(NSUB):
            es = np.nonzero(emask & (kblk == k))[0]
            es = es[np.argsort(slot[dst[es]], kind='stable')]
            s = np.arange(len(es))
            tt, p = s // 128, s % 128
            t = O_list[k] + tt
            efp[:, t * 128 + p] = ef[es].T
            sip[p, t] = pid[src[es]]
            a = slot[dst[es]] - k * ABLK
            base = t * (16 * ABLK) + a
            for lm in range(16):
                segY[p, base + lm * ABLK] = Y[es, lm] / AVG_NEI
        per['efTpack'] = efp

        def pack(entries, np_dt):
            width = sum(e[2] for e in entries)
            arr = np.zeros((128, width), np_dt)
            c0 = 0
            for name, rows, cols in entries:
                src_a = per.get(name, shared.get(name))
                arr[0:rows, c0:c0 + cols] = src_a
                c0 += cols
            return arr

        m = {'srcidx': sip, 'segYpack': segY.astype(segy_np), 'huG0': huG0,
             'cbA': pack(layouts['cbA'], bf16), 'cbB': pack(layouts['cbB'], bf16),
             'cf': pack(layouts['cf'], f32)}
        in_maps.append(m)

    e0 = np.zeros(BG, f32)
    ae = attrs @ np.asarray(inputs['atomic_E'], f32)
    for b in range(BG):
        e0[b] = ae[starts[b]:ends[b]].sum()
    return in_maps, T_list, G4, e0


# ---------------------------------------------------------------- device
def build_kernel(T_list, G4):
    import concourse.bass as bass
    import concourse.bacc as bacc
    import concourse.mybir as mybir
    import concourse.tile as tile

    f32 = mybir.dt.float32
    bf16 = mybir.dt.bfloat16
    sdt = mybir.dt.float8e4 if SEGY_FP8 else bf16
    A = mybir.ActivationFunctionType
    NT = int(sum(T_list))
    Tmax = max(T_list)
    O_list = np.concatenate([[0], np.cumsum(T_list)]).astype(int)
    nc = bacc.Bacc("TRN2", target_bir_lowering=False, debug=False, num_devices=BG)

    dins = {}
    def din(name, shape, dt=f32):
        dins[name] = nc.dram_tensor(name, list(shape), dt, kind="ExternalInput").ap()
        return dins[name]

    # load order = SP queue order: the layer-0 critical path first
    layouts = _const_layouts(G4)
    widths = {buf: sum(e[2] for e in entries) for buf, entries in layouts.items()}
    din('srcidx', (128, NT), mybir.dt.int32)
    din('cbA', (128, widths['cbA']), bf16)
    din('cf', (128, widths['cf']))
    din('cbB', (128, widths['cbB']), bf16)
    segY_d = din('segYpack', (128, NT * 16 * ABLK), sdt)
    huG0_d = din('huG0', (BG * NL, C), bf16)
    en_out = nc.dram_tensor('en_out', [1, 1], f32, kind="ExternalOutput").ap()

    with tile.TileContext(nc) as tc:
        with (
            tc.tile_pool(name="const", bufs=1) as cp,
            tc.tile_pool(name="work", bufs=2) as wp,
            tc.tile_pool(name="segy", bufs=3) as sy,
            tc.tile_pool(name="big", bufs=1) as bp,
            tc.tile_pool(name="psA", bufs=1, space="PSUM") as psA,
            tc.tile_pool(name="psS", bufs=2, space="PSUM") as psS,
            tc.tile_pool(name="psW", bufs=2, space="PSUM") as psW,
            tc.tile_pool(name="dram", bufs=1, space="DRAM") as dp,
        ):
            sb = {}
            for name in ('srcidx', 'cbA', 'cf', 'cbB'):
                ap = dins[name]
                t = cp.tile(list(ap.shape), ap.dtype, tag=name)
                nc.sync.dma_start(t[:], ap[:])
                if name == 'srcidx':
                    sb[name] = t
                else:
                    c0 = 0
                    for nm, rows, cols in layouts[name]:
                        sb[nm] = t[0:rows, c0:c0 + cols]
                        c0 += cols

            h = bp.tile([C, NL], bf16, tag="h")
            en = bp.tile([1, 1], f32, tag="en")
            feats_cm = bp.tile([C, NL], bf16, tag="feats_cm")
            hres_am = bp.tile([128, NBLK * 128], bf16, tag="hres_am")
            nc.vector.memset(en[:], 0.0)

            pe = psW.tile([C, NL], f32, tag="pb")
            nc.tensor.matmul(pe[:], sb['Wembed'][:], sb['attrsT'][:], start=True, stop=True)
            nc.scalar.activation(h[:], pe[:], A.Copy)

            hu_dt = mybir.dt.float8e4 if HU_FP8 else bf16
            coll = {}     # layer -> (huL, huG) for layers >= 1
            for i in range(L):
                # ---- gather source: host table (layer 0) or prior AllGather ----
                huG = huG0_d if i == 0 else coll[i][1][:]
                hugs = []
                TGmax = max(int(O_list[4 * k + 4] - O_list[4 * k]) for k in range(NBLK))
                for k in range(NBLK):
                    lo, hi = int(O_list[4 * k]), int(O_list[4 * k + 4])
                    hg = wp.tile([128, TGmax * 128], bf16 if i == 0 else hu_dt,
                                 tag=f"hug{k % 2}{i}", bufs=1)
                    nc.gpsimd.indirect_dma_start(
                        out=hg[:, 0:(hi - lo) * 128], out_offset=None, in_=huG[:],
                        in_offset=bass.IndirectOffsetOnAxis(
                            ap=sb['srcidx'][:, lo:hi], axis=0))
                    hugs.append(hg)
                if i + 1 < L:
                    huL_next = dp.tile([NL, C], hu_dt, tag=f"huL{i + 1}")
                    huG_next = dp.tile([BG * NL, C], hu_dt, tag=f"huG{i + 1}",
                                       addr_space="Shared")
                    hu_am = wp.tile([128, NL], hu_dt, tag="hu_am")
                    coll[i + 1] = (huL_next, huG_next)

                # ---- radial MLP: issued just-in-time, one 4-tile group ahead
                # of the edge loop (the matmul->silu chain is ~3us latency and
                # would serialize as a phase; interleaved it hides behind the
                # per-tile scatter work).
                s3sb = wp.tile([64, G4 * 128], bf16, tag="s3sb")
                radial_next = [0]

                def radial_group(gidx, i=i, s3sb=s3sb):
                    gsl = slice(gidx * 512, (gidx + 1) * 512)
                    pr1 = psS.tile([128, 512], f32, tag="ps")
                    nc.tensor.matmul(pr1[0:64, :], sb[f'rW1_{i}'][:], sb['efTpack'][:, gsl],
                                     start=True, stop=True)
                    s1 = wp.tile([64, 512], bf16, tag="s1")
                    nc.scalar.activation(s1[:], pr1[0:64, :], A.Silu, bias=sb[f'rb1_{i}'][:])
                    pr2 = psS.tile([128, 512], f32, tag="ps")
                    nc.tensor.matmul(pr2[0:64, :], sb[f'rW2_{i}'][:], s1[:], start=True, stop=True)
                    s2 = wp.tile([64, 512], bf16, tag="s1")
                    nc.scalar.activation(s2[:], pr2[0:64, :], A.Silu, bias=sb[f'rb2_{i}'][:])
                    pr3 = psS.tile([128, 512], f32, tag="ps")
                    nc.tensor.matmul(pr3[0:64, :], sb[f'rW3_{i}'][:], s2[:], start=True, stop=True)
                    nc.scalar.activation(s3sb[:, gsl], pr3[0:64, :], A.Silu, bias=sb[f'rb3_{i}'][:])

                def ensure_radial(gwant):
                    while radial_next[0] <= min(gwant, G4 // 4 - 1):
                        radial_group(radial_next[0])
                        radial_next[0] += 1

                ensure_radial(1)
                # ---- Ewald block (independent of the collective) ----
                p1 = psW.tile([C, NL], f32, tag="pb")
                nc.tensor.matmul(p1[:], sb[f'Wpre1_{i}'][:], h[:], start=True, stop=True)
                t1 = wp.tile([C, NL], bf16, tag="t1")
                nc.scalar.activation(t1[:], p1[:], A.Silu, bias=sb[f'bpre1_{i}'][:])
                p2 = psW.tile([C, NL], f32, tag="pb")
                nc.tensor.matmul(p2[:], sb[f'Wpre2_{i}'][:], t1[:], start=True, stop=True)
                hres = wp.tile([C, NL], bf16, tag="hres")
                nc.vector.tensor_scalar_add(hres[:], p2[:], sb[f'bpre2_{i}'][:])
                nc.vector.tensor_add(hres[:], hres[:], h[:])
                for k in range(NBLK):
                    pt = psS.tile([128, 512], f32, tag="ps")
                    ptb = pt[:].bitcast(bf16)[:, 0:128]
                    nc.tensor.transpose(ptb, hres[:, k * 128:(k + 1) * 128], sb['ident'][:])
                    nc.scalar.activation(hres_am[:, k * 128:(k + 1) * 128], ptb, A.Copy)
                sfk = {}
                for nm, am in (('r', 'cosdam'), ('i', 'sindam')):
                    psf = psS.tile([128, 512], f32, tag="ps")
                    for k in range(NBLK):
                        nc.tensor.matmul(psf[:, 0:128], sb[am][:, k * KPAD:(k + 1) * KPAD],
                                         hres_am[:, k * 128:(k + 1) * 128],
                                         start=(k == 0), stop=(k == NBLK - 1))
                    s = wp.tile([KPAD, C], bf16, tag=f"sfk{nm}")
                    nc.vector.tensor_tensor(s[:], psf[:, 0:128], sb[f'kfilt_{i}'][:],
                                            op=mybir.AluOpType.mult)
                    sfk[nm] = s
                phe = psW.tile([C, NL], f32, tag="pb")
                nc.tensor.matmul(phe[:], sfk['r'][:], sb['cosdkm'][:], start=True, stop=False)
                nc.tensor.matmul(phe[:], sfk['i'][:], sb['sindkm'][:], start=False, stop=True)
                he0 = wp.tile([C, NL], bf16, tag="he0")
                nc.scalar.activation(he0[:], phe[:], A.Copy)
                pm1 = psW.tile([C, NL], f32, tag="pb")
                nc.tensor.matmul(pm1[:], sb[f'Wm1_{i}'][:], he0[:], start=True, stop=True)
                tm = wp.tile([C, NL], bf16, tag="t1")
                nc.scalar.activation(tm[:], pm1[:], A.Silu, bias=sb[f'bm1_{i}'][:])
                pm2 = psW.tile([C, NL], f32, tag="pb")
                nc.tensor.matmul(pm2[:], sb[f'Wm2_{i}'][:], tm[:], start=True, stop=True)
                he2 = wp.tile([C, NL], bf16, tag="he2")
                nc.scalar.activation(he2[:], pm2[:], A.Silu, bias=sb[f'bm2_{i}'][:])
                if i > 0:
                    # PE is otherwise idle while the AllGather is in flight:
                    # run the whole radial MLP pipeline under it.
                    ensure_radial(G4 // 4 - 1)

                # ---- edge loop ----
                SW = 16 * ABLK      # segY columns per tile

                def issue_pair(gs, tt0, alt, i=i, hugs=hugs):
                    # one 2-tile unit of sub-block gs: paired segY DMA, two rW4
                    # matmuls, two mw products written fp8 into one [128,1024]
                    # tile for the DoubleRow scatter.
                    t = int(O_list[gs]) + tt0
                    loc = t - int(O_list[4 * (gs // 4)])   # tile idx within group
                    ensure_radial((t + 1) // 4 + 1)
                    sgt = sy.tile([128, 2 * SW], sdt, tag="sg")
                    nc.sync.dma_start(sgt[:], segY_d[:, t * SW:(t + 2) * SW])
                    mw = wp.tile([128, 1024], sdt, tag="mw")
                    for j in range(2):
                        pw = psW.tile([C, NL], f32, tag="pb")
                        nc.tensor.matmul(pw[:], s3sb[:, (t + j) * 128:(t + j + 1) * 128],
                                         sb[f'rW4_{i}'][:], start=True, stop=True)
                        nc.vector.tensor_tensor(
                            mw[:, j * 512:(j + 1) * 512].rearrange("p (l c) -> p l c", l=4),
                            pw[:].rearrange("p (l c) -> p l c", l=4),
                            hugs[gs // 4][:, (loc + j) * 128:(loc + j + 1) * 128]
                                .unsqueeze(1).broadcast_to([128, 4, 128]),
                            op=mybir.AluOpType.mult)
                    return mw, sgt

                PAIRS = [(gs, tt0) for gs in range(NSUB) for tt0 in range(0, int(T_list[gs]), 2)]
                LEAD = 1
                fifo = [issue_pair(*PAIRS[j], alt=(j % 2 == 0)) for j in range(LEAD)]
                tidx = [0]

                def next_mw_sgt():
                    j = tidx[0]
                    if j + LEAD < len(PAIRS):
                        fifo.append(issue_pair(*PAIRS[j + LEAD], alt=(j % 2 == 0)))
                    tidx[0] += 1
                    return fifo.pop(0)

                def make_tail(k, i=i, he2=he2):
                    # layer tail for block k: h update, readout, next layer's hu.
                    # Deferred into the next block's tile stream so the product
                    # basis (DVE) overlaps the next block's scatters (PE).
                    def tail():
                        blk = slice(k * 128, (k + 1) * 128)
                        pmx = psW.tile([C, NL], f32, tag="pb")
                        nc.tensor.matmul(pmx[:, 0:128], sb[f'Wmix_{i}'][:],
                                         feats_cm[:, blk], start=True, stop=True)
                        hnk = wp.tile([C, 128], f32, tag="hn")
                        nc.vector.tensor_add(hnk[:], pmx[:, 0:128], he2[:, blk])
                        nc.vector.tensor_add(hnk[:], hnk[:], h[:, blk])
                        nc.vector.tensor_scalar_mul(h[:, blk], hnk[:], float(SKIP))
                        if i == 0:
                            prd = psS.tile([128, 512], f32, tag="ps")
                            nc.tensor.matmul(prd[0:1, 0:128], sb['Wr0'][:], h[:, blk],
                                             start=True, stop=True)
                            rs = wp.tile([1, 1], f32, tag="rs")
                            nc.vector.reduce_sum(rs[:], prd[0:1, 0:128],
                                                 axis=mybir.AxisListType.X)
                            nc.vector.tensor_add(en[:], en[:], rs[:])
                        else:
                            pra = psS.tile([128, 512], f32, tag="ps")
                            nc.tensor.matmul(pra[0:16, 0:128], sb['Wr1a'][:], h[:, blk],
                                             start=True, stop=True)
                            ta = wp.tile([16, 128], bf16, tag="ta")
                            nc.scalar.activation(ta[:], pra[0:16, 0:128], A.Silu)
                            prb = psS.tile([128, 512], f32, tag="ps")
                            nc.tensor.matmul(prb[0:1, 0:128], sb['Wr1b'][:], ta[:],
                                             start=True, stop=True)
                            rs = wp.tile([1, 1], f32, tag="rs")
                            nc.vector.reduce_sum(rs[:], prb[0:1, 0:128],
                                                 axis=mybir.AxisListType.X)
                            nc.vector.tensor_add(en[:], en[:], rs[:])
                        if i + 1 < L:
                            ph = psW.tile([C, NL], f32, tag="pb")
                            nc.tensor.matmul(ph[:, 0:128], h[:, blk],
                                             sb[f'Wup_{i + 1}'][:], start=True, stop=True)
                            nc.scalar.activation(hu_am[:, blk], ph[:, 0:128], A.Copy)
                            nc.sync.dma_start(huL_next[k * 128:(k + 1) * 128, :],
                                              hu_am[:, blk])
                    return tail

                pending_tail = None
                for k in range(NBLK):
                    pA1 = psA.tile([128, 1024], f32, tag="pA1")
                    pA2 = psA.tile([128, 1024], f32, tag="pA2")
                    for s in range(SUB):
                        gs = 4 * k + s
                        Tk = int(T_list[gs])
                        pAh = pA1 if s < 2 else pA2
                        cb = (s % 2) * 512
                        for pp in range(Tk // 2):
                            mw, sgt = next_mw_sgt()
                            mw3 = mw[:].rearrange("p (two x) -> p two x", two=2)
                            sg3 = sgt[:].rearrange("p (two x) -> p two x", two=2)
                            for (l, m0, w) in CHUNKS:
                                nc.tensor.matmul(
                                    pAh[:, cb + m0 * ABLK:cb + (m0 + w) * ABLK],
                                    mw3[:, :, l * 128:(l + 1) * 128],
                                    sg3[:, :, m0 * ABLK:(m0 + w) * ABLK],
                                    start=(pp == 0), stop=(pp == Tk // 2 - 1),
                                    perf_mode=mybir.MatmulPerfMode.DoubleRow)
                            if pending_tail is not None and s == 0 and pp == min(1, Tk // 2 - 1):
                                pending_tail()
                                pending_tail = None
                    # free pA early: scal copy + A^2 on the scalar engine, then
                    # the DVE product basis reads SBUF only.
                    scal = wp.tile([128, 128], bf16, tag="scal")
                    nc.scalar.activation(scal[:, 0:64].rearrange("c (s a) -> c s a", s=2),
                                         pA1[:].rearrange("c (s x) -> c s x", s=2)[:, :, 0:ABLK],
                                         A.Copy)
                    nc.scalar.activation(scal[:, 64:128].rearrange("c (s a) -> c s a", s=2),
                                         pA2[:].rearrange("c (s x) -> c s x", s=2)[:, :, 0:ABLK],
                                         A.Copy)
                    AA = wp.tile([128, 2048], bf16, tag="AA")
                    nc.scalar.activation(AA[:, 0:1024], pA1[:], A.Square)
                    nc.scalar.activation(AA[:, 1024:2048], pA2[:], A.Square)
                    # ---- product basis for this group (c-major; the a axis is
                    # (sub, a) so everything downstream is as wide as before).
                    # Split across DVE and the mostly-idle gpsimd engine.
                    AA4 = AA[:].rearrange("c (s m a) -> c s a m", s=4, m=16)
                    inv = wp.tile([128, 512], f32, tag="inv")
                    nc.vector.tensor_copy(inv[:, 0:128].rearrange("c (s a) -> c s a", s=4),
                                          AA4[:, :, :, 0])
                    nc.vector.reduce_sum(
                        inv[:, 128:256].rearrange("c (s a) -> c s a", s=4).unsqueeze(3),
                        AA4[:, :, :, 1:4], axis=mybir.AxisListType.X)
                    for l in (2, 3):
                        isl = inv[:, l * 128:(l + 1) * 128].rearrange("c (s a) -> c s a", s=4)
                        m0, wl = L_START[l], L_WIDTH[l]
                        nc.gpsimd.tensor_tensor(isl, AA4[:, :, :, m0], AA4[:, :, :, m0 + 1],
                                                op=mybir.AluOpType.add)
                        for mm in range(m0 + 2, m0 + wl):
                            nc.gpsimd.tensor_tensor(isl, isl, AA4[:, :, :, mm],
                                                    op=mybir.AluOpType.add)
                    acc = {}
                    for wnm, eng in (('w2T', nc.vector), ('w3T', nc.gpsimd)):
                        t2 = wp.tile([128, 512], f32, tag=f"t2{wnm}")
                        eng.tensor_tensor(
                            t2[:].rearrange("c (l a) -> c l a", l=4),
                            inv[:].rearrange("c (l a) -> c l a", l=4),
                            sb[f'{wnm}_{i}'][:].unsqueeze(2).broadcast_to([128, 4, 128]),
                            op=mybir.AluOpType.mult)
                        ac = wp.tile([128, 128], f32, tag=f"ac{wnm}")
                        if eng is nc.vector:
                            eng.reduce_sum(ac[:].unsqueeze(2),
                                           t2[:].rearrange("c (l a) -> c a l", l=4),
                                           axis=mybir.AxisListType.X)
                        else:
                            t23 = t2[:].rearrange("c (l a) -> c l a", l=4)
                            eng.tensor_tensor(ac[:], t23[:, 0, :], t23[:, 1, :],
                                              op=mybir.AluOpType.add)
                            eng.tensor_tensor(ac[:], ac[:], t23[:, 2, :],
                                              op=mybir.AluOpType.add)
                            eng.tensor_tensor(ac[:], ac[:], t23[:, 3, :],
                                              op=mybir.AluOpType.add)
                        acc[wnm] = ac
                    fe = wp.tile([128, 128], f32, tag="fe")
                    nc.vector.tensor_tensor(fe[:], scal[:], acc['w3T'][:],
                                            op=mybir.AluOpType.mult)
                    nc.vector.tensor_add(fe[:], fe[:], acc['w2T'][:])
                    nc.vector.tensor_tensor(feats_cm[:, k * 128:(k + 1) * 128], fe[:],
                                            scal[:], op=mybir.AluOpType.add)
                    pending_tail = make_tail(k)
                pending_tail()
                if i + 1 < L:
                    cin, cout = huL_next[:], huG_next[:]
                    if HU_FP8:
                        cin, cout = cin.bitcast(bf16), cout.bitcast(bf16)
                    nc.gpsimd.collective_compute(
                        "AllGather", mybir.AluOpType.bypass,
                        replica_groups=[list(range(BG))],
                        ins=[cin.opt()], outs=[cout.opt()])
            nc.sync.dma_start(en_out[:], en[:])
    nc.compile()
    return nc


def kernel(**inputs):
    from concourse import bass_utils
    in_maps, T_list, G4, e0 = host_prep(inputs)
    key = (tuple(T_list), G4)
    if key not in _CACHE:
        _CACHE[key] = build_kernel(T_list, G4)
    nc = _CACHE[key]
    res = bass_utils.run_bass_kernel_spmd(nc, in_maps, core_ids=list(range(BG)))
    energy = np.zeros(BG, np.float32)
    for b in range(BG):
        energy[b] = res.results[b]['en_out'].reshape(-1)[0] + e0[b]
    return energy


# revision 33
# speedup vs baseline: 1.0332x; 1.0332x over previous
"""MACE+Ewald forward on 8 Trainium2 NeuronCores.

Sharding: graph-per-core (8 graphs, 8 cores). Atoms balanced across 4 blocks
of <=128 slots (padded NL=512 per core); edges assigned to the core/block
owning their dst atom, packed into 128-edge tiles with per-block tile counts.

Key device-side structure per layer:
  1. Layer 0's gather table huG0 = (attrs@Wembed)@Wup_0 is weight-only and
     precomputed on the host, so only ONE AllGather remains (layer 1's hu,
     fp8 payload bitcast to bf16 for transport, Shared output). It is kicked
     from the previous layer's per-block tails and overlapped by the Ewald
     block plus an eager radial-MLP pipeline.
  2. Ewald: structure factors / he MLP, all bf16 matmuls.
  3. Radial MLP issued just-in-time one 4-tile group ahead of the edge loop
     (as a phase its matmul->silu chain latency would serialize).
  4. Edge loop: one batched indirect gather of hu rows per block; edge tiles
     are processed in PAIRS: the per-(channel,l) weights times gathered hu
     (mw, fp8) is the stationary operand and a host-precomputed segY matrix
     (one-hot dst scatter with spherical harmonics Y and 1/avg_nei folded
     in, fp8) is the moving operand of DoubleRow matmuls that contract 256
     edges per pass, c-major output so no transposes are needed afterwards.
     rW4 products are issued via a lead-2 FIFO so the DVE mw latency hides.
  5. Product basis (A^2 contractions) per block split across DVE/gpsimd,
     PSUM freed early through scalar-engine copies; h update, readout and
     the next layer's hu are deferred into the next block's tile stream.

All matmuls run bf16 or fp8 operands with fp32 PSUM accumulation (the
harness tolerance is 2e-2; measured error stays ~2.6e-3). Constants load
as three fused buffers (one DMA each) to cut HWDGE issue latency.
"""

import numpy as np
import ml_dtypes

C = 128
L = 2
NB = 8
NEL = 10
BG = 8
N_ATOMS = 3200
N_EDGES = 51200
R_MAX = 5.0
P_CUT = 5.0
AVG_NEI = 16.0
DELTA_K = 0.2
NKRBF = 128
DP = 8
SKIP = (2.0 + 1.0) ** -0.5
NL = 512            # padded atoms per core
NBLK = NL // 128    # atom blocks per core
KPAD = 128          # padded k-point count (real: 123)
LOFLM = np.repeat(np.arange(4), [1, 3, 5, 7])   # [16]
L_START = [0, 1, 4, 9]
L_WIDTH = [1, 3, 5, 7]
# scatter matmul chunks: (l, first lm, number of lm) with moving-free <= 512
CHUNKS = [(0, 0, 1), (1, 1, 3), (2, 4, 4), (2, 8, 1), (3, 9, 4), (3, 13, 3)]
SEGY_FP8 = True
HU_FP8 = True
SCAT_DR = True   # fp8 DoubleRow scatter (2 edge tiles per PE pass)      # layer>=1 hu AllGather + gather in fp8e4m3

_CACHE = {}


def _const_layouts(G4):
    """Constant packing: 3 fused SBUF-resident buffers loaded with one DMA
    each (HWDGE issue time for ~50 separate loads dominated kernel startup).
    cbA = layer-0 critical path, cbB = the rest, cf = fp32 smalls."""
    bfA = [('attrsT', NEL, NL), ('Wembed', NEL, C),
           ('rW1_0', NB, 64), ('rW2_0', 64, 64), ('rW3_0', 64, 64), ('rW4_0', 64, 4 * C),
           ('efTpack', 8, G4 * 128)]
    bfB = [('Wpre1_0', C, C), ('Wpre2_0', C, C), ('Wm1_0', C, C), ('Wm2_0', C, C),
           ('Wmix_0', C, C), ('Wup_1', C, C),
           ('cosdam', 128, NBLK * KPAD), ('sindam', 128, NBLK * KPAD),
           ('cosdkm', KPAD, NL), ('sindkm', KPAD, NL),
           ('ident', 128, 128), ('Wr0', C, 1), ('Wr1a', C, 16), ('Wr1b', 16, 1),
           ('Wpre1_1', C, C), ('Wpre2_1', C, C), ('Wm1_1', C, C), ('Wm2_1', C, C),
           ('Wmix_1', C, C),
           ('rW1_1', NB, 64), ('rW2_1', 64, 64), ('rW3_1', 64, 64), ('rW4_1', 64, 4 * C)]
    cf = ([('kfilt_0', KPAD, C), ('kfilt_1', KPAD, C)]
          + [(f'w{j}T_{i}', C, 4) for i in range(L) for j in (2, 3)]
          + [(f'{nm}_{i}', C, 1) for i in range(L) for nm in ('bpre1', 'bpre2', 'bm1', 'bm2')]
          + [(f'{nm}_{i}', 64, 1) for i in range(L) for nm in ('rb1', 'rb2', 'rb3')])
    return {'cbA': bfA, 'cbB': bfB, 'cf': cf}


def unpack_consts(m, G4):
    """Recover named f32 views from a core's fused const buffers (for host_sim)."""
    out = {}
    for buf, entries in _const_layouts(G4).items():
        c0 = 0
        for name, rows, cols in entries:
            out[name] = np.asarray(m[buf][0:rows, c0:c0 + cols], np.float32)
            c0 += cols
    return out


# ---------------------------------------------------------------- host math
def _sph_np(u):
    x, y, z = u[:, 0], u[:, 1], u[:, 2]
    s3, s5, s15 = 3.0 ** 0.5, 5.0 ** 0.5, 15.0 ** 0.5
    c70, c105, c42, c7 = 70.0 ** 0.5 / 4.0, 105.0 ** 0.5, 42.0 ** 0.5 / 4.0, 7.0 ** 0.5 / 2.0
    comps = [np.ones_like(x),
             s3 * x, s3 * y, s3 * z,
             s15 * x * y, s15 * y * z, 0.5 * s5 * (3 * z * z - 1.0), s15 * x * z,
             0.5 * s15 * (x * x - y * y),
             c70 * y * (3 * x * x - y * y), c105 * x * y * z, c42 * y * (5 * z * z - 1.0),
             c7 * z * (5 * z * z - 3.0), c42 * x * (5 * z * z - 1.0),
             0.5 * c105 * z * (x * x - y * y), c70 * x * (x * x - 3 * y * y)]
    return np.stack(comps, axis=-1).astype(np.float32)


def _radial_np(r):
    n = np.arange(1, NB + 1, dtype=np.float32)
    rb = np.sqrt(2.0 / R_MAX) * np.sin(n * np.pi * r[:, None] / R_MAX) / np.maximum(r, 1e-9)[:, None]
    uu = np.clip(r / R_MAX, 0.0, 1.0)
    p = P_CUT
    env = 1.0 - (p + 1.0) * (p + 2.0) / 2.0 * uu ** 5 + p * (p + 2.0) * uu ** 6 - p * (p + 1.0) / 2.0 * uu ** 7
    env = env * (r < R_MAX)
    return (rb * env[:, None]).astype(np.float32)


def host_prep(inputs):
    """Build per-core padded arrays. Returns (in_maps, T_list, G4, e0)."""
    f32 = np.float32
    bf16 = ml_dtypes.bfloat16
    segy_np = ml_dtypes.float8_e4m3 if SEGY_FP8 else bf16
    pos = np.asarray(inputs['positions'], f32)
    attrs = np.asarray(inputs['node_attrs'], f32)
    shifts = np.asarray(inputs['shifts'], f32)
    eidx = np.asarray(inputs['edge_index']).astype(np.int64)
    batch = np.asarray(inputs['batch']).astype(np.int64)
    kgrid = np.asarray(inputs['kgrid'], f32)
    krbf = np.asarray(inputs['krbf'], f32)
    K = kgrid.shape[0]

    # per-graph contiguous atom ranges (batch is sorted)
    starts = np.searchsorted(batch, np.arange(BG))
    ends = np.searchsorted(batch, np.arange(BG), side='right')
    counts = ends - starts
    assert counts.max() <= NL, counts

    # balanced split of each graph's atoms into NBLK blocks of <=128 slots
    slot = np.zeros(N_ATOMS, np.int64)          # padded local slot per atom
    for b in range(BG):
        n = int(counts[b])
        base, rem = divmod(n, NBLK)
        sizes = [base + (k < rem) for k in range(NBLK)]
        assert max(sizes) <= 128
        cum = 0
        for k in range(NBLK):
            j = np.arange(cum, cum + sizes[k])
            slot[starts[b] + j] = k * 128 + (j - cum)
            cum += sizes[k]
    pid = (batch * NL + slot).astype(np.int32)  # padded global id [N]

    # ---- edge geometry (host) ----
    src, dst = eidx[0], eidx[1]
    vec = pos[dst] - pos[src] + shifts
    r = np.linalg.norm(vec.astype(np.float64), axis=1).astype(f32)
    uvec = vec / np.maximum(r, 1e-9)[:, None]
    Y = _sph_np(uvec)                           # [E,16]
    ef = _radial_np(r)                          # [E,8]

    # ---- Ewald geometry (host) ----
    dot = pos @ kgrid.T                         # [N,K]
    sd = np.prod(np.sinc(0.5 * DELTA_K * pos), axis=1).astype(f32)   # [N]
    cosd = (sd[:, None] * np.cos(dot)).astype(f32)
    sind = (sd[:, None] * np.sin(dot)).astype(f32)

    kdown = krbf @ np.asarray(inputs['Wdown'], f32)      # [K,DP]

    # ---- edge -> (core, block) assignment, per-block tile counts ----
    gdst = batch[dst]
    kblk = slot[dst] // 128
    ecount = np.zeros((BG, NBLK), np.int64)
    np.add.at(ecount, (gdst, kblk), 1)
    T_list = [max(1, int(np.ceil(ecount[:, k].max() / 128))) for k in range(NBLK)]
    if SCAT_DR:
        T_list = [t + (t % 2) for t in T_list]
    O_list = np.concatenate([[0], np.cumsum(T_list)]).astype(int)
    NT = int(O_list[-1])
    G4 = ((NT + 3) // 4) * 4

    # ---- shared (replicated) weight arrays ----
    g = lambda k: np.asarray(inputs[k], f32)
    shared = {'Wembed': g('W_embed'),
              'ident': np.eye(128, dtype=f32),
              'Wr0': g('Wr0'), 'Wr1a': g('Wr1a'), 'Wr1b': g('Wr1b')}
    # layer-0 hu is weight-only (h0 = attrs @ Wembed): precompute the full
    # gathered table on the host, killing the first AllGather.
    h0_full = attrs @ g('W_embed')                       # [N, C]
    huG0 = np.zeros((BG * NL, C), f32)
    huG0[pid] = h0_full @ g('Wup')[0]
    huG0 = huG0.astype(bf16)
    for i in range(L):
        for nm in ('Wpre1', 'Wpre2', 'Wm1', 'Wm2', 'Wup', 'Wmix'):
            shared[f'{nm}_{i}'] = g(nm)[i]
        shared[f'rW1_{i}'] = g('rW1')[i]
        shared[f'rW2_{i}'] = g('rW2')[i]
        shared[f'rW3_{i}'] = g('rW3')[i]
        # rW4 reshaped l-major: [64, l*128 + c]
        shared[f'rW4_{i}'] = g('rW4')[i].reshape(64, C, 4).transpose(0, 2, 1).reshape(64, 4 * C)
        for nm in ('bpre1', 'bpre2', 'bm1', 'bm2'):
            shared[f'{nm}_{i}'] = g(nm)[i].reshape(C, 1)
        for nm in ('rb1', 'rb2', 'rb3'):
            shared[f'{nm}_{i}'] = g(nm)[i].reshape(64, 1)
        kf = np.zeros((KPAD, C), f32)
        kf[:K] = 0.01 * (kdown @ g('WupE')[i])
        shared[f'kfilt_{i}'] = kf
        shared[f'w2T_{i}'] = g('w2')[i].T.copy()             # [C,4] f32
        shared[f'w3T_{i}'] = g('w3')[i].T.copy()

    layouts = _const_layouts(G4)

    # ---- per-core arrays ----
    in_maps = []
    for b in range(BG):
        sl = slice(starts[b], ends[b])
        per = {}
        slot_b = slot[sl]
        at = np.zeros((NEL, NL), f32)
        at[:, slot_b] = attrs[sl].T
        per['attrsT'] = at
        cam = np.zeros((128, NBLK * KPAD), f32)   # atom-major cosd, per block
        sam = np.zeros((128, NBLK * KPAD), f32)
        ckm = np.zeros((KPAD, NL), f32)           # k-major
        skm = np.zeros((KPAD, NL), f32)
        pr, bb = slot_b % 128, slot_b // 128
        cam.reshape(128, NBLK, KPAD)[pr, bb, :K] = cosd[sl]
        sam.reshape(128, NBLK, KPAD)[pr, bb, :K] = sind[sl]
        ckm[:K, slot_b] = cosd[sl].T
        skm[:K, slot_b] = sind[sl].T
        per['cosdam'], per['sindam'] = cam, sam
        per['cosdkm'], per['sindkm'] = ckm, skm

        efp = np.zeros((8, G4 * 128), f32)
        sip = np.zeros((128, NT), np.int32)
        segY = np.zeros((128, NT * 16 * 128), f32)
        emask = gdst == b
        for k in range(NBLK):
            es = np.nonzero(emask & (kblk == k))[0]
            es = es[np.argsort(slot[dst[es]], kind='stable')]
            s = np.arange(len(es))
            tt, p = s // 128, s % 128
            t = O_list[k] + tt
            efp[:, t * 128 + p] = ef[es].T
            sip[p, t] = pid[src[es]]
            a = slot[dst[es]] - k * 128
            base = t * 2048 + a
            for lm in range(16):
                segY[p, base + lm * 128] = Y[es, lm] / AVG_NEI
        per['efTpack'] = efp

        def pack(entries, np_dt):
            width = sum(e[2] for e in entries)
            arr = np.zeros((128, width), np_dt)
            c0 = 0
            for name, rows, cols in entries:
                src_a = per.get(name, shared.get(name))
                arr[0:rows, c0:c0 + cols] = src_a
                c0 += cols
            return arr

        m = {'srcidx': sip, 'segYpack': segY.astype(segy_np), 'huG0': huG0,
             'cbA': pack(layouts['cbA'], bf16), 'cbB': pack(layouts['cbB'], bf16),
             'cf': pack(layouts['cf'], f32)}
        in_maps.append(m)

    e0 = np.zeros(BG, f32)
    ae = attrs @ np.asarray(inputs['atomic_E'], f32)
    for b in range(BG):
        e0[b] = ae[starts[b]:ends[b]].sum()
    return in_maps, T_list, G4, e0


# ---------------------------------------------------------------- device
def build_kernel(T_list, G4):
    import concourse.bass as bass
    import concourse.bacc as bacc
    import concourse.mybir as mybir
    import concourse.tile as tile

    f32 = mybir.dt.float32
    bf16 = mybir.dt.bfloat16
    sdt = mybir.dt.float8e4 if SEGY_FP8 else bf16
    A = mybir.ActivationFunctionType
    NT = int(sum(T_list))
    Tmax = max(T_list)
    O_list = np.concatenate([[0], np.cumsum(T_list)]).astype(int)
    nc = bacc.Bacc("TRN2", target_bir_lowering=False, debug=False, num_devices=BG)

    dins = {}
    def din(name, shape, dt=f32):
        dins[name] = nc.dram_tensor(name, list(shape), dt, kind="ExternalInput").ap()
        return dins[name]

    # load order = SP queue order: the layer-0 critical path first
    layouts = _const_layouts(G4)
    widths = {buf: sum(e[2] for e in entries) for buf, entries in layouts.items()}
    din('srcidx', (128, NT), mybir.dt.int32)
    din('cbA', (128, widths['cbA']), bf16)
    din('cf', (128, widths['cf']))
    din('cbB', (128, widths['cbB']), bf16)
    segY_d = din('segYpack', (128, NT * 2048), sdt)
    huG0_d = din('huG0', (BG * NL, C), bf16)
    en_out = nc.dram_tensor('en_out', [1, 1], f32, kind="ExternalOutput").ap()

    with tile.TileContext(nc) as tc:
        with (
            tc.tile_pool(name="const", bufs=1) as cp,
            tc.tile_pool(name="work", bufs=2) as wp,
            tc.tile_pool(name="segy", bufs=3) as sy,
            tc.tile_pool(name="big", bufs=1) as bp,
            tc.tile_pool(name="psA", bufs=1, space="PSUM") as psA,
            tc.tile_pool(name="psS", bufs=2, space="PSUM") as psS,
            tc.tile_pool(name="psW", bufs=2, space="PSUM") as psW,
            tc.tile_pool(name="dram", bufs=1, space="DRAM") as dp,
        ):
            sb = {}
            for name in ('srcidx', 'cbA', 'cf', 'cbB'):
                ap = dins[name]
                t = cp.tile(list(ap.shape), ap.dtype, tag=name)
                nc.sync.dma_start(t[:], ap[:])
                if name == 'srcidx':
                    sb[name] = t
                else:
                    c0 = 0
                    for nm, rows, cols in layouts[name]:
                        sb[nm] = t[0:rows, c0:c0 + cols]
                        c0 += cols

            h = bp.tile([C, NL], bf16, tag="h")
            en = bp.tile([1, 1], f32, tag="en")
            feats_cm = bp.tile([C, NL], bf16, tag="feats_cm")
            hres_am = bp.tile([128, NBLK * 128], bf16, tag="hres_am")
            nc.vector.memset(en[:], 0.0)

            pe = psW.tile([C, NL], f32, tag="pb")
            nc.tensor.matmul(pe[:], sb['Wembed'][:], sb['attrsT'][:], start=True, stop=True)
            nc.scalar.activation(h[:], pe[:], A.Copy)

            hu_dt = mybir.dt.float8e4 if HU_FP8 else bf16
            coll = {}     # layer -> (huL, huG) for layers >= 1
            for i in range(L):
                # ---- gather source: host table (layer 0) or prior AllGather ----
                huG = huG0_d if i == 0 else coll[i][1][:]
                hugs = []
                for k in range(NBLK):
                    lo, hi = int(O_list[k]), int(O_list[k + 1])
                    hg = wp.tile([128, Tmax * 128], bf16 if i == 0 else hu_dt,
                                 tag=f"hug{k % 2}{i}", bufs=1)
                    nc.gpsimd.indirect_dma_start(
                        out=hg[:, 0:(hi - lo) * 128], out_offset=None, in_=huG[:],
                        in_offset=bass.IndirectOffsetOnAxis(
                            ap=sb['srcidx'][:, lo:hi], axis=0))
                    hugs.append(hg)
                if i + 1 < L:
                    huL_next = dp.tile([NL, C], hu_dt, tag=f"huL{i + 1}")
                    huG_next = dp.tile([BG * NL, C], hu_dt, tag=f"huG{i + 1}",
                                       addr_space="Shared")
                    hu_am = wp.tile([128, NL], hu_dt, tag="hu_am")
                    coll[i + 1] = (huL_next, huG_next)

                # ---- radial MLP: issued just-in-time, one 4-tile group ahead
                # of the edge loop (the matmul->silu chain is ~3us latency and
                # would serialize as a phase; interleaved it hides behind the
                # per-tile scatter work).
                s3sb = wp.tile([64, G4 * 128], bf16, tag="s3sb")
                radial_next = [0]

                def radial_group(gidx, i=i, s3sb=s3sb):
                    gsl = slice(gidx * 512, (gidx + 1) * 512)
                    pr1 = psS.tile([128, 512], f32, tag="ps")
                    nc.tensor.matmul(pr1[0:64, :], sb[f'rW1_{i}'][:], sb['efTpack'][:, gsl],
                                     start=True, stop=True)
                    s1 = wp.tile([64, 512], bf16, tag="s1")
                    nc.scalar.activation(s1[:], pr1[0:64, :], A.Silu, bias=sb[f'rb1_{i}'][:])
                    pr2 = psS.tile([128, 512], f32, tag="ps")
                    nc.tensor.matmul(pr2[0:64, :], sb[f'rW2_{i}'][:], s1[:], start=True, stop=True)
                    s2 = wp.tile([64, 512], bf16, tag="s1")
                    nc.scalar.activation(s2[:], pr2[0:64, :], A.Silu, bias=sb[f'rb2_{i}'][:])
                    pr3 = psS.tile([128, 512], f32, tag="ps")
                    nc.tensor.matmul(pr3[0:64, :], sb[f'rW3_{i}'][:], s2[:], start=True, stop=True)
                    nc.scalar.activation(s3sb[:, gsl], pr3[0:64, :], A.Silu, bias=sb[f'rb3_{i}'][:])

                def ensure_radial(gwant):
                    while radial_next[0] <= min(gwant, G4 // 4 - 1):
                        radial_group(radial_next[0])
                        radial_next[0] += 1

                ensure_radial(1)
                # ---- Ewald block (independent of the collective) ----
                p1 = psW.tile([C, NL], f32, tag="pb")
                nc.tensor.matmul(p1[:], sb[f'Wpre1_{i}'][:], h[:], start=True, stop=True)
                t1 = wp.tile([C, NL], bf16, tag="t1")
                nc.scalar.activation(t1[:], p1[:], A.Silu, bias=sb[f'bpre1_{i}'][:])
                p2 = psW.tile([C, NL], f32, tag="pb")
                nc.tensor.matmul(p2[:], sb[f'Wpre2_{i}'][:], t1[:], start=True, stop=True)
                hres = wp.tile([C, NL], bf16, tag="hres")
                nc.vector.tensor_scalar_add(hres[:], p2[:], sb[f'bpre2_{i}'][:])
                nc.vector.tensor_add(hres[:], hres[:], h[:])
                for k in range(NBLK):
                    pt = psS.tile([128, 512], f32, tag="ps")
                    ptb = pt[:].bitcast(bf16)[:, 0:128]
                    nc.tensor.transpose(ptb, hres[:, k * 128:(k + 1) * 128], sb['ident'][:])
                    nc.scalar.activation(hres_am[:, k * 128:(k + 1) * 128], ptb, A.Copy)
                sfk = {}
                for nm, am in (('r', 'cosdam'), ('i', 'sindam')):
                    psf = psS.tile([128, 512], f32, tag="ps")
                    for k in range(NBLK):
                        nc.tensor.matmul(psf[:, 0:128], sb[am][:, k * KPAD:(k + 1) * KPAD],
                                         hres_am[:, k * 128:(k + 1) * 128],
                                         start=(k == 0), stop=(k == NBLK - 1))
                    s = wp.tile([KPAD, C], bf16, tag=f"sfk{nm}")
                    nc.vector.tensor_tensor(s[:], psf[:, 0:128], sb[f'kfilt_{i}'][:],
                                            op=mybir.AluOpType.mult)
                    sfk[nm] = s
                phe = psW.tile([C, NL], f32, tag="pb")
                nc.tensor.matmul(phe[:], sfk['r'][:], sb['cosdkm'][:], start=True, stop=False)
                nc.tensor.matmul(phe[:], sfk['i'][:], sb['sindkm'][:], start=False, stop=True)
                he0 = wp.tile([C, NL], bf16, tag="he0")
                nc.scalar.activation(he0[:], phe[:], A.Copy)
                pm1 = psW.tile([C, NL], f32, tag="pb")
                nc.tensor.matmul(pm1[:], sb[f'Wm1_{i}'][:], he0[:], start=True, stop=True)
                tm = wp.tile([C, NL], bf16, tag="t1")
                nc.scalar.activation(tm[:], pm1[:], A.Silu, bias=sb[f'bm1_{i}'][:])
                pm2 = psW.tile([C, NL], f32, tag="pb")
                nc.tensor.matmul(pm2[:], sb[f'Wm2_{i}'][:], tm[:], start=True, stop=True)
                he2 = wp.tile([C, NL], bf16, tag="he2")
                nc.scalar.activation(he2[:], pm2[:], A.Silu, bias=sb[f'bm2_{i}'][:])
                if i > 0:
                    # PE is otherwise idle while the AllGather is in flight:
                    # run the whole radial MLP pipeline under it.
                    ensure_radial(G4 // 4 - 1)

                # ---- edge loop ----
                def issue_pair(k, tt0, alt, i=i, hugs=hugs):
                    # one 2-tile unit: paired segY DMA, two rW4 matmuls, two mw
                    # products written fp8 into one [128,1024] tile for the
                    # DoubleRow scatter. Second mw alternates DVE/gpsimd.
                    t = int(O_list[k]) + tt0
                    ensure_radial((t + 1) // 4 + 1)
                    sgt = sy.tile([128, 4096], sdt, tag="sg")
                    nc.sync.dma_start(sgt[:], segY_d[:, t * 2048:(t + 2) * 2048])
                    mw = wp.tile([128, 1024], sdt, tag="mw")
                    for j in range(2):
                        pw = psW.tile([C, NL], f32, tag="pb")
                        nc.tensor.matmul(pw[:], s3sb[:, (t + j) * 128:(t + j + 1) * 128],
                                         sb[f'rW4_{i}'][:], start=True, stop=True)
                        nc.vector.tensor_tensor(
                            mw[:, j * 512:(j + 1) * 512].rearrange("p (l c) -> p l c", l=4),
                            pw[:].rearrange("p (l c) -> p l c", l=4),
                            hugs[k][:, (tt0 + j) * 128:(tt0 + j + 1) * 128]
                                .unsqueeze(1).broadcast_to([128, 4, 128]),
                            op=mybir.AluOpType.mult)
                    return mw, sgt

                PAIRS = [(k, tt0) for k in range(NBLK) for tt0 in range(0, int(T_list[k]), 2)]
                LEAD = 1
                fifo = [issue_pair(*PAIRS[j], alt=(j % 2 == 0)) for j in range(LEAD)]
                tidx = [0]

                def next_mw_sgt():
                    j = tidx[0]
                    if j + LEAD < len(PAIRS):
                        fifo.append(issue_pair(*PAIRS[j + LEAD], alt=(j % 2 == 0)))
                    tidx[0] += 1
                    return fifo.pop(0)

                def make_tail(k, i=i, he2=he2):
                    # layer tail for block k: h update, readout, next layer's hu.
                    # Deferred into the next block's tile stream so the product
                    # basis (DVE) overlaps the next block's scatters (PE).
                    def tail():
                        blk = slice(k * 128, (k + 1) * 128)
                        pmx = psW.tile([C, NL], f32, tag="pb")
                        nc.tensor.matmul(pmx[:, 0:128], sb[f'Wmix_{i}'][:],
                                         feats_cm[:, blk], start=True, stop=True)
                        hnk = wp.tile([C, 128], f32, tag="hn")
                        nc.vector.tensor_add(hnk[:], pmx[:, 0:128], he2[:, blk])
                        nc.vector.tensor_add(hnk[:], hnk[:], h[:, blk])
                        nc.vector.tensor_scalar_mul(h[:, blk], hnk[:], float(SKIP))
                        if i == 0:
                            prd = psS.tile([128, 512], f32, tag="ps")
                            nc.tensor.matmul(prd[0:1, 0:128], sb['Wr0'][:], h[:, blk],
                                             start=True, stop=True)
                            rs = wp.tile([1, 1], f32, tag="rs")
                            nc.vector.reduce_sum(rs[:], prd[0:1, 0:128],
                                                 axis=mybir.AxisListType.X)
                            nc.vector.tensor_add(en[:], en[:], rs[:])
                        else:
                            pra = psS.tile([128, 512], f32, tag="ps")
                            nc.tensor.matmul(pra[0:16, 0:128], sb['Wr1a'][:], h[:, blk],
                                             start=True, stop=True)
                            ta = wp.tile([16, 128], bf16, tag="ta")
                            nc.scalar.activation(ta[:], pra[0:16, 0:128], A.Silu)
                            prb = psS.tile([128, 512], f32, tag="ps")
                            nc.tensor.matmul(prb[0:1, 0:128], sb['Wr1b'][:], ta[:],
                                             start=True, stop=True)
                            rs = wp.tile([1, 1], f32, tag="rs")
                            nc.vector.reduce_sum(rs[:], prb[0:1, 0:128],
                                                 axis=mybir.AxisListType.X)
                            nc.vector.tensor_add(en[:], en[:], rs[:])
                        if i + 1 < L:
                            ph = psW.tile([C, NL], f32, tag="pb")
                            nc.tensor.matmul(ph[:, 0:128], h[:, blk],
                                             sb[f'Wup_{i + 1}'][:], start=True, stop=True)
                            nc.scalar.activation(hu_am[:, blk], ph[:, 0:128], A.Copy)
                            nc.sync.dma_start(huL_next[k * 128:(k + 1) * 128, :],
                                              hu_am[:, blk])
                    return tail

                pending_tail = None
                for k in range(NBLK):
                    Tk = int(T_list[k])
                    pA1 = psA.tile([128, 1024], f32, tag="pA1")
                    pA2 = psA.tile([128, 1024], f32, tag="pA2")
                    for pp in range(Tk // 2):
                        mw, sgt = next_mw_sgt()
                        mw3 = mw[:].rearrange("p (two x) -> p two x", two=2)
                        sg3 = sgt[:].rearrange("p (two x) -> p two x", two=2)
                        for (l, m0, w) in CHUNKS:
                            pAh, off = (pA1, m0) if m0 < 8 else (pA2, m0 - 8)
                            nc.tensor.matmul(pAh[:, off * 128:(off + w) * 128],
                                             mw3[:, :, l * 128:(l + 1) * 128],
                                             sg3[:, :, m0 * 128:(m0 + w) * 128],
                                             start=(pp == 0), stop=(pp == Tk // 2 - 1),
                                             perf_mode=mybir.MatmulPerfMode.DoubleRow)
                        if pending_tail is not None and pp == min(1, Tk // 2 - 1):
                            pending_tail()
                            pending_tail = None
                    # free pA early: scal copy + A^2 on the scalar engine, then
                    # the DVE product basis reads SBUF only.
                    scal = wp.tile([128, 128], bf16, tag="scal")
                    nc.scalar.activation(scal[:], pA1[:, 0:128], A.Copy)
                    AA = wp.tile([128, 2048], bf16, tag="AA")
                    nc.scalar.activation(AA[:, 0:1024], pA1[:], A.Square)
                    nc.scalar.activation(AA[:, 1024:2048], pA2[:], A.Square)
                    # ---- product basis for this block (c-major throughout).
                    # Split across DVE and the mostly-idle gpsimd engine so the
                    # per-tile mw ops on DVE don't stall at block boundaries.
                    AA3 = AA[:].rearrange("c (m a) -> c a m", m=16)
                    inv = wp.tile([128, 512], f32, tag="inv")
                    nc.vector.tensor_copy(inv[:, 0:128], AA3[:, :, 0])
                    nc.vector.reduce_sum(inv[:, 128:256].unsqueeze(2), AA3[:, :, 1:4],
                                         axis=mybir.AxisListType.X)
                    for l in (2, 3):
                        isl = inv[:, l * 128:(l + 1) * 128]
                        m0, wl = L_START[l], L_WIDTH[l]
                        nc.gpsimd.tensor_tensor(isl, AA3[:, :, m0], AA3[:, :, m0 + 1],
                                                op=mybir.AluOpType.add)
                        for mm in range(m0 + 2, m0 + wl):
                            nc.gpsimd.tensor_tensor(isl, isl, AA3[:, :, mm],
                                                    op=mybir.AluOpType.add)
                    acc = {}
                    for wnm, eng in (('w2T', nc.vector), ('w3T', nc.gpsimd)):
                        t2 = wp.tile([128, 512], f32, tag=f"t2{wnm}")
                        eng.tensor_tensor(
                            t2[:].rearrange("c (l a) -> c l a", l=4),
                            inv[:].rearrange("c (l a) -> c l a", l=4),
                            sb[f'{wnm}_{i}'][:].unsqueeze(2).broadcast_to([128, 4, 128]),
                            op=mybir.AluOpType.mult)
                        ac = wp.tile([128, 128], f32, tag=f"ac{wnm}")
                        if eng is nc.vector:
                            eng.reduce_sum(ac[:].unsqueeze(2),
                                           t2[:].rearrange("c (l a) -> c a l", l=4),
                                           axis=mybir.AxisListType.X)
                        else:
                            t23 = t2[:].rearrange("c (l a) -> c l a", l=4)
                            eng.tensor_tensor(ac[:], t23[:, 0, :], t23[:, 1, :],
                                              op=mybir.AluOpType.add)
                            eng.tensor_tensor(ac[:], ac[:], t23[:, 2, :],
                                              op=mybir.AluOpType.add)
                            eng.tensor_tensor(ac[:], ac[:], t23[:, 3, :],
                                              op=mybir.AluOpType.add)
                        acc[wnm] = ac
                    fe = wp.tile([128, 128], f32, tag="fe")
                    nc.vector.tensor_tensor(fe[:], scal[:], acc['w3T'][:],
                                            op=mybir.AluOpType.mult)
                    nc.vector.tensor_add(fe[:], fe[:], acc['w2T'][:])
                    nc.vector.tensor_tensor(feats_cm[:, k * 128:(k + 1) * 128], fe[:],
                                            scal[:], op=mybir.AluOpType.add)
                    pending_tail = make_tail(k)
                pending_tail()
                if i + 1 < L:
                    cin, cout = huL_next[:], huG_next[:]
                    if HU_FP8:
                        cin, cout = cin.bitcast(bf16), cout.bitcast(bf16)
                    nc.gpsimd.collective_compute(
                        "AllGather", mybir.AluOpType.bypass,
                        replica_groups=[list(range(BG))],
                        ins=[cin.opt()], outs=[cout.opt()])
            nc.sync.dma_start(en_out[:], en[:])
    nc.compile()
    return nc


def kernel(**inputs):
    from concourse import bass_utils
    in_maps, T_list, G4, e0 = host_prep(inputs)
    key = (tuple(T_list), G4)
    if key not in _CACHE:
        _CACHE[key] = build_kernel(T_list, G4)
    nc = _CACHE[key]
    res = bass_utils.run_bass_kernel_spmd(nc, in_maps, core_ids=list(range(BG)))
    energy = np.zeros(BG, np.float32)
    for b in range(BG):
        energy[b] = res.results[b]['en_out'].reshape(-1)[0] + e0[b]
    return energy


# revision 34
# speedup vs baseline: 1.0694x; 1.0349x over previous
"""MACE+Ewald forward on 8 Trainium2 NeuronCores.

Sharding: graph-per-core (8 graphs, 8 cores). Atoms balanced across 4 blocks
of <=128 slots (padded NL=512 per core); edges assigned to the core/block
owning their dst atom, packed into 128-edge tiles with per-block tile counts.

Key device-side structure per layer:
  1. Layer 0's gather table huG0 = (attrs@Wembed)@Wup_0 is weight-only and
     precomputed on the host, so only ONE AllGather remains (layer 1's hu,
     fp8 payload bitcast to bf16 for transport, Shared output). It is kicked
     from the previous layer's per-block tails and overlapped by the Ewald
     block plus an eager radial-MLP pipeline.
  2. Ewald: structure factors / he MLP, all bf16 matmuls.
  3. Radial MLP issued just-in-time one 4-tile group ahead of the edge loop
     (as a phase its matmul->silu chain latency would serialize).
  4. Edge loop: one batched indirect gather of hu rows per block; edge tiles
     are processed in PAIRS: the per-(channel,l) weights times gathered hu
     (mw, fp8) is the stationary operand and a host-precomputed segY matrix
     (one-hot dst scatter with spherical harmonics Y and 1/avg_nei folded
     in, fp8) is the moving operand of DoubleRow matmuls that contract 256
     edges per pass, c-major output so no transposes are needed afterwards.
     rW4 products are issued via a lead-2 FIFO so the DVE mw latency hides.
  5. Product basis (A^2 contractions) per block split across DVE/gpsimd,
     PSUM freed early through scalar-engine copies; h update, readout and
     the next layer's hu are deferred into the next block's tile stream.

All matmuls run bf16 or fp8 operands with fp32 PSUM accumulation (the
harness tolerance is 2e-2; measured error stays ~2.6e-3). Constants load
as three fused buffers (one DMA each) to cut HWDGE issue latency.
"""

import numpy as np
import ml_dtypes

C = 128
L = 2
NB = 8
NEL = 10
BG = 8
N_ATOMS = 3200
N_EDGES = 51200
R_MAX = 5.0
P_CUT = 5.0
AVG_NEI = 16.0
DELTA_K = 0.2
NKRBF = 128
DP = 8
SKIP = (2.0 + 1.0) ** -0.5
NL = 512            # padded atoms per core
NBLK = NL // 128    # atom blocks per core
KPAD = 128          # padded k-point count (real: 123)
LOFLM = np.repeat(np.arange(4), [1, 3, 5, 7])   # [16]
L_START = [0, 1, 4, 9]
L_WIDTH = [1, 3, 5, 7]
# scatter matmul chunks: (l, first lm, number of lm) with moving-free <= 512
CHUNKS = [(0, 0, 1), (1, 1, 3), (2, 4, 4), (2, 8, 1), (3, 9, 4), (3, 13, 3)]
SEGY_FP8 = True
HU_FP8 = True
SCAT_DR = True   # fp8 DoubleRow scatter (2 edge tiles per PE pass)      # layer>=1 hu AllGather + gather in fp8e4m3

_CACHE = {}


def _const_layouts(G4):
    """Constant packing: 3 fused SBUF-resident buffers loaded with one DMA
    each (HWDGE issue time for ~50 separate loads dominated kernel startup).
    cbA = layer-0 critical path, cbB = the rest, cf = fp32 smalls."""
    bfA = [('attrsT', NEL, NL), ('Wembed', NEL, C),
           ('rW1_0', NB, 64), ('rW2_0', 64, 64), ('rW3_0', 64, 64), ('rW4_0', 64, 4 * C),
           ('efTpack', 8, G4 * 128)]
    bfB = [('Wpre1_0', C, C), ('Wpre2_0', C, C), ('Wm1_0', C, C), ('Wm2_0', C, C),
           ('Wmix_0', C, C), ('Wup_1', C, C),
           ('cosdam', 128, NBLK * KPAD), ('sindam', 128, NBLK * KPAD),
           ('cosdkm', KPAD, NL), ('sindkm', KPAD, NL),
           ('ident', 128, 128), ('Wr0', C, 1), ('Wr1a', C, 16), ('Wr1b', 16, 1),
           ('Wpre1_1', C, C), ('Wpre2_1', C, C), ('Wm1_1', C, C), ('Wm2_1', C, C),
           ('Wmix_1', C, C),
           ('rW1_1', NB, 64), ('rW2_1', 64, 64), ('rW3_1', 64, 64), ('rW4_1', 64, 4 * C)]
    cf = ([('kfilt_0', KPAD, C), ('kfilt_1', KPAD, C)]
          + [(f'w{j}T_{i}', C, 4) for i in range(L) for j in (2, 3)]
          + [(f'{nm}_{i}', C, 1) for i in range(L) for nm in ('bpre1', 'bpre2', 'bm1', 'bm2')]
          + [(f'{nm}_{i}', 64, 1) for i in range(L) for nm in ('rb1', 'rb2', 'rb3')])
    return {'cbA': bfA, 'cbB': bfB, 'cf': cf}


def unpack_consts(m, G4):
    """Recover named f32 views from a core's fused const buffers (for host_sim)."""
    out = {}
    for buf, entries in _const_layouts(G4).items():
        c0 = 0
        for name, rows, cols in entries:
            out[name] = np.asarray(m[buf][0:rows, c0:c0 + cols], np.float32)
            c0 += cols
    return out


# ---------------------------------------------------------------- host math
def _sph_np(u):
    x, y, z = u[:, 0], u[:, 1], u[:, 2]
    s3, s5, s15 = 3.0 ** 0.5, 5.0 ** 0.5, 15.0 ** 0.5
    c70, c105, c42, c7 = 70.0 ** 0.5 / 4.0, 105.0 ** 0.5, 42.0 ** 0.5 / 4.0, 7.0 ** 0.5 / 2.0
    comps = [np.ones_like(x),
             s3 * x, s3 * y, s3 * z,
             s15 * x * y, s15 * y * z, 0.5 * s5 * (3 * z * z - 1.0), s15 * x * z,
             0.5 * s15 * (x * x - y * y),
             c70 * y * (3 * x * x - y * y), c105 * x * y * z, c42 * y * (5 * z * z - 1.0),
             c7 * z * (5 * z * z - 3.0), c42 * x * (5 * z * z - 1.0),
             0.5 * c105 * z * (x * x - y * y), c70 * x * (x * x - 3 * y * y)]
    return np.stack(comps, axis=-1).astype(np.float32)


def _radial_np(r):
    n = np.arange(1, NB + 1, dtype=np.float32)
    rb = np.sqrt(2.0 / R_MAX) * np.sin(n * np.pi * r[:, None] / R_MAX) / np.maximum(r, 1e-9)[:, None]
    uu = np.clip(r / R_MAX, 0.0, 1.0)
    p = P_CUT
    env = 1.0 - (p + 1.0) * (p + 2.0) / 2.0 * uu ** 5 + p * (p + 2.0) * uu ** 6 - p * (p + 1.0) / 2.0 * uu ** 7
    env = env * (r < R_MAX)
    return (rb * env[:, None]).astype(np.float32)


def host_prep(inputs):
    """Build per-core padded arrays. Returns (in_maps, T_list, G4, e0)."""
    f32 = np.float32
    bf16 = ml_dtypes.bfloat16
    segy_np = ml_dtypes.float8_e4m3 if SEGY_FP8 else bf16
    pos = np.asarray(inputs['positions'], f32)
    attrs = np.asarray(inputs['node_attrs'], f32)
    shifts = np.asarray(inputs['shifts'], f32)
    eidx = np.asarray(inputs['edge_index']).astype(np.int64)
    batch = np.asarray(inputs['batch']).astype(np.int64)
    kgrid = np.asarray(inputs['kgrid'], f32)
    krbf = np.asarray(inputs['krbf'], f32)
    K = kgrid.shape[0]

    # per-graph contiguous atom ranges (batch is sorted)
    starts = np.searchsorted(batch, np.arange(BG))
    ends = np.searchsorted(batch, np.arange(BG), side='right')
    counts = ends - starts
    assert counts.max() <= NL, counts

    # balanced split of each graph's atoms into NBLK blocks of <=128 slots
    slot = np.zeros(N_ATOMS, np.int64)          # padded local slot per atom
    for b in range(BG):
        n = int(counts[b])
        base, rem = divmod(n, NBLK)
        sizes = [base + (k < rem) for k in range(NBLK)]
        assert max(sizes) <= 128
        cum = 0
        for k in range(NBLK):
            j = np.arange(cum, cum + sizes[k])
            slot[starts[b] + j] = k * 128 + (j - cum)
            cum += sizes[k]
    pid = (batch * NL + slot).astype(np.int32)  # padded global id [N]

    # ---- edge geometry (host) ----
    src, dst = eidx[0], eidx[1]
    vec = pos[dst] - pos[src] + shifts
    r = np.linalg.norm(vec.astype(np.float64), axis=1).astype(f32)
    uvec = vec / np.maximum(r, 1e-9)[:, None]
    Y = _sph_np(uvec)                           # [E,16]
    ef = _radial_np(r)                          # [E,8]

    # ---- Ewald geometry (host) ----
    dot = pos @ kgrid.T                         # [N,K]
    sd = np.prod(np.sinc(0.5 * DELTA_K * pos), axis=1).astype(f32)   # [N]
    cosd = (sd[:, None] * np.cos(dot)).astype(f32)
    sind = (sd[:, None] * np.sin(dot)).astype(f32)

    kdown = krbf @ np.asarray(inputs['Wdown'], f32)      # [K,DP]

    # ---- edge -> (core, block) assignment, per-block tile counts ----
    gdst = batch[dst]
    kblk = slot[dst] // 128
    ecount = np.zeros((BG, NBLK), np.int64)
    np.add.at(ecount, (gdst, kblk), 1)
    T_list = [max(1, int(np.ceil(ecount[:, k].max() / 128))) for k in range(NBLK)]
    if SCAT_DR:
        T_list = [t + (t % 2) for t in T_list]
    O_list = np.concatenate([[0], np.cumsum(T_list)]).astype(int)
    NT = int(O_list[-1])
    G4 = ((NT + 3) // 4) * 4

    # ---- shared (replicated) weight arrays ----
    g = lambda k: np.asarray(inputs[k], f32)
    shared = {'Wembed': g('W_embed'),
              'ident': np.eye(128, dtype=f32),
              'Wr0': g('Wr0'), 'Wr1a': g('Wr1a'), 'Wr1b': g('Wr1b')}
    # layer-0 hu is weight-only (h0 = attrs @ Wembed): precompute the full
    # gathered table on the host, killing the first AllGather.
    h0_full = attrs @ g('W_embed')                       # [N, C]
    huG0 = np.zeros((BG * NL, C), f32)
    huG0[pid] = h0_full @ g('Wup')[0]
    huG0 = huG0.astype(bf16)
    for i in range(L):
        for nm in ('Wpre1', 'Wpre2', 'Wm1', 'Wm2', 'Wup', 'Wmix'):
            shared[f'{nm}_{i}'] = g(nm)[i]
        shared[f'rW1_{i}'] = g('rW1')[i]
        shared[f'rW2_{i}'] = g('rW2')[i]
        shared[f'rW3_{i}'] = g('rW3')[i]
        # rW4 reshaped l-major: [64, l*128 + c]
        shared[f'rW4_{i}'] = g('rW4')[i].reshape(64, C, 4).transpose(0, 2, 1).reshape(64, 4 * C)
        for nm in ('bpre1', 'bpre2', 'bm1', 'bm2'):
            shared[f'{nm}_{i}'] = g(nm)[i].reshape(C, 1)
        for nm in ('rb1', 'rb2', 'rb3'):
            shared[f'{nm}_{i}'] = g(nm)[i].reshape(64, 1)
        kf = np.zeros((KPAD, C), f32)
        kf[:K] = 0.01 * (kdown @ g('WupE')[i])
        shared[f'kfilt_{i}'] = kf
        shared[f'w2T_{i}'] = g('w2')[i].T.copy()             # [C,4] f32
        shared[f'w3T_{i}'] = g('w3')[i].T.copy()

    layouts = _const_layouts(G4)

    # ---- per-core arrays ----
    in_maps = []
    for b in range(BG):
        sl = slice(starts[b], ends[b])
        per = {}
        slot_b = slot[sl]
        at = np.zeros((NEL, NL), f32)
        at[:, slot_b] = attrs[sl].T
        per['attrsT'] = at
        cam = np.zeros((128, NBLK * KPAD), f32)   # atom-major cosd, per block
        sam = np.zeros((128, NBLK * KPAD), f32)
        ckm = np.zeros((KPAD, NL), f32)           # k-major
        skm = np.zeros((KPAD, NL), f32)
        pr, bb = slot_b % 128, slot_b // 128
        cam.reshape(128, NBLK, KPAD)[pr, bb, :K] = cosd[sl]
        sam.reshape(128, NBLK, KPAD)[pr, bb, :K] = sind[sl]
        ckm[:K, slot_b] = cosd[sl].T
        skm[:K, slot_b] = sind[sl].T
        per['cosdam'], per['sindam'] = cam, sam
        per['cosdkm'], per['sindkm'] = ckm, skm

        efp = np.zeros((8, G4 * 128), f32)
        sip = np.zeros((128, NT), np.int32)
        segY = np.zeros((128, NT * 16 * 128), f32)
        emask = gdst == b
        for k in range(NBLK):
            es = np.nonzero(emask & (kblk == k))[0]
            es = es[np.argsort(slot[dst[es]], kind='stable')]
            s = np.arange(len(es))
            tt, p = s // 128, s % 128
            t = O_list[k] + tt
            efp[:, t * 128 + p] = ef[es].T
            sip[p, t] = pid[src[es]]
            a = slot[dst[es]] - k * 128
            base = t * 2048 + a
            for lm in range(16):
                segY[p, base + lm * 128] = Y[es, lm] / AVG_NEI
        per['efTpack'] = efp

        def pack(entries, np_dt):
            width = sum(e[2] for e in entries)
            arr = np.zeros((128, width), np_dt)
            c0 = 0
            for name, rows, cols in entries:
                src_a = per.get(name, shared.get(name))
                arr[0:rows, c0:c0 + cols] = src_a
                c0 += cols
            return arr

        m = {'srcidx': sip, 'segYpack': segY.astype(segy_np), 'huG0': huG0,
             'cbA': pack(layouts['cbA'], bf16), 'cbB': pack(layouts['cbB'], bf16),
             'cf': pack(layouts['cf'], f32)}
        in_maps.append(m)

    e0 = np.zeros(BG, f32)
    ae = attrs @ np.asarray(inputs['atomic_E'], f32)
    for b in range(BG):
        e0[b] = ae[starts[b]:ends[b]].sum()
    return in_maps, T_list, G4, e0


# ---------------------------------------------------------------- device
def build_kernel(T_list, G4):
    import concourse.bass as bass
    import concourse.bacc as bacc
    import concourse.mybir as mybir
    import concourse.tile as tile

    f32 = mybir.dt.float32
    bf16 = mybir.dt.bfloat16
    sdt = mybir.dt.float8e4 if SEGY_FP8 else bf16
    A = mybir.ActivationFunctionType
    NT = int(sum(T_list))
    Tmax = max(T_list)
    O_list = np.concatenate([[0], np.cumsum(T_list)]).astype(int)
    nc = bacc.Bacc("TRN2", target_bir_lowering=False, debug=False, num_devices=BG)

    dins = {}
    def din(name, shape, dt=f32):
        dins[name] = nc.dram_tensor(name, list(shape), dt, kind="ExternalInput").ap()
        return dins[name]

    # load order = SP queue order: the layer-0 critical path first
    layouts = _const_layouts(G4)
    widths = {buf: sum(e[2] for e in entries) for buf, entries in layouts.items()}
    din('srcidx', (128, NT), mybir.dt.int32)
    din('cbA', (128, widths['cbA']), bf16)
    din('cf', (128, widths['cf']))
    din('cbB', (128, widths['cbB']), bf16)
    segY_d = din('segYpack', (128, NT * 2048), sdt)
    huG0_d = din('huG0', (BG * NL, C), bf16)
    en_out = nc.dram_tensor('en_out', [1, 1], f32, kind="ExternalOutput").ap()

    with tile.TileContext(nc) as tc:
        with (
            tc.tile_pool(name="const", bufs=1) as cp,
            tc.tile_pool(name="work", bufs=2) as wp,
            tc.tile_pool(name="segy", bufs=5) as sy,
            tc.tile_pool(name="big", bufs=1) as bp,
            tc.tile_pool(name="psA", bufs=1, space="PSUM") as psA,
            tc.tile_pool(name="psS", bufs=2, space="PSUM") as psS,
            tc.tile_pool(name="psW", bufs=2, space="PSUM") as psW,
            tc.tile_pool(name="dram", bufs=1, space="DRAM") as dp,
        ):
            sb = {}
            for name in ('srcidx', 'cbA', 'cf', 'cbB'):
                ap = dins[name]
                t = cp.tile(list(ap.shape), ap.dtype, tag=name)
                nc.sync.dma_start(t[:], ap[:])
                if name == 'srcidx':
                    sb[name] = t
                else:
                    c0 = 0
                    for nm, rows, cols in layouts[name]:
                        sb[nm] = t[0:rows, c0:c0 + cols]
                        c0 += cols

            h = bp.tile([C, NL], bf16, tag="h")
            en = bp.tile([1, 1], f32, tag="en")
            feats_cm = bp.tile([C, NL], bf16, tag="feats_cm")
            hres_am = bp.tile([128, NBLK * 128], bf16, tag="hres_am")
            nc.vector.memset(en[:], 0.0)

            pe = psW.tile([C, NL], f32, tag="pb")
            nc.tensor.matmul(pe[:], sb['Wembed'][:], sb['attrsT'][:], start=True, stop=True)
            nc.scalar.activation(h[:], pe[:], A.Copy)

            hu_dt = mybir.dt.float8e4 if HU_FP8 else bf16
            coll = {}     # layer -> (huL, huG) for layers >= 1
            for i in range(L):
                # ---- gather source: host table (layer 0) or prior AllGather ----
                huG = huG0_d if i == 0 else coll[i][1][:]
                hugs = []
                for k in range(NBLK):
                    lo, hi = int(O_list[k]), int(O_list[k + 1])
                    hg = wp.tile([128, Tmax * 128], bf16 if i == 0 else hu_dt,
                                 tag=f"hug{k % 2}{i}", bufs=1)
                    nc.gpsimd.indirect_dma_start(
                        out=hg[:, 0:(hi - lo) * 128], out_offset=None, in_=huG[:],
                        in_offset=bass.IndirectOffsetOnAxis(
                            ap=sb['srcidx'][:, lo:hi], axis=0))
                    hugs.append(hg)
                if i + 1 < L:
                    huL_next = dp.tile([NL, C], hu_dt, tag=f"huL{i + 1}")
                    huG_next = dp.tile([BG * NL, C], hu_dt, tag=f"huG{i + 1}",
                                       addr_space="Shared")
                    hu_am = wp.tile([128, NL], hu_dt, tag="hu_am")
                    coll[i + 1] = (huL_next, huG_next)

                # ---- radial MLP: issued just-in-time, one 4-tile group ahead
                # of the edge loop (the matmul->silu chain is ~3us latency and
                # would serialize as a phase; interleaved it hides behind the
                # per-tile scatter work).
                s3sb = wp.tile([64, G4 * 128], bf16, tag="s3sb")
                radial_next = [0]

                def radial_group(gidx, i=i, s3sb=s3sb):
                    gsl = slice(gidx * 512, (gidx + 1) * 512)
                    pr1 = psS.tile([128, 512], f32, tag="ps")
                    nc.tensor.matmul(pr1[0:64, :], sb[f'rW1_{i}'][:], sb['efTpack'][:, gsl],
                                     start=True, stop=True)
                    s1 = wp.tile([64, 512], bf16, tag="s1")
                    nc.scalar.activation(s1[:], pr1[0:64, :], A.Silu, bias=sb[f'rb1_{i}'][:])
                    pr2 = psS.tile([128, 512], f32, tag="ps")
                    nc.tensor.matmul(pr2[0:64, :], sb[f'rW2_{i}'][:], s1[:], start=True, stop=True)
                    s2 = wp.tile([64, 512], bf16, tag="s1")
                    nc.scalar.activation(s2[:], pr2[0:64, :], A.Silu, bias=sb[f'rb2_{i}'][:])
                    pr3 = psS.tile([128, 512], f32, tag="ps")
                    nc.tensor.matmul(pr3[0:64, :], sb[f'rW3_{i}'][:], s2[:], start=True, stop=True)
                    nc.scalar.activation(s3sb[:, gsl], pr3[0:64, :], A.Silu, bias=sb[f'rb3_{i}'][:])

                def ensure_radial(gwant):
                    while radial_next[0] <= min(gwant, G4 // 4 - 1):
                        radial_group(radial_next[0])
                        radial_next[0] += 1

                ensure_radial(1)
                # ---- Ewald block (independent of the collective) ----
                p1 = psW.tile([C, NL], f32, tag="pb")
                nc.tensor.matmul(p1[:], sb[f'Wpre1_{i}'][:], h[:], start=True, stop=True)
                t1 = wp.tile([C, NL], bf16, tag="t1")
                nc.scalar.activation(t1[:], p1[:], A.Silu, bias=sb[f'bpre1_{i}'][:])
                p2 = psW.tile([C, NL], f32, tag="pb")
                nc.tensor.matmul(p2[:], sb[f'Wpre2_{i}'][:], t1[:], start=True, stop=True)
                hres = wp.tile([C, NL], bf16, tag="hres")
                nc.vector.tensor_scalar_add(hres[:], p2[:], sb[f'bpre2_{i}'][:])
                nc.vector.tensor_add(hres[:], hres[:], h[:])
                for k in range(NBLK):
                    pt = psS.tile([128, 512], f32, tag="ps")
                    ptb = pt[:].bitcast(bf16)[:, 0:128]
                    nc.tensor.transpose(ptb, hres[:, k * 128:(k + 1) * 128], sb['ident'][:])
                    nc.scalar.activation(hres_am[:, k * 128:(k + 1) * 128], ptb, A.Copy)
                sfk = {}
                for nm, am in (('r', 'cosdam'), ('i', 'sindam')):
                    psf = psS.tile([128, 512], f32, tag="ps")
                    for k in range(NBLK):
                        nc.tensor.matmul(psf[:, 0:128], sb[am][:, k * KPAD:(k + 1) * KPAD],
                                         hres_am[:, k * 128:(k + 1) * 128],
                                         start=(k == 0), stop=(k == NBLK - 1))
                    s = wp.tile([KPAD, C], bf16, tag=f"sfk{nm}")
                    nc.vector.tensor_tensor(s[:], psf[:, 0:128], sb[f'kfilt_{i}'][:],
                                            op=mybir.AluOpType.mult)
                    sfk[nm] = s
                phe = psW.tile([C, NL], f32, tag="pb")
                nc.tensor.matmul(phe[:], sfk['r'][:], sb['cosdkm'][:], start=True, stop=False)
                nc.tensor.matmul(phe[:], sfk['i'][:], sb['sindkm'][:], start=False, stop=True)
                he0 = wp.tile([C, NL], bf16, tag="he0")
                nc.scalar.activation(he0[:], phe[:], A.Copy)
                pm1 = psW.tile([C, NL], f32, tag="pb")
                nc.tensor.matmul(pm1[:], sb[f'Wm1_{i}'][:], he0[:], start=True, stop=True)
                tm = wp.tile([C, NL], bf16, tag="t1")
                nc.scalar.activation(tm[:], pm1[:], A.Silu, bias=sb[f'bm1_{i}'][:])
                pm2 = psW.tile([C, NL], f32, tag="pb")
                nc.tensor.matmul(pm2[:], sb[f'Wm2_{i}'][:], tm[:], start=True, stop=True)
                he2 = wp.tile([C, NL], bf16, tag="he2")
                nc.scalar.activation(he2[:], pm2[:], A.Silu, bias=sb[f'bm2_{i}'][:])
                if i > 0:
                    # PE is otherwise idle while the AllGather is in flight:
                    # run the whole radial MLP pipeline under it.
                    ensure_radial(G4 // 4 - 1)

                # ---- edge loop ----
                def issue_pair(k, tt0, alt, i=i, hugs=hugs):
                    # one 2-tile unit: paired segY DMA, two rW4 matmuls, two mw
                    # products written fp8 into one [128,1024] tile for the
                    # DoubleRow scatter. Second mw alternates DVE/gpsimd.
                    t = int(O_list[k]) + tt0
                    ensure_radial((t + 1) // 4 + 2)
                    sgt = sy.tile([128, 4096], sdt, tag="sg")
                    nc.sync.dma_start(sgt[:], segY_d[:, t * 2048:(t + 2) * 2048])
                    mw = wp.tile([128, 1024], sdt, tag="mw")
                    for j in range(2):
                        pw = psW.tile([C, NL], f32, tag="pb")
                        nc.tensor.matmul(pw[:], s3sb[:, (t + j) * 128:(t + j + 1) * 128],
                                         sb[f'rW4_{i}'][:], start=True, stop=True)
                        nc.vector.tensor_tensor(
                            mw[:, j * 512:(j + 1) * 512].rearrange("p (l c) -> p l c", l=4),
                            pw[:].rearrange("p (l c) -> p l c", l=4),
                            hugs[k][:, (tt0 + j) * 128:(tt0 + j + 1) * 128]
                                .unsqueeze(1).broadcast_to([128, 4, 128]),
                            op=mybir.AluOpType.mult)
                    return mw, sgt

                PAIRS = [(k, tt0) for k in range(NBLK) for tt0 in range(0, int(T_list[k]), 2)]
                LEAD = 1
                fifo = [issue_pair(*PAIRS[j], alt=(j % 2 == 0)) for j in range(LEAD)]
                tidx = [0]

                def next_mw_sgt():
                    j = tidx[0]
                    if j + LEAD < len(PAIRS):
                        fifo.append(issue_pair(*PAIRS[j + LEAD], alt=(j % 2 == 0)))
                    tidx[0] += 1
                    return fifo.pop(0)

                def make_tail(k, i=i, he2=he2):
                    # layer tail for block k: h update, readout, next layer's hu.
                    # Deferred into the next block's tile stream so the product
                    # basis (DVE) overlaps the next block's scatters (PE).
                    def tail():
                        blk = slice(k * 128, (k + 1) * 128)
                        pmx = psW.tile([C, NL], f32, tag="pb")
                        nc.tensor.matmul(pmx[:, 0:128], sb[f'Wmix_{i}'][:],
                                         feats_cm[:, blk], start=True, stop=True)
                        hnk = wp.tile([C, 128], f32, tag="hn")
                        nc.vector.tensor_add(hnk[:], pmx[:, 0:128], he2[:, blk])
                        nc.vector.tensor_add(hnk[:], hnk[:], h[:, blk])
                        nc.vector.tensor_scalar_mul(h[:, blk], hnk[:], float(SKIP))
                        if i == 0:
                            prd = psS.tile([128, 512], f32, tag="ps")
                            nc.tensor.matmul(prd[0:1, 0:128], sb['Wr0'][:], h[:, blk],
                                             start=True, stop=True)
                            rs = wp.tile([1, 1], f32, tag="rs")
                            nc.vector.reduce_sum(rs[:], prd[0:1, 0:128],
                                                 axis=mybir.AxisListType.X)
                            nc.vector.tensor_add(en[:], en[:], rs[:])
                        else:
                            pra = psS.tile([128, 512], f32, tag="ps")
                            nc.tensor.matmul(pra[0:16, 0:128], sb['Wr1a'][:], h[:, blk],
                                             start=True, stop=True)
                            ta = wp.tile([16, 128], bf16, tag="ta")
                            nc.scalar.activation(ta[:], pra[0:16, 0:128], A.Silu)
                            prb = psS.tile([128, 512], f32, tag="ps")
                            nc.tensor.matmul(prb[0:1, 0:128], sb['Wr1b'][:], ta[:],
                                             start=True, stop=True)
                            rs = wp.tile([1, 1], f32, tag="rs")
                            nc.vector.reduce_sum(rs[:], prb[0:1, 0:128],
                                                 axis=mybir.AxisListType.X)
                            nc.vector.tensor_add(en[:], en[:], rs[:])
                        if i + 1 < L:
                            ph = psW.tile([C, NL], f32, tag="pb")
                            nc.tensor.matmul(ph[:, 0:128], h[:, blk],
                                             sb[f'Wup_{i + 1}'][:], start=True, stop=True)
                            nc.scalar.activation(hu_am[:, blk], ph[:, 0:128], A.Copy)
                            nc.sync.dma_start(huL_next[k * 128:(k + 1) * 128, :],
                                              hu_am[:, blk])
                    return tail

                pending_tail = None
                for k in range(NBLK):
                    Tk = int(T_list[k])
                    pA1 = psA.tile([128, 1024], f32, tag="pA1")
                    pA2 = psA.tile([128, 1024], f32, tag="pA2")
                    for pp in range(Tk // 2):
                        mw, sgt = next_mw_sgt()
                        mw3 = mw[:].rearrange("p (two x) -> p two x", two=2)
                        sg3 = sgt[:].rearrange("p (two x) -> p two x", two=2)
                        for (l, m0, w) in CHUNKS:
                            pAh, off = (pA1, m0) if m0 < 8 else (pA2, m0 - 8)
                            nc.tensor.matmul(pAh[:, off * 128:(off + w) * 128],
                                             mw3[:, :, l * 128:(l + 1) * 128],
                                             sg3[:, :, m0 * 128:(m0 + w) * 128],
                                             start=(pp == 0), stop=(pp == Tk // 2 - 1),
                                             perf_mode=mybir.MatmulPerfMode.DoubleRow)
                        if pending_tail is not None and pp == min(2, Tk // 2 - 1):
                            pending_tail()
                            pending_tail = None
                    # free pA early: scal copy + A^2 on the scalar engine, then
                    # the DVE product basis reads SBUF only.
                    scal = wp.tile([128, 128], bf16, tag="scal")
                    nc.scalar.activation(scal[:], pA1[:, 0:128], A.Copy)
                    AA = wp.tile([128, 2048], bf16, tag="AA")
                    nc.scalar.activation(AA[:, 0:1024], pA1[:], A.Square)
                    nc.scalar.activation(AA[:, 1024:2048], pA2[:], A.Square)
                    # ---- product basis for this block (c-major throughout).
                    # Split across DVE and the mostly-idle gpsimd engine so the
                    # per-tile mw ops on DVE don't stall at block boundaries.
                    AA3 = AA[:].rearrange("c (m a) -> c a m", m=16)
                    inv = wp.tile([128, 512], f32, tag="inv")
                    nc.vector.tensor_copy(inv[:, 0:128], AA3[:, :, 0])
                    nc.vector.reduce_sum(inv[:, 128:256].unsqueeze(2), AA3[:, :, 1:4],
                                         axis=mybir.AxisListType.X)
                    for l in (2, 3):
                        isl = inv[:, l * 128:(l + 1) * 128]
                        m0, wl = L_START[l], L_WIDTH[l]
                        nc.gpsimd.tensor_tensor(isl, AA3[:, :, m0], AA3[:, :, m0 + 1],
                                                op=mybir.AluOpType.add)
                        for mm in range(m0 + 2, m0 + wl):
                            nc.gpsimd.tensor_tensor(isl, isl, AA3[:, :, mm],
                                                    op=mybir.AluOpType.add)
                    acc = {}
                    for wnm, eng in (('w2T', nc.vector), ('w3T', nc.gpsimd)):
                        t2 = wp.tile([128, 512], f32, tag=f"t2{wnm}")
                        eng.tensor_tensor(
                            t2[:].rearrange("c (l a) -> c l a", l=4),
                            inv[:].rearrange("c (l a) -> c l a", l=4),
                            sb[f'{wnm}_{i}'][:].unsqueeze(2).broadcast_to([128, 4, 128]),
                            op=mybir.AluOpType.mult)
                        ac = wp.tile([128, 128], f32, tag=f"ac{wnm}")
                        if eng is nc.vector:
                            eng.reduce_sum(ac[:].unsqueeze(2),
                                           t2[:].rearrange("c (l a) -> c a l", l=4),
                                           axis=mybir.AxisListType.X)
                        else:
                            t23 = t2[:].rearrange("c (l a) -> c l a", l=4)
                            eng.tensor_tensor(ac[:], t23[:, 0, :], t23[:, 1, :],
                                              op=mybir.AluOpType.add)
                            eng.tensor_tensor(ac[:], ac[:], t23[:, 2, :],
                                              op=mybir.AluOpType.add)
                            eng.tensor_tensor(ac[:], ac[:], t23[:, 3, :],
                                              op=mybir.AluOpType.add)
                        acc[wnm] = ac
                    fe = wp.tile([128, 128], f32, tag="fe")
                    nc.vector.tensor_tensor(fe[:], scal[:], acc['w3T'][:],
                                            op=mybir.AluOpType.mult)
                    nc.vector.tensor_add(fe[:], fe[:], acc['w2T'][:])
                    nc.vector.tensor_tensor(feats_cm[:, k * 128:(k + 1) * 128], fe[:],
                                            scal[:], op=mybir.AluOpType.add)
                    pending_tail = make_tail(k)
                pending_tail()
                if i + 1 < L:
                    cin, cout = huL_next[:], huG_next[:]
                    if HU_FP8:
                        cin, cout = cin.bitcast(bf16), cout.bitcast(bf16)
                    nc.gpsimd.collective_compute(
                        "AllGather", mybir.AluOpType.bypass,
                        replica_groups=[list(range(BG))],
                        ins=[cin.opt()], outs=[cout.opt()])
            nc.sync.dma_start(en_out[:], en[:])
    nc.compile()
    return nc


def kernel(**inputs):
    from concourse import bass_utils
    in_maps, T_list, G4, e0 = host_prep(inputs)
    key = (tuple(T_list), G4)
    if key not in _CACHE:
        _CACHE[key] = build_kernel(T_list, G4)
    nc = _CACHE[key]
    res = bass_utils.run_bass_kernel_spmd(nc, in_maps, core_ids=list(range(BG)))
    energy = np.zeros(BG, np.float32)
    for b in range(BG):
        energy[b] = res.results[b]['en_out'].reshape(-1)[0] + e0[b]
    return energy


# revision 35
# speedup vs baseline: 1.0824x; 1.0122x over previous
"""MACE+Ewald forward on 8 Trainium2 NeuronCores.

Sharding: graph-per-core (8 graphs, 8 cores). Atoms balanced across 4 blocks
of <=128 slots (padded NL=512 per core); edges assigned to the core/block
owning their dst atom, packed into 128-edge tiles with per-block tile counts.

Key device-side structure per layer:
  1. Layer 0's gather table huG0 = (attrs@Wembed)@Wup_0 is weight-only and
     precomputed on the host, so only ONE AllGather remains (layer 1's hu,
     fp8 payload bitcast to bf16 for transport, Shared output). It is kicked
     from the previous layer's per-block tails and overlapped by the Ewald
     block plus an eager radial-MLP pipeline.
  2. Ewald: structure factors / he MLP, all bf16 matmuls.
  3. Radial MLP issued just-in-time one 4-tile group ahead of the edge loop
     (as a phase its matmul->silu chain latency would serialize).
  4. Edge loop: one batched indirect gather of hu rows per block; edge tiles
     are processed in PAIRS: the per-(channel,l) weights times gathered hu
     (mw, fp8) is the stationary operand and a host-precomputed segY matrix
     (one-hot dst scatter with spherical harmonics Y and 1/avg_nei folded
     in, fp8) is the moving operand of DoubleRow matmuls that contract 256
     edges per pass, c-major output so no transposes are needed afterwards.
     rW4 products are issued via a lead-2 FIFO so the DVE mw latency hides.
  5. Product basis (A^2 contractions) per block split across DVE/gpsimd,
     PSUM freed early through scalar-engine copies; h update, readout and
     the next layer's hu are deferred into the next block's tile stream.

All matmuls run bf16 or fp8 operands with fp32 PSUM accumulation (the
harness tolerance is 2e-2; measured error stays ~2.6e-3). Constants load
as three fused buffers (one DMA each) to cut HWDGE issue latency.
"""

import numpy as np
import ml_dtypes

C = 128
L = 2
NB = 8
NEL = 10
BG = 8
N_ATOMS = 3200
N_EDGES = 51200
R_MAX = 5.0
P_CUT = 5.0
AVG_NEI = 16.0
DELTA_K = 0.2
NKRBF = 128
DP = 8
SKIP = (2.0 + 1.0) ** -0.5
NL = 512            # padded atoms per core
NBLK = NL // 128    # atom blocks per core
KPAD = 128          # padded k-point count (real: 123)
LOFLM = np.repeat(np.arange(4), [1, 3, 5, 7])   # [16]
L_START = [0, 1, 4, 9]
L_WIDTH = [1, 3, 5, 7]
# scatter matmul chunks: (l, first lm, number of lm) with moving-free <= 512
CHUNKS = [(2, 8, 1), (3, 9, 4), (3, 13, 3), (0, 0, 1), (1, 1, 3), (2, 4, 4)]
SEGY_FP8 = True
HU_FP8 = True
SCAT_DR = True   # fp8 DoubleRow scatter (2 edge tiles per PE pass)      # layer>=1 hu AllGather + gather in fp8e4m3

_CACHE = {}


def _const_layouts(G4):
    """Constant packing: 3 fused SBUF-resident buffers loaded with one DMA
    each (HWDGE issue time for ~50 separate loads dominated kernel startup).
    cbA = layer-0 critical path, cbB = the rest, cf = fp32 smalls."""
    bfA = [('attrsT', NEL, NL), ('Wembed', NEL, C),
           ('rW1_0', NB, 64), ('rW2_0', 64, 64), ('rW3_0', 64, 64), ('rW4_0', 64, 4 * C),
           ('efTpack', 8, G4 * 128)]
    bfB = [('Wpre1_0', C, C), ('Wpre2_0', C, C), ('Wm1_0', C, C), ('Wm2_0', C, C),
           ('Wmix_0', C, C), ('Wup_1', C, C),
           ('cosdam', 128, NBLK * KPAD), ('sindam', 128, NBLK * KPAD),
           ('cosdkm', KPAD, NL), ('sindkm', KPAD, NL),
           ('ident', 128, 128), ('Wr0', C, 1), ('Wr1a', C, 16), ('Wr1b', 16, 1),
           ('Wpre1_1', C, C), ('Wpre2_1', C, C), ('Wm1_1', C, C), ('Wm2_1', C, C),
           ('Wmix_1', C, C),
           ('rW1_1', NB, 64), ('rW2_1', 64, 64), ('rW3_1', 64, 64), ('rW4_1', 64, 4 * C)]
    cf = ([('kfilt_0', KPAD, C), ('kfilt_1', KPAD, C)]
          + [(f'w{j}T_{i}', C, 4) for i in range(L) for j in (2, 3)]
          + [(f'{nm}_{i}', C, 1) for i in range(L) for nm in ('bpre1', 'bpre2', 'bm1', 'bm2')]
          + [(f'{nm}_{i}', 64, 1) for i in range(L) for nm in ('rb1', 'rb2', 'rb3')])
    return {'cbA': bfA, 'cbB': bfB, 'cf': cf}


def unpack_consts(m, G4):
    """Recover named f32 views from a core's fused const buffers (for host_sim)."""
    out = {}
    for buf, entries in _const_layouts(G4).items():
        c0 = 0
        for name, rows, cols in entries:
            out[name] = np.asarray(m[buf][0:rows, c0:c0 + cols], np.float32)
            c0 += cols
    return out


# ---------------------------------------------------------------- host math
def _sph_np(u):
    x, y, z = u[:, 0], u[:, 1], u[:, 2]
    s3, s5, s15 = 3.0 ** 0.5, 5.0 ** 0.5, 15.0 ** 0.5
    c70, c105, c42, c7 = 70.0 ** 0.5 / 4.0, 105.0 ** 0.5, 42.0 ** 0.5 / 4.0, 7.0 ** 0.5 / 2.0
    comps = [np.ones_like(x),
             s3 * x, s3 * y, s3 * z,
             s15 * x * y, s15 * y * z, 0.5 * s5 * (3 * z * z - 1.0), s15 * x * z,
             0.5 * s15 * (x * x - y * y),
             c70 * y * (3 * x * x - y * y), c105 * x * y * z, c42 * y * (5 * z * z - 1.0),
             c7 * z * (5 * z * z - 3.0), c42 * x * (5 * z * z - 1.0),
             0.5 * c105 * z * (x * x - y * y), c70 * x * (x * x - 3 * y * y)]
    return np.stack(comps, axis=-1).astype(np.float32)


def _radial_np(r):
    n = np.arange(1, NB + 1, dtype=np.float32)
    rb = np.sqrt(2.0 / R_MAX) * np.sin(n * np.pi * r[:, None] / R_MAX) / np.maximum(r, 1e-9)[:, None]
    uu = np.clip(r / R_MAX, 0.0, 1.0)
    p = P_CUT
    env = 1.0 - (p + 1.0) * (p + 2.0) / 2.0 * uu ** 5 + p * (p + 2.0) * uu ** 6 - p * (p + 1.0) / 2.0 * uu ** 7
    env = env * (r < R_MAX)
    return (rb * env[:, None]).astype(np.float32)


def host_prep(inputs):
    """Build per-core padded arrays. Returns (in_maps, T_list, G4, e0)."""
    f32 = np.float32
    bf16 = ml_dtypes.bfloat16
    segy_np = ml_dtypes.float8_e4m3 if SEGY_FP8 else bf16
    pos = np.asarray(inputs['positions'], f32)
    attrs = np.asarray(inputs['node_attrs'], f32)
    shifts = np.asarray(inputs['shifts'], f32)
    eidx = np.asarray(inputs['edge_index']).astype(np.int64)
    batch = np.asarray(inputs['batch']).astype(np.int64)
    kgrid = np.asarray(inputs['kgrid'], f32)
    krbf = np.asarray(inputs['krbf'], f32)
    K = kgrid.shape[0]

    # per-graph contiguous atom ranges (batch is sorted)
    starts = np.searchsorted(batch, np.arange(BG))
    ends = np.searchsorted(batch, np.arange(BG), side='right')
    counts = ends - starts
    assert counts.max() <= NL, counts

    # balanced split of each graph's atoms into NBLK blocks of <=128 slots
    slot = np.zeros(N_ATOMS, np.int64)          # padded local slot per atom
    for b in range(BG):
        n = int(counts[b])
        base, rem = divmod(n, NBLK)
        sizes = [base + (k < rem) for k in range(NBLK)]
        assert max(sizes) <= 128
        cum = 0
        for k in range(NBLK):
            j = np.arange(cum, cum + sizes[k])
            slot[starts[b] + j] = k * 128 + (j - cum)
            cum += sizes[k]
    pid = (batch * NL + slot).astype(np.int32)  # padded global id [N]

    # ---- edge geometry (host) ----
    src, dst = eidx[0], eidx[1]
    vec = pos[dst] - pos[src] + shifts
    r = np.linalg.norm(vec.astype(np.float64), axis=1).astype(f32)
    uvec = vec / np.maximum(r, 1e-9)[:, None]
    Y = _sph_np(uvec)                           # [E,16]
    ef = _radial_np(r)                          # [E,8]

    # ---- Ewald geometry (host) ----
    dot = pos @ kgrid.T                         # [N,K]
    sd = np.prod(np.sinc(0.5 * DELTA_K * pos), axis=1).astype(f32)   # [N]
    cosd = (sd[:, None] * np.cos(dot)).astype(f32)
    sind = (sd[:, None] * np.sin(dot)).astype(f32)

    kdown = krbf @ np.asarray(inputs['Wdown'], f32)      # [K,DP]

    # ---- edge -> (core, block) assignment, per-block tile counts ----
    gdst = batch[dst]
    kblk = slot[dst] // 128
    ecount = np.zeros((BG, NBLK), np.int64)
    np.add.at(ecount, (gdst, kblk), 1)
    T_list = [max(1, int(np.ceil(ecount[:, k].max() / 128))) for k in range(NBLK)]
    if SCAT_DR:
        T_list = [t + (t % 2) for t in T_list]
    O_list = np.concatenate([[0], np.cumsum(T_list)]).astype(int)
    NT = int(O_list[-1])
    G4 = ((NT + 3) // 4) * 4

    # ---- shared (replicated) weight arrays ----
    g = lambda k: np.asarray(inputs[k], f32)
    shared = {'Wembed': g('W_embed'),
              'ident': np.eye(128, dtype=f32),
              'Wr0': g('Wr0'), 'Wr1a': g('Wr1a'), 'Wr1b': g('Wr1b')}
    # layer-0 hu is weight-only (h0 = attrs @ Wembed): precompute the full
    # gathered table on the host, killing the first AllGather.
    h0_full = attrs @ g('W_embed')                       # [N, C]
    huG0 = np.zeros((BG * NL, C), f32)
    huG0[pid] = h0_full @ g('Wup')[0]
    huG0 = huG0.astype(bf16)
    for i in range(L):
        for nm in ('Wpre1', 'Wpre2', 'Wm1', 'Wm2', 'Wup', 'Wmix'):
            shared[f'{nm}_{i}'] = g(nm)[i]
        shared[f'rW1_{i}'] = g('rW1')[i]
        shared[f'rW2_{i}'] = g('rW2')[i]
        shared[f'rW3_{i}'] = g('rW3')[i]
        # rW4 reshaped l-major: [64, l*128 + c]
        shared[f'rW4_{i}'] = g('rW4')[i].reshape(64, C, 4).transpose(0, 2, 1).reshape(64, 4 * C)
        for nm in ('bpre1', 'bpre2', 'bm1', 'bm2'):
            shared[f'{nm}_{i}'] = g(nm)[i].reshape(C, 1)
        for nm in ('rb1', 'rb2', 'rb3'):
            shared[f'{nm}_{i}'] = g(nm)[i].reshape(64, 1)
        kf = np.zeros((KPAD, C), f32)
        kf[:K] = 0.01 * (kdown @ g('WupE')[i])
        shared[f'kfilt_{i}'] = kf
        shared[f'w2T_{i}'] = g('w2')[i].T.copy()             # [C,4] f32
        shared[f'w3T_{i}'] = g('w3')[i].T.copy()

    layouts = _const_layouts(G4)

    # ---- per-core arrays ----
    in_maps = []
    for b in range(BG):
        sl = slice(starts[b], ends[b])
        per = {}
        slot_b = slot[sl]
        at = np.zeros((NEL, NL), f32)
        at[:, slot_b] = attrs[sl].T
        per['attrsT'] = at
        cam = np.zeros((128, NBLK * KPAD), f32)   # atom-major cosd, per block
        sam = np.zeros((128, NBLK * KPAD), f32)
        ckm = np.zeros((KPAD, NL), f32)           # k-major
        skm = np.zeros((KPAD, NL), f32)
        pr, bb = slot_b % 128, slot_b // 128
        cam.reshape(128, NBLK, KPAD)[pr, bb, :K] = cosd[sl]
        sam.reshape(128, NBLK, KPAD)[pr, bb, :K] = sind[sl]
        ckm[:K, slot_b] = cosd[sl].T
        skm[:K, slot_b] = sind[sl].T
        per['cosdam'], per['sindam'] = cam, sam
        per['cosdkm'], per['sindkm'] = ckm, skm

        efp = np.zeros((8, G4 * 128), f32)
        sip = np.zeros((128, NT), np.int32)
        segY = np.zeros((128, NT * 16 * 128), f32)
        emask = gdst == b
        for k in range(NBLK):
            es = np.nonzero(emask & (kblk == k))[0]
            es = es[np.argsort(slot[dst[es]], kind='stable')]
            s = np.arange(len(es))
            tt, p = s // 128, s % 128
            t = O_list[k] + tt
            efp[:, t * 128 + p] = ef[es].T
            sip[p, t] = pid[src[es]]
            a = slot[dst[es]] - k * 128
            base = t * 2048 + a
            for lm in range(16):
                segY[p, base + lm * 128] = Y[es, lm] / AVG_NEI
        per['efTpack'] = efp

        def pack(entries, np_dt):
            width = sum(e[2] for e in entries)
            arr = np.zeros((128, width), np_dt)
            c0 = 0
            for name, rows, cols in entries:
                src_a = per.get(name, shared.get(name))
                arr[0:rows, c0:c0 + cols] = src_a
                c0 += cols
            return arr

        m = {'srcidx': sip, 'segYpack': segY.astype(segy_np), 'huG0': huG0,
             'cbA': pack(layouts['cbA'], bf16), 'cbB': pack(layouts['cbB'], bf16),
             'cf': pack(layouts['cf'], f32)}
        in_maps.append(m)

    e0 = np.zeros(BG, f32)
    ae = attrs @ np.asarray(inputs['atomic_E'], f32)
    for b in range(BG):
        e0[b] = ae[starts[b]:ends[b]].sum()
    return in_maps, T_list, G4, e0


# ---------------------------------------------------------------- device
def build_kernel(T_list, G4):
    import concourse.bass as bass
    import concourse.bacc as bacc
    import concourse.mybir as mybir
    import concourse.tile as tile

    f32 = mybir.dt.float32
    bf16 = mybir.dt.bfloat16
    sdt = mybir.dt.float8e4 if SEGY_FP8 else bf16
    A = mybir.ActivationFunctionType
    NT = int(sum(T_list))
    Tmax = max(T_list)
    O_list = np.concatenate([[0], np.cumsum(T_list)]).astype(int)
    nc = bacc.Bacc("TRN2", target_bir_lowering=False, debug=False, num_devices=BG)

    dins = {}
    def din(name, shape, dt=f32):
        dins[name] = nc.dram_tensor(name, list(shape), dt, kind="ExternalInput").ap()
        return dins[name]

    # load order = SP queue order: the layer-0 critical path first
    layouts = _const_layouts(G4)
    widths = {buf: sum(e[2] for e in entries) for buf, entries in layouts.items()}
    din('srcidx', (128, NT), mybir.dt.int32)
    din('cbA', (128, widths['cbA']), bf16)
    din('cf', (128, widths['cf']))
    din('cbB', (128, widths['cbB']), bf16)
    segY_d = din('segYpack', (128, NT * 2048), sdt)
    huG0_d = din('huG0', (BG * NL, C), bf16)
    en_out = nc.dram_tensor('en_out', [1, 1], f32, kind="ExternalOutput").ap()

    with tile.TileContext(nc) as tc:
        with (
            tc.tile_pool(name="const", bufs=1) as cp,
            tc.tile_pool(name="work", bufs=2) as wp,
            tc.tile_pool(name="segy", bufs=5) as sy,
            tc.tile_pool(name="big", bufs=1) as bp,
            tc.tile_pool(name="psA", bufs=1, space="PSUM") as psA,
            tc.tile_pool(name="psS", bufs=2, space="PSUM") as psS,
            tc.tile_pool(name="psW", bufs=2, space="PSUM") as psW,
            tc.tile_pool(name="dram", bufs=1, space="DRAM") as dp,
        ):
            sb = {}
            for name in ('srcidx', 'cbA', 'cf', 'cbB'):
                ap = dins[name]
                t = cp.tile(list(ap.shape), ap.dtype, tag=name)
                nc.sync.dma_start(t[:], ap[:])
                if name == 'srcidx':
                    sb[name] = t
                else:
                    c0 = 0
                    for nm, rows, cols in layouts[name]:
                        sb[nm] = t[0:rows, c0:c0 + cols]
                        c0 += cols

            h = bp.tile([C, NL], bf16, tag="h")
            en = bp.tile([1, 1], f32, tag="en")
            feats_cm = bp.tile([C, NL], bf16, tag="feats_cm")
            hres_am = bp.tile([128, NBLK * 128], bf16, tag="hres_am")
            nc.vector.memset(en[:], 0.0)

            pe = psW.tile([C, NL], f32, tag="pb")
            nc.tensor.matmul(pe[:], sb['Wembed'][:], sb['attrsT'][:], start=True, stop=True)
            nc.scalar.activation(h[:], pe[:], A.Copy)

            hu_dt = mybir.dt.float8e4 if HU_FP8 else bf16
            coll = {}     # layer -> (huL, huG) for layers >= 1
            for i in range(L):
                # ---- gather source: host table (layer 0) or prior AllGather ----
                huG = huG0_d if i == 0 else coll[i][1][:]
                hugs = []
                for k in range(NBLK):
                    lo, hi = int(O_list[k]), int(O_list[k + 1])
                    hg = wp.tile([128, Tmax * 128], bf16 if i == 0 else hu_dt,
                                 tag=f"hug{k % 2}{i}", bufs=1)
                    nc.gpsimd.indirect_dma_start(
                        out=hg[:, 0:(hi - lo) * 128], out_offset=None, in_=huG[:],
                        in_offset=bass.IndirectOffsetOnAxis(
                            ap=sb['srcidx'][:, lo:hi], axis=0))
                    hugs.append(hg)
                if i + 1 < L:
                    huL_next = dp.tile([NL, C], hu_dt, tag=f"huL{i + 1}")
                    huG_next = dp.tile([BG * NL, C], hu_dt, tag=f"huG{i + 1}",
                                       addr_space="Shared")
                    hu_am = wp.tile([128, NL], hu_dt, tag="hu_am")
                    coll[i + 1] = (huL_next, huG_next)

                # ---- radial MLP: issued just-in-time, one 4-tile group ahead
                # of the edge loop (the matmul->silu chain is ~3us latency and
                # would serialize as a phase; interleaved it hides behind the
                # per-tile scatter work).
                s3sb = wp.tile([64, G4 * 128], bf16, tag="s3sb")
                radial_next = [0]

                def radial_group(gidx, i=i, s3sb=s3sb):
                    gsl = slice(gidx * 512, (gidx + 1) * 512)
                    pr1 = psS.tile([128, 512], f32, tag="ps")
                    nc.tensor.matmul(pr1[0:64, :], sb[f'rW1_{i}'][:], sb['efTpack'][:, gsl],
                                     start=True, stop=True)
                    s1 = wp.tile([64, 512], bf16, tag="s1")
                    nc.scalar.activation(s1[:], pr1[0:64, :], A.Silu, bias=sb[f'rb1_{i}'][:])
                    pr2 = psS.tile([128, 512], f32, tag="ps")
                    nc.tensor.matmul(pr2[0:64, :], sb[f'rW2_{i}'][:], s1[:], start=True, stop=True)
                    s2 = wp.tile([64, 512], bf16, tag="s1")
                    nc.scalar.activation(s2[:], pr2[0:64, :], A.Silu, bias=sb[f'rb2_{i}'][:])
                    pr3 = psS.tile([128, 512], f32, tag="ps")
                    nc.tensor.matmul(pr3[0:64, :], sb[f'rW3_{i}'][:], s2[:], start=True, stop=True)
                    nc.scalar.activation(s3sb[:, gsl], pr3[0:64, :], A.Silu, bias=sb[f'rb3_{i}'][:])

                def ensure_radial(gwant):
                    while radial_next[0] <= min(gwant, G4 // 4 - 1):
                        radial_group(radial_next[0])
                        radial_next[0] += 1

                ensure_radial(1)
                # ---- Ewald block (independent of the collective) ----
                p1 = psW.tile([C, NL], f32, tag="pb")
                nc.tensor.matmul(p1[:], sb[f'Wpre1_{i}'][:], h[:], start=True, stop=True)
                t1 = wp.tile([C, NL], bf16, tag="t1")
                nc.scalar.activation(t1[:], p1[:], A.Silu, bias=sb[f'bpre1_{i}'][:])
                p2 = psW.tile([C, NL], f32, tag="pb")
                nc.tensor.matmul(p2[:], sb[f'Wpre2_{i}'][:], t1[:], start=True, stop=True)
                hres = wp.tile([C, NL], bf16, tag="hres")
                nc.vector.tensor_scalar_add(hres[:], p2[:], sb[f'bpre2_{i}'][:])
                nc.vector.tensor_add(hres[:], hres[:], h[:])
                for k in range(NBLK):
                    pt = psS.tile([128, 512], f32, tag="ps")
                    ptb = pt[:].bitcast(bf16)[:, 0:128]
                    nc.tensor.transpose(ptb, hres[:, k * 128:(k + 1) * 128], sb['ident'][:])
                    nc.scalar.activation(hres_am[:, k * 128:(k + 1) * 128], ptb, A.Copy)
                sfk = {}
                for nm, am in (('r', 'cosdam'), ('i', 'sindam')):
                    psf = psS.tile([128, 512], f32, tag="ps")
                    for k in range(NBLK):
                        nc.tensor.matmul(psf[:, 0:128], sb[am][:, k * KPAD:(k + 1) * KPAD],
                                         hres_am[:, k * 128:(k + 1) * 128],
                                         start=(k == 0), stop=(k == NBLK - 1))
                    s = wp.tile([KPAD, C], bf16, tag=f"sfk{nm}")
                    nc.vector.tensor_tensor(s[:], psf[:, 0:128], sb[f'kfilt_{i}'][:],
                                            op=mybir.AluOpType.mult)
                    sfk[nm] = s
                phe = psW.tile([C, NL], f32, tag="pb")
                nc.tensor.matmul(phe[:], sfk['r'][:], sb['cosdkm'][:], start=True, stop=False)
                nc.tensor.matmul(phe[:], sfk['i'][:], sb['sindkm'][:], start=False, stop=True)
                he0 = wp.tile([C, NL], bf16, tag="he0")
                nc.scalar.activation(he0[:], phe[:], A.Copy)
                pm1 = psW.tile([C, NL], f32, tag="pb")
                nc.tensor.matmul(pm1[:], sb[f'Wm1_{i}'][:], he0[:], start=True, stop=True)
                tm = wp.tile([C, NL], bf16, tag="t1")
                nc.scalar.activation(tm[:], pm1[:], A.Silu, bias=sb[f'bm1_{i}'][:])
                pm2 = psW.tile([C, NL], f32, tag="pb")
                nc.tensor.matmul(pm2[:], sb[f'Wm2_{i}'][:], tm[:], start=True, stop=True)
                he2 = wp.tile([C, NL], bf16, tag="he2")
                nc.scalar.activation(he2[:], pm2[:], A.Silu, bias=sb[f'bm2_{i}'][:])
                if i > 0:
                    # PE is otherwise idle while the AllGather is in flight:
                    # run the whole radial MLP pipeline under it.
                    ensure_radial(G4 // 4 - 1)

                # ---- edge loop ----
                def issue_pair(k, tt0, alt, i=i, hugs=hugs):
                    # one 2-tile unit: paired segY DMA, two rW4 matmuls, two mw
                    # products written fp8 into one [128,1024] tile for the
                    # DoubleRow scatter. Second mw alternates DVE/gpsimd.
                    t = int(O_list[k]) + tt0
                    ensure_radial((t + 1) // 4 + 2)
                    sgt = sy.tile([128, 4096], sdt, tag="sg")
                    nc.sync.dma_start(sgt[:], segY_d[:, t * 2048:(t + 2) * 2048])
                    mw = wp.tile([128, 1024], sdt, tag="mw")
                    for j in range(2):
                        pw = psW.tile([C, NL], f32, tag="pb")
                        nc.tensor.matmul(pw[:], s3sb[:, (t + j) * 128:(t + j + 1) * 128],
                                         sb[f'rW4_{i}'][:], start=True, stop=True)
                        nc.vector.tensor_tensor(
                            mw[:, j * 512:(j + 1) * 512].rearrange("p (l c) -> p l c", l=4),
                            pw[:].rearrange("p (l c) -> p l c", l=4),
                            hugs[k][:, (tt0 + j) * 128:(tt0 + j + 1) * 128]
                                .unsqueeze(1).broadcast_to([128, 4, 128]),
                            op=mybir.AluOpType.mult)
                    return mw, sgt

                PAIRS = [(k, tt0) for k in range(NBLK) for tt0 in range(0, int(T_list[k]), 2)]
                LEAD = 1
                fifo = [issue_pair(*PAIRS[j], alt=(j % 2 == 0)) for j in range(LEAD)]
                tidx = [0]

                def next_mw_sgt():
                    j = tidx[0]
                    if j + LEAD < len(PAIRS):
                        fifo.append(issue_pair(*PAIRS[j + LEAD], alt=(j % 2 == 0)))
                    tidx[0] += 1
                    return fifo.pop(0)

                def make_tail(k, i=i, he2=he2):
                    # layer tail for block k: h update, readout, next layer's hu.
                    # Deferred into the next block's tile stream so the product
                    # basis (DVE) overlaps the next block's scatters (PE).
                    def tail():
                        blk = slice(k * 128, (k + 1) * 128)
                        pmx = psW.tile([C, NL], f32, tag="pb")
                        nc.tensor.matmul(pmx[:, 0:128], sb[f'Wmix_{i}'][:],
                                         feats_cm[:, blk], start=True, stop=True)
                        hnk = wp.tile([C, 128], f32, tag="hn")
                        nc.vector.tensor_add(hnk[:], pmx[:, 0:128], he2[:, blk])
                        nc.vector.tensor_add(hnk[:], hnk[:], h[:, blk])
                        nc.vector.tensor_scalar_mul(h[:, blk], hnk[:], float(SKIP))
                        if i == 0:
                            prd = psS.tile([128, 512], f32, tag="ps")
                            nc.tensor.matmul(prd[0:1, 0:128], sb['Wr0'][:], h[:, blk],
                                             start=True, stop=True)
                            rs = wp.tile([1, 1], f32, tag="rs")
                            nc.vector.reduce_sum(rs[:], prd[0:1, 0:128],
                                                 axis=mybir.AxisListType.X)
                            nc.vector.tensor_add(en[:], en[:], rs[:])
                        else:
                            pra = psS.tile([128, 512], f32, tag="ps")
                            nc.tensor.matmul(pra[0:16, 0:128], sb['Wr1a'][:], h[:, blk],
                                             start=True, stop=True)
                            ta = wp.tile([16, 128], bf16, tag="ta")
                            nc.scalar.activation(ta[:], pra[0:16, 0:128], A.Silu)
                            prb = psS.tile([128, 512], f32, tag="ps")
                            nc.tensor.matmul(prb[0:1, 0:128], sb['Wr1b'][:], ta[:],
                                             start=True, stop=True)
                            rs = wp.tile([1, 1], f32, tag="rs")
                            nc.vector.reduce_sum(rs[:], prb[0:1, 0:128],
                                                 axis=mybir.AxisListType.X)
                            nc.vector.tensor_add(en[:], en[:], rs[:])
                        if i + 1 < L:
                            ph = psW.tile([C, NL], f32, tag="pb")
                            nc.tensor.matmul(ph[:, 0:128], h[:, blk],
                                             sb[f'Wup_{i + 1}'][:], start=True, stop=True)
                            nc.scalar.activation(hu_am[:, blk], ph[:, 0:128], A.Copy)
                            nc.sync.dma_start(huL_next[k * 128:(k + 1) * 128, :],
                                              hu_am[:, blk])
                    return tail

                def make_pb(k, scal, AA, i=i):
                    # product basis for block k (c-major), reading only SBUF.
                    # Deferred into the next block's pair stream so its DVE ops
                    # don't sit between consecutive blocks' mw ops.
                    def pb():
                        AA3 = AA[:].rearrange("c (m a) -> c a m", m=16)
                        inv = wp.tile([128, 512], f32, tag="inv")
                        nc.vector.tensor_copy(inv[:, 0:128], AA3[:, :, 0])
                        nc.vector.reduce_sum(inv[:, 128:256].unsqueeze(2), AA3[:, :, 1:4],
                                             axis=mybir.AxisListType.X)
                        for l in (2, 3):
                            isl = inv[:, l * 128:(l + 1) * 128]
                            m0, wl = L_START[l], L_WIDTH[l]
                            nc.gpsimd.tensor_tensor(isl, AA3[:, :, m0], AA3[:, :, m0 + 1],
                                                    op=mybir.AluOpType.add)
                            for mm in range(m0 + 2, m0 + wl):
                                nc.gpsimd.tensor_tensor(isl, isl, AA3[:, :, mm],
                                                        op=mybir.AluOpType.add)
                        acc = {}
                        for wnm, eng in (('w2T', nc.vector), ('w3T', nc.gpsimd)):
                            t2 = wp.tile([128, 512], f32, tag=f"t2{wnm}")
                            eng.tensor_tensor(
                                t2[:].rearrange("c (l a) -> c l a", l=4),
                                inv[:].rearrange("c (l a) -> c l a", l=4),
                                sb[f'{wnm}_{i}'][:].unsqueeze(2).broadcast_to([128, 4, 128]),
                                op=mybir.AluOpType.mult)
                            ac = wp.tile([128, 128], f32, tag=f"ac{wnm}")
                            if eng is nc.vector:
                                eng.reduce_sum(ac[:].unsqueeze(2),
                                               t2[:].rearrange("c (l a) -> c a l", l=4),
                                               axis=mybir.AxisListType.X)
                            else:
                                t23 = t2[:].rearrange("c (l a) -> c l a", l=4)
                                eng.tensor_tensor(ac[:], t23[:, 0, :], t23[:, 1, :],
                                                  op=mybir.AluOpType.add)
                                eng.tensor_tensor(ac[:], ac[:], t23[:, 2, :],
                                                  op=mybir.AluOpType.add)
                                eng.tensor_tensor(ac[:], ac[:], t23[:, 3, :],
                                                  op=mybir.AluOpType.add)
                            acc[wnm] = ac
                        fe = wp.tile([128, 128], f32, tag="fe")
                        nc.vector.tensor_tensor(fe[:], scal[:], acc['w3T'][:],
                                                op=mybir.AluOpType.mult)
                        nc.vector.tensor_add(fe[:], fe[:], acc['w2T'][:])
                        nc.vector.tensor_tensor(feats_cm[:, k * 128:(k + 1) * 128], fe[:],
                                                scal[:], op=mybir.AluOpType.add)
                    return pb

                pending = []
                for k in range(NBLK):
                    Tk = int(T_list[k])
                    pA1 = psA.tile([128, 1024], f32, tag="pA1")
                    pA2 = psA.tile([128, 1024], f32, tag="pA2")
                    for pp in range(Tk // 2):
                        mw, sgt = next_mw_sgt()
                        mw3 = mw[:].rearrange("p (two x) -> p two x", two=2)
                        sg3 = sgt[:].rearrange("p (two x) -> p two x", two=2)
                        for (l, m0, w) in CHUNKS:
                            pAh, off = (pA1, m0) if m0 < 8 else (pA2, m0 - 8)
                            nc.tensor.matmul(pAh[:, off * 128:(off + w) * 128],
                                             mw3[:, :, l * 128:(l + 1) * 128],
                                             sg3[:, :, m0 * 128:(m0 + w) * 128],
                                             start=(pp == 0), stop=(pp == Tk // 2 - 1),
                                             perf_mode=mybir.MatmulPerfMode.DoubleRow)
                        if pending and pp == min(1, Tk // 2 - 1):
                            pending.pop(0)()            # previous block's PB
                        if pending and pp == min(3, Tk // 2 - 1):
                            pending.pop(0)()            # previous block's tail
                    # free pA early: AA2 alone frees pA2 (whose half is written
                    # first by the next block), then AA1 + scal free pA1.
                    AA = wp.tile([128, 2048], bf16, tag="AA")
                    nc.scalar.activation(AA[:, 1024:2048], pA2[:], A.Square)
                    nc.scalar.activation(AA[:, 0:1024], pA1[:], A.Square)
                    scal = wp.tile([128, 128], bf16, tag="scal")
                    nc.scalar.activation(scal[:], pA1[:, 0:128], A.Copy)
                    pending = [make_pb(k, scal, AA), make_tail(k)]
                for c in pending:
                    c()
                if i + 1 < L:
                    cin, cout = huL_next[:], huG_next[:]
                    if HU_FP8:
                        cin, cout = cin.bitcast(bf16), cout.bitcast(bf16)
                    nc.gpsimd.collective_compute(
                        "AllGather", mybir.AluOpType.bypass,
                        replica_groups=[list(range(BG))],
                        ins=[cin.opt()], outs=[cout.opt()])
            nc.sync.dma_start(en_out[:], en[:])
    nc.compile()
    return nc


def kernel(**inputs):
    from concourse import bass_utils
    in_maps, T_list, G4, e0 = host_prep(inputs)
    key = (tuple(T_list), G4)
    if key not in _CACHE:
        _CACHE[key] = build_kernel(T_list, G4)
    nc = _CACHE[key]
    res = bass_utils.run_bass_kernel_spmd(nc, in_maps, core_ids=list(range(BG)))
    energy = np.zeros(BG, np.float32)
    for b in range(BG):
        energy[b] = res.results[b]['en_out'].reshape(-1)[0] + e0[b]
    return energy


# revision 36
# speedup vs baseline: 1.1075x; 1.0232x over previous
"""MACE+Ewald forward on 8 Trainium2 NeuronCores.

Sharding: graph-per-core (8 graphs, 8 cores). Atoms balanced across 4 blocks
of <=128 slots (padded NL=512 per core); edges assigned to the core/block
owning their dst atom, packed into 128-edge tiles with per-block tile counts.

Key device-side structure per layer:
  1. Layer 0's gather table huG0 = (attrs@Wembed)@Wup_0 is weight-only and
     precomputed on the host, so only ONE AllGather remains (layer 1's hu,
     fp8 payload bitcast to bf16 for transport, Shared output). It is kicked
     from the previous layer's per-block tails and overlapped by the Ewald
     block plus an eager radial-MLP pipeline.
  2. Ewald: structure factors / he MLP, all bf16 matmuls.
  3. Radial MLP issued just-in-time one 4-tile group ahead of the edge loop
     (as a phase its matmul->silu chain latency would serialize).
  4. Edge loop: one batched indirect gather of hu rows per block; edge tiles
     are processed in PAIRS: the per-(channel,l) weights times gathered hu
     (mw, fp8) is the stationary operand and a host-precomputed segY matrix
     (one-hot dst scatter with spherical harmonics Y and 1/avg_nei folded
     in, fp8) is the moving operand of DoubleRow matmuls that contract 256
     edges per pass, c-major output so no transposes are needed afterwards.
     rW4 products are issued via a lead-2 FIFO so the DVE mw latency hides.
  5. Product basis (A^2 contractions) per block split across DVE/gpsimd,
     PSUM freed early through scalar-engine copies; h update, readout and
     the next layer's hu are deferred into the next block's tile stream.

All matmuls run bf16 or fp8 operands with fp32 PSUM accumulation (the
harness tolerance is 2e-2; measured error stays ~2.6e-3). Constants load
as three fused buffers (one DMA each) to cut HWDGE issue latency.
"""

import numpy as np
import ml_dtypes

C = 128
L = 2
NB = 8
NEL = 10
BG = 8
N_ATOMS = 3200
N_EDGES = 51200
R_MAX = 5.0
P_CUT = 5.0
AVG_NEI = 16.0
DELTA_K = 0.2
NKRBF = 128
DP = 8
SKIP = (2.0 + 1.0) ** -0.5
NL = 512            # padded atoms per core
NBLK = NL // 128    # atom blocks per core
KPAD = 128          # padded k-point count (real: 123)
LOFLM = np.repeat(np.arange(4), [1, 3, 5, 7])   # [16]
L_START = [0, 1, 4, 9]
L_WIDTH = [1, 3, 5, 7]
# scatter matmul chunks: (l, first lm, number of lm) with moving-free <= 512
CHUNKS = [(2, 8, 1), (3, 9, 4), (3, 13, 3), (0, 0, 1), (1, 1, 3), (2, 4, 4)]
SEGY_FP8 = True
HU_FP8 = True
SCAT_DR = True   # fp8 DoubleRow scatter (2 edge tiles per PE pass)      # layer>=1 hu AllGather + gather in fp8e4m3

_CACHE = {}


def _const_layouts(G4):
    """Constant packing: 3 fused SBUF-resident buffers loaded with one DMA
    each (HWDGE issue time for ~50 separate loads dominated kernel startup).
    cbA = layer-0 critical path, cbB = the rest, cf = fp32 smalls."""
    bfA0 = [('attrsT', NEL, NL), ('Wembed', NEL, C),
            ('rW1_0', NB, 64), ('rW2_0', 64, 64), ('rW3_0', 64, 64)]
    bfA = [('rW4_0', 64, 4 * C), ('efTpack', 8, G4 * 128)]
    bfB = [('Wpre1_0', C, C), ('Wpre2_0', C, C), ('Wm1_0', C, C), ('Wm2_0', C, C),
           ('Wmix_0', C, C), ('Wup_1', C, C),
           ('cosdam', 128, NBLK * KPAD), ('sindam', 128, NBLK * KPAD),
           ('cosdkm', KPAD, NL), ('sindkm', KPAD, NL),
           ('ident', 128, 128), ('Wr0', C, 1), ('Wr1a', C, 16), ('Wr1b', 16, 1),
           ('Wpre1_1', C, C), ('Wpre2_1', C, C), ('Wm1_1', C, C), ('Wm2_1', C, C),
           ('Wmix_1', C, C),
           ('rW1_1', NB, 64), ('rW2_1', 64, 64), ('rW3_1', 64, 64), ('rW4_1', 64, 4 * C)]
    cf = ([('kfilt_0', KPAD, C), ('kfilt_1', KPAD, C)]
          + [(f'w{j}T_{i}', C, 4) for i in range(L) for j in (2, 3)]
          + [(f'{nm}_{i}', C, 1) for i in range(L) for nm in ('bpre1', 'bpre2', 'bm1', 'bm2')]
          + [(f'{nm}_{i}', 64, 1) for i in range(L) for nm in ('rb1', 'rb2', 'rb3')])
    return {'cbA0': bfA0, 'cbA': bfA, 'cbB': bfB, 'cf': cf}


def unpack_consts(m, G4):
    """Recover named f32 views from a core's fused const buffers (for host_sim)."""
    out = {}
    for buf, entries in _const_layouts(G4).items():
        c0 = 0
        for name, rows, cols in entries:
            out[name] = np.asarray(m[buf][0:rows, c0:c0 + cols], np.float32)
            c0 += cols
    return out


# ---------------------------------------------------------------- host math
def _sph_np(u):
    x, y, z = u[:, 0], u[:, 1], u[:, 2]
    s3, s5, s15 = 3.0 ** 0.5, 5.0 ** 0.5, 15.0 ** 0.5
    c70, c105, c42, c7 = 70.0 ** 0.5 / 4.0, 105.0 ** 0.5, 42.0 ** 0.5 / 4.0, 7.0 ** 0.5 / 2.0
    comps = [np.ones_like(x),
             s3 * x, s3 * y, s3 * z,
             s15 * x * y, s15 * y * z, 0.5 * s5 * (3 * z * z - 1.0), s15 * x * z,
             0.5 * s15 * (x * x - y * y),
             c70 * y * (3 * x * x - y * y), c105 * x * y * z, c42 * y * (5 * z * z - 1.0),
             c7 * z * (5 * z * z - 3.0), c42 * x * (5 * z * z - 1.0),
             0.5 * c105 * z * (x * x - y * y), c70 * x * (x * x - 3 * y * y)]
    return np.stack(comps, axis=-1).astype(np.float32)


def _radial_np(r):
    n = np.arange(1, NB + 1, dtype=np.float32)
    rb = np.sqrt(2.0 / R_MAX) * np.sin(n * np.pi * r[:, None] / R_MAX) / np.maximum(r, 1e-9)[:, None]
    uu = np.clip(r / R_MAX, 0.0, 1.0)
    p = P_CUT
    env = 1.0 - (p + 1.0) * (p + 2.0) / 2.0 * uu ** 5 + p * (p + 2.0) * uu ** 6 - p * (p + 1.0) / 2.0 * uu ** 7
    env = env * (r < R_MAX)
    return (rb * env[:, None]).astype(np.float32)


def host_prep(inputs):
    """Build per-core padded arrays. Returns (in_maps, T_list, G4, e0)."""
    f32 = np.float32
    bf16 = ml_dtypes.bfloat16
    segy_np = ml_dtypes.float8_e4m3 if SEGY_FP8 else bf16
    pos = np.asarray(inputs['positions'], f32)
    attrs = np.asarray(inputs['node_attrs'], f32)
    shifts = np.asarray(inputs['shifts'], f32)
    eidx = np.asarray(inputs['edge_index']).astype(np.int64)
    batch = np.asarray(inputs['batch']).astype(np.int64)
    kgrid = np.asarray(inputs['kgrid'], f32)
    krbf = np.asarray(inputs['krbf'], f32)
    K = kgrid.shape[0]

    # per-graph contiguous atom ranges (batch is sorted)
    starts = np.searchsorted(batch, np.arange(BG))
    ends = np.searchsorted(batch, np.arange(BG), side='right')
    counts = ends - starts
    assert counts.max() <= NL, counts

    # balanced split of each graph's atoms into NBLK blocks of <=128 slots
    slot = np.zeros(N_ATOMS, np.int64)          # padded local slot per atom
    for b in range(BG):
        n = int(counts[b])
        base, rem = divmod(n, NBLK)
        sizes = [base + (k < rem) for k in range(NBLK)]
        assert max(sizes) <= 128
        cum = 0
        for k in range(NBLK):
            j = np.arange(cum, cum + sizes[k])
            slot[starts[b] + j] = k * 128 + (j - cum)
            cum += sizes[k]
    pid = (batch * NL + slot).astype(np.int32)  # padded global id [N]

    # ---- edge geometry (host) ----
    src, dst = eidx[0], eidx[1]
    vec = pos[dst] - pos[src] + shifts
    r = np.linalg.norm(vec.astype(np.float64), axis=1).astype(f32)
    uvec = vec / np.maximum(r, 1e-9)[:, None]
    Y = _sph_np(uvec)                           # [E,16]
    ef = _radial_np(r)                          # [E,8]

    # ---- Ewald geometry (host) ----
    dot = pos @ kgrid.T                         # [N,K]
    sd = np.prod(np.sinc(0.5 * DELTA_K * pos), axis=1).astype(f32)   # [N]
    cosd = (sd[:, None] * np.cos(dot)).astype(f32)
    sind = (sd[:, None] * np.sin(dot)).astype(f32)

    kdown = krbf @ np.asarray(inputs['Wdown'], f32)      # [K,DP]

    # ---- edge -> (core, block) assignment, per-block tile counts ----
    gdst = batch[dst]
    kblk = slot[dst] // 128
    ecount = np.zeros((BG, NBLK), np.int64)
    np.add.at(ecount, (gdst, kblk), 1)
    T_list = [max(1, int(np.ceil(ecount[:, k].max() / 128))) for k in range(NBLK)]
    if SCAT_DR:
        T_list = [t + (t % 2) for t in T_list]
    O_list = np.concatenate([[0], np.cumsum(T_list)]).astype(int)
    NT = int(O_list[-1])
    G4 = ((NT + 3) // 4) * 4

    # ---- shared (replicated) weight arrays ----
    g = lambda k: np.asarray(inputs[k], f32)
    shared = {'Wembed': g('W_embed'),
              'ident': np.eye(128, dtype=f32),
              'Wr0': g('Wr0'), 'Wr1a': g('Wr1a'), 'Wr1b': g('Wr1b')}
    # layer-0 hu is weight-only (h0 = attrs @ Wembed): precompute the full
    # gathered table on the host, killing the first AllGather.
    h0_full = attrs @ g('W_embed')                       # [N, C]
    huG0 = np.zeros((BG * NL, C), f32)
    huG0[pid] = h0_full @ g('Wup')[0]
    huG0 = huG0.astype(bf16)
    for i in range(L):
        for nm in ('Wpre1', 'Wpre2', 'Wm1', 'Wm2', 'Wup', 'Wmix'):
            shared[f'{nm}_{i}'] = g(nm)[i]
        shared[f'rW1_{i}'] = g('rW1')[i]
        shared[f'rW2_{i}'] = g('rW2')[i]
        shared[f'rW3_{i}'] = g('rW3')[i]
        # rW4 reshaped l-major: [64, l*128 + c]
        shared[f'rW4_{i}'] = g('rW4')[i].reshape(64, C, 4).transpose(0, 2, 1).reshape(64, 4 * C)
        for nm in ('bpre1', 'bpre2', 'bm1', 'bm2'):
            shared[f'{nm}_{i}'] = g(nm)[i].reshape(C, 1)
        for nm in ('rb1', 'rb2', 'rb3'):
            shared[f'{nm}_{i}'] = g(nm)[i].reshape(64, 1)
        kf = np.zeros((KPAD, C), f32)
        kf[:K] = 0.01 * (kdown @ g('WupE')[i])
        shared[f'kfilt_{i}'] = kf
        shared[f'w2T_{i}'] = g('w2')[i].T.copy()             # [C,4] f32
        shared[f'w3T_{i}'] = g('w3')[i].T.copy()

    layouts = _const_layouts(G4)

    # ---- per-core arrays ----
    in_maps = []
    for b in range(BG):
        sl = slice(starts[b], ends[b])
        per = {}
        slot_b = slot[sl]
        at = np.zeros((NEL, NL), f32)
        at[:, slot_b] = attrs[sl].T
        per['attrsT'] = at
        cam = np.zeros((128, NBLK * KPAD), f32)   # atom-major cosd, per block
        sam = np.zeros((128, NBLK * KPAD), f32)
        ckm = np.zeros((KPAD, NL), f32)           # k-major
        skm = np.zeros((KPAD, NL), f32)
        pr, bb = slot_b % 128, slot_b // 128
        cam.reshape(128, NBLK, KPAD)[pr, bb, :K] = cosd[sl]
        sam.reshape(128, NBLK, KPAD)[pr, bb, :K] = sind[sl]
        ckm[:K, slot_b] = cosd[sl].T
        skm[:K, slot_b] = sind[sl].T
        per['cosdam'], per['sindam'] = cam, sam
        per['cosdkm'], per['sindkm'] = ckm, skm

        efp = np.zeros((8, G4 * 128), f32)
        sip = np.zeros((128, NT), np.int32)
        segY = np.zeros((128, NT * 16 * 128), f32)
        emask = gdst == b
        for k in range(NBLK):
            es = np.nonzero(emask & (kblk == k))[0]
            es = es[np.argsort(slot[dst[es]], kind='stable')]
            s = np.arange(len(es))
            tt, p = s // 128, s % 128
            t = O_list[k] + tt
            efp[:, t * 128 + p] = ef[es].T
            sip[p, t] = pid[src[es]]
            a = slot[dst[es]] - k * 128
            base = t * 2048 + a
            for lm in range(16):
                segY[p, base + lm * 128] = Y[es, lm] / AVG_NEI
        per['efTpack'] = efp

        def pack(entries, np_dt):
            width = sum(e[2] for e in entries)
            arr = np.zeros((128, width), np_dt)
            c0 = 0
            for name, rows, cols in entries:
                src_a = per.get(name, shared.get(name))
                arr[0:rows, c0:c0 + cols] = src_a
                c0 += cols
            return arr

        m = {'srcidx': sip, 'segYpack': segY.astype(segy_np), 'huG0': huG0,
             'cbA0': pack(layouts['cbA0'], bf16), 'cbA': pack(layouts['cbA'], bf16),
             'cbB': pack(layouts['cbB'], bf16), 'cf': pack(layouts['cf'], f32)}
        in_maps.append(m)

    e0 = np.zeros(BG, f32)
    ae = attrs @ np.asarray(inputs['atomic_E'], f32)
    for b in range(BG):
        e0[b] = ae[starts[b]:ends[b]].sum()
    return in_maps, T_list, G4, e0


# ---------------------------------------------------------------- device
def build_kernel(T_list, G4):
    import concourse.bass as bass
    import concourse.bacc as bacc
    import concourse.mybir as mybir
    import concourse.tile as tile

    f32 = mybir.dt.float32
    bf16 = mybir.dt.bfloat16
    sdt = mybir.dt.float8e4 if SEGY_FP8 else bf16
    A = mybir.ActivationFunctionType
    NT = int(sum(T_list))
    Tmax = max(T_list)
    O_list = np.concatenate([[0], np.cumsum(T_list)]).astype(int)
    nc = bacc.Bacc("TRN2", target_bir_lowering=False, debug=False, num_devices=BG)

    dins = {}
    def din(name, shape, dt=f32):
        dins[name] = nc.dram_tensor(name, list(shape), dt, kind="ExternalInput").ap()
        return dins[name]

    # load order = SP queue order: the layer-0 critical path first
    layouts = _const_layouts(G4)
    widths = {buf: sum(e[2] for e in entries) for buf, entries in layouts.items()}
    din('srcidx', (128, NT), mybir.dt.int32)
    din('cbA0', (128, widths['cbA0']), bf16)
    din('cf', (128, widths['cf']))
    din('cbB', (128, widths['cbB']), bf16)
    din('cbA', (128, widths['cbA']), bf16)
    segY_d = din('segYpack', (128, NT * 2048), sdt)
    huG0_d = din('huG0', (BG * NL, C), bf16)
    en_out = nc.dram_tensor('en_out', [1, 1], f32, kind="ExternalOutput").ap()

    with tile.TileContext(nc) as tc:
        with (
            tc.tile_pool(name="const", bufs=1) as cp,
            tc.tile_pool(name="work", bufs=2) as wp,
            tc.tile_pool(name="segy", bufs=5) as sy,
            tc.tile_pool(name="big", bufs=1) as bp,
            tc.tile_pool(name="psA", bufs=1, space="PSUM") as psA,
            tc.tile_pool(name="psS", bufs=2, space="PSUM") as psS,
            tc.tile_pool(name="psW", bufs=2, space="PSUM") as psW,
            tc.tile_pool(name="dram", bufs=1, space="DRAM") as dp,
        ):
            sb = {}
            for name in ('srcidx', 'cbA0', 'cf', 'cbB', 'cbA'):
                ap = dins[name]
                t = cp.tile(list(ap.shape), ap.dtype, tag=name)
                nc.sync.dma_start(t[:], ap[:])
                if name == 'srcidx':
                    sb[name] = t
                else:
                    c0 = 0
                    for nm, rows, cols in layouts[name]:
                        sb[nm] = t[0:rows, c0:c0 + cols]
                        c0 += cols

            h = bp.tile([C, NL], bf16, tag="h")
            en = bp.tile([1, 1], f32, tag="en")
            feats_cm = bp.tile([C, NL], bf16, tag="feats_cm")
            hres_am = bp.tile([128, NBLK * 128], bf16, tag="hres_am")
            nc.vector.memset(en[:], 0.0)

            pe = psW.tile([C, NL], f32, tag="pb")
            nc.tensor.matmul(pe[:], sb['Wembed'][:], sb['attrsT'][:], start=True, stop=True)
            nc.scalar.activation(h[:], pe[:], A.Copy)

            hu_dt = mybir.dt.float8e4 if HU_FP8 else bf16
            coll = {}     # layer -> (huL, huG) for layers >= 1
            for i in range(L):
                # ---- gather source: host table (layer 0) or prior AllGather ----
                huG = huG0_d if i == 0 else coll[i][1][:]
                hugs = []
                for k in range(NBLK):
                    lo, hi = int(O_list[k]), int(O_list[k + 1])
                    hg = wp.tile([128, Tmax * 128], bf16 if i == 0 else hu_dt,
                                 tag=f"hug{k % 2}{i}", bufs=1)
                    nc.gpsimd.indirect_dma_start(
                        out=hg[:, 0:(hi - lo) * 128], out_offset=None, in_=huG[:],
                        in_offset=bass.IndirectOffsetOnAxis(
                            ap=sb['srcidx'][:, lo:hi], axis=0))
                    hugs.append(hg)
                if i + 1 < L:
                    huL_next = dp.tile([NL, C], hu_dt, tag=f"huL{i + 1}")
                    huG_next = dp.tile([BG * NL, C], hu_dt, tag=f"huG{i + 1}",
                                       addr_space="Shared")
                    hu_am = wp.tile([128, NL], hu_dt, tag="hu_am")
                    coll[i + 1] = (huL_next, huG_next)

                # ---- radial MLP: issued just-in-time, one 4-tile group ahead
                # of the edge loop (the matmul->silu chain is ~3us latency and
                # would serialize as a phase; interleaved it hides behind the
                # per-tile scatter work).
                s3sb = wp.tile([64, G4 * 128], bf16, tag="s3sb")
                radial_next = [0]

                def radial_group(gidx, i=i, s3sb=s3sb):
                    gsl = slice(gidx * 512, (gidx + 1) * 512)
                    pr1 = psS.tile([128, 512], f32, tag="ps")
                    nc.tensor.matmul(pr1[0:64, :], sb[f'rW1_{i}'][:], sb['efTpack'][:, gsl],
                                     start=True, stop=True)
                    s1 = wp.tile([64, 512], bf16, tag="s1")
                    nc.scalar.activation(s1[:], pr1[0:64, :], A.Silu, bias=sb[f'rb1_{i}'][:])
                    pr2 = psS.tile([128, 512], f32, tag="ps")
                    nc.tensor.matmul(pr2[0:64, :], sb[f'rW2_{i}'][:], s1[:], start=True, stop=True)
                    s2 = wp.tile([64, 512], bf16, tag="s1")
                    nc.scalar.activation(s2[:], pr2[0:64, :], A.Silu, bias=sb[f'rb2_{i}'][:])
                    pr3 = psS.tile([128, 512], f32, tag="ps")
                    nc.tensor.matmul(pr3[0:64, :], sb[f'rW3_{i}'][:], s2[:], start=True, stop=True)
                    nc.scalar.activation(s3sb[:, gsl], pr3[0:64, :], A.Silu, bias=sb[f'rb3_{i}'][:])

                def ensure_radial(gwant):
                    while radial_next[0] <= min(gwant, G4 // 4 - 1):
                        radial_group(radial_next[0])
                        radial_next[0] += 1

                # ---- Ewald block (independent of the collective) ----
                p1 = psW.tile([C, NL], f32, tag="pb")
                nc.tensor.matmul(p1[:], sb[f'Wpre1_{i}'][:], h[:], start=True, stop=True)
                t1 = wp.tile([C, NL], bf16, tag="t1")
                nc.scalar.activation(t1[:], p1[:], A.Silu, bias=sb[f'bpre1_{i}'][:])
                p2 = psW.tile([C, NL], f32, tag="pb")
                nc.tensor.matmul(p2[:], sb[f'Wpre2_{i}'][:], t1[:], start=True, stop=True)
                hres = wp.tile([C, NL], bf16, tag="hres")
                nc.vector.tensor_scalar_add(hres[:], p2[:], sb[f'bpre2_{i}'][:])
                nc.vector.tensor_add(hres[:], hres[:], h[:])
                for k in range(NBLK):
                    pt = psS.tile([128, 512], f32, tag="ps")
                    ptb = pt[:].bitcast(bf16)[:, 0:128]
                    nc.tensor.transpose(ptb, hres[:, k * 128:(k + 1) * 128], sb['ident'][:])
                    nc.scalar.activation(hres_am[:, k * 128:(k + 1) * 128], ptb, A.Copy)
                sfk = {}
                for nm, am in (('r', 'cosdam'), ('i', 'sindam')):
                    psf = psS.tile([128, 512], f32, tag="ps")
                    for k in range(NBLK):
                        nc.tensor.matmul(psf[:, 0:128], sb[am][:, k * KPAD:(k + 1) * KPAD],
                                         hres_am[:, k * 128:(k + 1) * 128],
                                         start=(k == 0), stop=(k == NBLK - 1))
                    s = wp.tile([KPAD, C], bf16, tag=f"sfk{nm}")
                    nc.vector.tensor_tensor(s[:], psf[:, 0:128], sb[f'kfilt_{i}'][:],
                                            op=mybir.AluOpType.mult)
                    sfk[nm] = s
                phe = psW.tile([C, NL], f32, tag="pb")
                nc.tensor.matmul(phe[:], sfk['r'][:], sb['cosdkm'][:], start=True, stop=False)
                nc.tensor.matmul(phe[:], sfk['i'][:], sb['sindkm'][:], start=False, stop=True)
                he0 = wp.tile([C, NL], bf16, tag="he0")
                nc.scalar.activation(he0[:], phe[:], A.Copy)
                pm1 = psW.tile([C, NL], f32, tag="pb")
                nc.tensor.matmul(pm1[:], sb[f'Wm1_{i}'][:], he0[:], start=True, stop=True)
                tm = wp.tile([C, NL], bf16, tag="t1")
                nc.scalar.activation(tm[:], pm1[:], A.Silu, bias=sb[f'bm1_{i}'][:])
                pm2 = psW.tile([C, NL], f32, tag="pb")
                nc.tensor.matmul(pm2[:], sb[f'Wm2_{i}'][:], tm[:], start=True, stop=True)
                he2 = wp.tile([C, NL], bf16, tag="he2")
                nc.scalar.activation(he2[:], pm2[:], A.Silu, bias=sb[f'bm2_{i}'][:])
                # layer 0: start the radial chains right after the dense
                # phase; layer >0: PE is idle under the AllGather, so run the
                # whole radial pipeline there.
                ensure_radial(G4 // 4 - 1 if i > 0 else 1)

                # ---- edge loop ----
                def issue_pair(k, tt0, alt, i=i, hugs=hugs):
                    # one 2-tile unit: paired segY DMA, two rW4 matmuls, two mw
                    # products written fp8 into one [128,1024] tile for the
                    # DoubleRow scatter. Second mw alternates DVE/gpsimd.
                    t = int(O_list[k]) + tt0
                    ensure_radial((t + 1) // 4 + 2)
                    sgt = sy.tile([128, 4096], sdt, tag="sg")
                    nc.sync.dma_start(sgt[:], segY_d[:, t * 2048:(t + 2) * 2048])
                    mw = wp.tile([128, 1024], sdt, tag="mw")
                    for j in range(2):
                        pw = psW.tile([C, NL], f32, tag="pb")
                        nc.tensor.matmul(pw[:], s3sb[:, (t + j) * 128:(t + j + 1) * 128],
                                         sb[f'rW4_{i}'][:], start=True, stop=True)
                        nc.vector.tensor_tensor(
                            mw[:, j * 512:(j + 1) * 512].rearrange("p (l c) -> p l c", l=4),
                            pw[:].rearrange("p (l c) -> p l c", l=4),
                            hugs[k][:, (tt0 + j) * 128:(tt0 + j + 1) * 128]
                                .unsqueeze(1).broadcast_to([128, 4, 128]),
                            op=mybir.AluOpType.mult)
                    return mw, sgt

                PAIRS = [(k, tt0) for k in range(NBLK) for tt0 in range(0, int(T_list[k]), 2)]
                LEAD = 1
                fifo = [issue_pair(*PAIRS[j], alt=(j % 2 == 0)) for j in range(LEAD)]
                tidx = [0]

                def next_mw_sgt():
                    j = tidx[0]
                    if j + LEAD < len(PAIRS):
                        fifo.append(issue_pair(*PAIRS[j + LEAD], alt=(j % 2 == 0)))
                    tidx[0] += 1
                    return fifo.pop(0)

                def make_tail(k, i=i, he2=he2):
                    # layer tail for block k: h update, readout, next layer's hu.
                    # Deferred into the next block's tile stream so the product
                    # basis (DVE) overlaps the next block's scatters (PE).
                    def tail():
                        blk = slice(k * 128, (k + 1) * 128)
                        pmx = psW.tile([C, NL], f32, tag="pb")
                        nc.tensor.matmul(pmx[:, 0:128], sb[f'Wmix_{i}'][:],
                                         feats_cm[:, blk], start=True, stop=True)
                        hnk = wp.tile([C, 128], f32, tag="hn")
                        nc.vector.tensor_add(hnk[:], pmx[:, 0:128], he2[:, blk])
                        nc.vector.tensor_add(hnk[:], hnk[:], h[:, blk])
                        nc.vector.tensor_scalar_mul(h[:, blk], hnk[:], float(SKIP))
                        if i == 0:
                            prd = psS.tile([128, 512], f32, tag="ps")
                            nc.tensor.matmul(prd[0:1, 0:128], sb['Wr0'][:], h[:, blk],
                                             start=True, stop=True)
                            rs = wp.tile([1, 1], f32, tag="rs")
                            nc.vector.reduce_sum(rs[:], prd[0:1, 0:128],
                                                 axis=mybir.AxisListType.X)
                            nc.vector.tensor_add(en[:], en[:], rs[:])
                        else:
                            pra = psS.tile([128, 512], f32, tag="ps")
                            nc.tensor.matmul(pra[0:16, 0:128], sb['Wr1a'][:], h[:, blk],
                                             start=True, stop=True)
                            ta = wp.tile([16, 128], bf16, tag="ta")
                            nc.scalar.activation(ta[:], pra[0:16, 0:128], A.Silu)
                            prb = psS.tile([128, 512], f32, tag="ps")
                            nc.tensor.matmul(prb[0:1, 0:128], sb['Wr1b'][:], ta[:],
                                             start=True, stop=True)
                            rs = wp.tile([1, 1], f32, tag="rs")
                            nc.vector.reduce_sum(rs[:], prb[0:1, 0:128],
                                                 axis=mybir.AxisListType.X)
                            nc.vector.tensor_add(en[:], en[:], rs[:])
                        if i + 1 < L:
                            ph = psW.tile([C, NL], f32, tag="pb")
                            nc.tensor.matmul(ph[:, 0:128], h[:, blk],
                                             sb[f'Wup_{i + 1}'][:], start=True, stop=True)
                            nc.scalar.activation(hu_am[:, blk], ph[:, 0:128], A.Copy)
                            nc.sync.dma_start(huL_next[k * 128:(k + 1) * 128, :],
                                              hu_am[:, blk])
                    return tail

                def make_pb(k, scal, AA, i=i):
                    # product basis for block k (c-major), reading only SBUF.
                    # Deferred into the next block's pair stream so its DVE ops
                    # don't sit between consecutive blocks' mw ops.
                    def pb():
                        AA3 = AA[:].rearrange("c (m a) -> c a m", m=16)
                        inv = wp.tile([128, 512], f32, tag="inv")
                        nc.vector.tensor_copy(inv[:, 0:128], AA3[:, :, 0])
                        nc.vector.reduce_sum(inv[:, 128:256].unsqueeze(2), AA3[:, :, 1:4],
                                             axis=mybir.AxisListType.X)
                        for l in (2, 3):
                            isl = inv[:, l * 128:(l + 1) * 128]
                            m0, wl = L_START[l], L_WIDTH[l]
                            nc.gpsimd.tensor_tensor(isl, AA3[:, :, m0], AA3[:, :, m0 + 1],
                                                    op=mybir.AluOpType.add)
                            for mm in range(m0 + 2, m0 + wl):
                                nc.gpsimd.tensor_tensor(isl, isl, AA3[:, :, mm],
                                                        op=mybir.AluOpType.add)
                        acc = {}
                        for wnm, eng in (('w2T', nc.vector), ('w3T', nc.gpsimd)):
                            t2 = wp.tile([128, 512], f32, tag=f"t2{wnm}")
                            eng.tensor_tensor(
                                t2[:].rearrange("c (l a) -> c l a", l=4),
                                inv[:].rearrange("c (l a) -> c l a", l=4),
                                sb[f'{wnm}_{i}'][:].unsqueeze(2).broadcast_to([128, 4, 128]),
                                op=mybir.AluOpType.mult)
                            ac = wp.tile([128, 128], f32, tag=f"ac{wnm}")
                            if eng is nc.vector:
                                eng.reduce_sum(ac[:].unsqueeze(2),
                                               t2[:].rearrange("c (l a) -> c a l", l=4),
                                               axis=mybir.AxisListType.X)
                            else:
                                t23 = t2[:].rearrange("c (l a) -> c l a", l=4)
                                eng.tensor_tensor(ac[:], t23[:, 0, :], t23[:, 1, :],
                                                  op=mybir.AluOpType.add)
                                eng.tensor_tensor(ac[:], ac[:], t23[:, 2, :],
                                                  op=mybir.AluOpType.add)
                                eng.tensor_tensor(ac[:], ac[:], t23[:, 3, :],
                                                  op=mybir.AluOpType.add)
                            acc[wnm] = ac
                        fe = wp.tile([128, 128], f32, tag="fe")
                        nc.vector.tensor_tensor(fe[:], scal[:], acc['w3T'][:],
                                                op=mybir.AluOpType.mult)
                        nc.vector.tensor_add(fe[:], fe[:], acc['w2T'][:])
                        nc.vector.tensor_tensor(feats_cm[:, k * 128:(k + 1) * 128], fe[:],
                                                scal[:], op=mybir.AluOpType.add)
                    return pb

                pending = []
                for k in range(NBLK):
                    Tk = int(T_list[k])
                    pA1 = psA.tile([128, 1024], f32, tag="pA1")
                    pA2 = psA.tile([128, 1024], f32, tag="pA2")
                    for pp in range(Tk // 2):
                        mw, sgt = next_mw_sgt()
                        mw3 = mw[:].rearrange("p (two x) -> p two x", two=2)
                        sg3 = sgt[:].rearrange("p (two x) -> p two x", two=2)
                        for (l, m0, w) in CHUNKS:
                            pAh, off = (pA1, m0) if m0 < 8 else (pA2, m0 - 8)
                            nc.tensor.matmul(pAh[:, off * 128:(off + w) * 128],
                                             mw3[:, :, l * 128:(l + 1) * 128],
                                             sg3[:, :, m0 * 128:(m0 + w) * 128],
                                             start=(pp == 0), stop=(pp == Tk // 2 - 1),
                                             perf_mode=mybir.MatmulPerfMode.DoubleRow)
                        if pending and pp == min(1, Tk // 2 - 1):
                            pending.pop(0)()            # previous block's PB
                        if pending and pp == min(3, Tk // 2 - 1):
                            pending.pop(0)()            # previous block's tail
                    # free pA early: AA2 alone frees pA2 (whose half is written
                    # first by the next block), then AA1 + scal free pA1.
                    AA = wp.tile([128, 2048], bf16, tag="AA")
                    nc.scalar.activation(AA[:, 1024:2048], pA2[:], A.Square)
                    nc.scalar.activation(AA[:, 0:1024], pA1[:], A.Square)
                    scal = wp.tile([128, 128], bf16, tag="scal")
                    nc.scalar.activation(scal[:], pA1[:, 0:128], A.Copy)
                    pending = [make_pb(k, scal, AA), make_tail(k)]
                for c in pending:
                    c()
                if i + 1 < L:
                    cin, cout = huL_next[:], huG_next[:]
                    if HU_FP8:
                        cin, cout = cin.bitcast(bf16), cout.bitcast(bf16)
                    nc.gpsimd.collective_compute(
                        "AllGather", mybir.AluOpType.bypass,
                        replica_groups=[list(range(BG))],
                        ins=[cin.opt()], outs=[cout.opt()])
            nc.sync.dma_start(en_out[:], en[:])
    nc.compile()
    return nc


def kernel(**inputs):
    from concourse import bass_utils
    in_maps, T_list, G4, e0 = host_prep(inputs)
    key = (tuple(T_list), G4)
    if key not in _CACHE:
        _CACHE[key] = build_kernel(T_list, G4)
    nc = _CACHE[key]
    res = bass_utils.run_bass_kernel_spmd(nc, in_maps, core_ids=list(range(BG)))
    energy = np.zeros(BG, np.float32)
    for b in range(BG):
        energy[b] = res.results[b]['en_out'].reshape(-1)[0] + e0[b]
    return energy


# revision 37
# speedup vs baseline: 1.1201x; 1.0114x over previous
"""MACE+Ewald forward on 8 Trainium2 NeuronCores.

Sharding: graph-per-core (8 graphs, 8 cores). Atoms balanced across 4 blocks
of <=128 slots (padded NL=512 per core); edges assigned to the core/block
owning their dst atom, packed into 128-edge tiles with per-block tile counts.

Key device-side structure per layer:
  1. Layer 0's gather table huG0 = (attrs@Wembed)@Wup_0 is weight-only and
     precomputed on the host, so only ONE AllGather remains (layer 1's hu,
     fp8 payload bitcast to bf16 for transport, Shared output). It is kicked
     from the previous layer's per-block tails and overlapped by the Ewald
     block plus an eager radial-MLP pipeline.
  2. Ewald: structure factors / he MLP, all bf16 matmuls.
  3. Radial MLP issued just-in-time one 4-tile group ahead of the edge loop
     (as a phase its matmul->silu chain latency would serialize).
  4. Edge loop: one batched indirect gather of hu rows per block; edge tiles
     are processed in PAIRS: the per-(channel,l) weights times gathered hu
     (mw, fp8) is the stationary operand and a host-precomputed segY matrix
     (one-hot dst scatter with spherical harmonics Y and 1/avg_nei folded
     in, fp8) is the moving operand of DoubleRow matmuls that contract 256
     edges per pass, c-major output so no transposes are needed afterwards.
     rW4 products are issued via a lead-2 FIFO so the DVE mw latency hides.
  5. Product basis (A^2 contractions) per block split across DVE/gpsimd,
     PSUM freed early through scalar-engine copies; h update, readout and
     the next layer's hu are deferred into the next block's tile stream.

All matmuls run bf16 or fp8 operands with fp32 PSUM accumulation (the
harness tolerance is 2e-2; measured error stays ~2.6e-3). Constants load
as three fused buffers (one DMA each) to cut HWDGE issue latency.
"""

import numpy as np
import ml_dtypes

C = 128
L = 2
NB = 8
NEL = 10
BG = 8
N_ATOMS = 3200
N_EDGES = 51200
R_MAX = 5.0
P_CUT = 5.0
AVG_NEI = 16.0
DELTA_K = 0.2
NKRBF = 128
DP = 8
SKIP = (2.0 + 1.0) ** -0.5
NL = 512            # padded atoms per core
NBLK = NL // 128    # atom blocks per core
KPAD = 128          # padded k-point count (real: 123)
LOFLM = np.repeat(np.arange(4), [1, 3, 5, 7])   # [16]
L_START = [0, 1, 4, 9]
L_WIDTH = [1, 3, 5, 7]
# scatter matmul chunks: (l, first lm, number of lm) with moving-free <= 512
CHUNKS = [(2, 8, 1), (3, 9, 4), (3, 13, 3), (0, 0, 1), (1, 1, 3), (2, 4, 4)]
SEGY_FP8 = True
HU_FP8 = True
SCAT_DR = True   # fp8 DoubleRow scatter (2 edge tiles per PE pass)      # layer>=1 hu AllGather + gather in fp8e4m3

_CACHE = {}


def _const_layouts(G4):
    """Constant packing: 3 fused SBUF-resident buffers loaded with one DMA
    each (HWDGE issue time for ~50 separate loads dominated kernel startup).
    cbA = layer-0 critical path, cbB = the rest, cf = fp32 smalls."""
    bfA0 = [('attrsT', NEL, NL), ('Wembed', NEL, C),
            ('rW1_0', NB, 64), ('rW2_0', 64, 64), ('rW3_0', 64, 64)]
    bfA = [('rW4_0', 64, 4 * C), ('efTpack', 8, G4 * 128)]
    bfB = [('Wpre1_0', C, C), ('Wpre2_0', C, C), ('Wm1_0', C, C), ('Wm2_0', C, C),
           ('Wmix_0', C, C), ('Wup_1', C, C),
           ('cosdam', 128, NBLK * KPAD), ('sindam', 128, NBLK * KPAD),
           ('cosdkm', KPAD, NL), ('sindkm', KPAD, NL),
           ('ident', 128, 128), ('Wr0', C, 1), ('Wr1a', C, 16), ('Wr1b', 16, 1),
           ('Wpre1_1', C, C), ('Wpre2_1', C, C), ('Wm1_1', C, C), ('Wm2_1', C, C),
           ('Wmix_1', C, C),
           ('rW1_1', NB, 64), ('rW2_1', 64, 64), ('rW3_1', 64, 64), ('rW4_1', 64, 4 * C)]
    cf = ([('kfilt_0', KPAD, C), ('kfilt_1', KPAD, C)]
          + [(f'w{j}T_{i}', C, 4) for i in range(L) for j in (2, 3)]
          + [(f'{nm}_{i}', C, 1) for i in range(L) for nm in ('bpre1', 'bpre2', 'bm1', 'bm2')]
          + [(f'{nm}_{i}', 64, 1) for i in range(L) for nm in ('rb1', 'rb2', 'rb3')])
    return {'cbA0': bfA0, 'cbA': bfA, 'cbB': bfB, 'cf': cf}


def unpack_consts(m, G4):
    """Recover named f32 views from a core's fused const buffers (for host_sim)."""
    out = {}
    for buf, entries in _const_layouts(G4).items():
        c0 = 0
        for name, rows, cols in entries:
            out[name] = np.asarray(m[buf][0:rows, c0:c0 + cols], np.float32)
            c0 += cols
    return out


# ---------------------------------------------------------------- host math
def _sph_np(u):
    x, y, z = u[:, 0], u[:, 1], u[:, 2]
    s3, s5, s15 = 3.0 ** 0.5, 5.0 ** 0.5, 15.0 ** 0.5
    c70, c105, c42, c7 = 70.0 ** 0.5 / 4.0, 105.0 ** 0.5, 42.0 ** 0.5 / 4.0, 7.0 ** 0.5 / 2.0
    comps = [np.ones_like(x),
             s3 * x, s3 * y, s3 * z,
             s15 * x * y, s15 * y * z, 0.5 * s5 * (3 * z * z - 1.0), s15 * x * z,
             0.5 * s15 * (x * x - y * y),
             c70 * y * (3 * x * x - y * y), c105 * x * y * z, c42 * y * (5 * z * z - 1.0),
             c7 * z * (5 * z * z - 3.0), c42 * x * (5 * z * z - 1.0),
             0.5 * c105 * z * (x * x - y * y), c70 * x * (x * x - 3 * y * y)]
    return np.stack(comps, axis=-1).astype(np.float32)


def _radial_np(r):
    n = np.arange(1, NB + 1, dtype=np.float32)
    rb = np.sqrt(2.0 / R_MAX) * np.sin(n * np.pi * r[:, None] / R_MAX) / np.maximum(r, 1e-9)[:, None]
    uu = np.clip(r / R_MAX, 0.0, 1.0)
    p = P_CUT
    env = 1.0 - (p + 1.0) * (p + 2.0) / 2.0 * uu ** 5 + p * (p + 2.0) * uu ** 6 - p * (p + 1.0) / 2.0 * uu ** 7
    env = env * (r < R_MAX)
    return (rb * env[:, None]).astype(np.float32)


def host_prep(inputs):
    """Build per-core padded arrays. Returns (in_maps, T_list, G4, e0)."""
    f32 = np.float32
    bf16 = ml_dtypes.bfloat16
    segy_np = ml_dtypes.float8_e4m3 if SEGY_FP8 else bf16
    pos = np.asarray(inputs['positions'], f32)
    attrs = np.asarray(inputs['node_attrs'], f32)
    shifts = np.asarray(inputs['shifts'], f32)
    eidx = np.asarray(inputs['edge_index']).astype(np.int64)
    batch = np.asarray(inputs['batch']).astype(np.int64)
    kgrid = np.asarray(inputs['kgrid'], f32)
    krbf = np.asarray(inputs['krbf'], f32)
    K = kgrid.shape[0]

    # per-graph contiguous atom ranges (batch is sorted)
    starts = np.searchsorted(batch, np.arange(BG))
    ends = np.searchsorted(batch, np.arange(BG), side='right')
    counts = ends - starts
    assert counts.max() <= NL, counts

    # balanced split of each graph's atoms into NBLK blocks of <=128 slots
    slot = np.zeros(N_ATOMS, np.int64)          # padded local slot per atom
    for b in range(BG):
        n = int(counts[b])
        base, rem = divmod(n, NBLK)
        sizes = [base + (k < rem) for k in range(NBLK)]
        assert max(sizes) <= 128
        cum = 0
        for k in range(NBLK):
            j = np.arange(cum, cum + sizes[k])
            slot[starts[b] + j] = k * 128 + (j - cum)
            cum += sizes[k]
    pid = (batch * NL + slot).astype(np.int32)  # padded global id [N]

    # ---- edge geometry (host) ----
    src, dst = eidx[0], eidx[1]
    vec = pos[dst] - pos[src] + shifts
    r = np.linalg.norm(vec.astype(np.float64), axis=1).astype(f32)
    uvec = vec / np.maximum(r, 1e-9)[:, None]
    Y = _sph_np(uvec)                           # [E,16]
    ef = _radial_np(r)                          # [E,8]

    # ---- Ewald geometry (host) ----
    dot = pos @ kgrid.T                         # [N,K]
    sd = np.prod(np.sinc(0.5 * DELTA_K * pos), axis=1).astype(f32)   # [N]
    cosd = (sd[:, None] * np.cos(dot)).astype(f32)
    sind = (sd[:, None] * np.sin(dot)).astype(f32)

    kdown = krbf @ np.asarray(inputs['Wdown'], f32)      # [K,DP]

    # ---- edge -> (core, block) assignment, per-block tile counts ----
    gdst = batch[dst]
    kblk = slot[dst] // 128
    ecount = np.zeros((BG, NBLK), np.int64)
    np.add.at(ecount, (gdst, kblk), 1)
    T_list = [max(1, int(np.ceil(ecount[:, k].max() / 128))) for k in range(NBLK)]
    if SCAT_DR:
        T_list = [t + (t % 2) for t in T_list]
    O_list = np.concatenate([[0], np.cumsum(T_list)]).astype(int)
    NT = int(O_list[-1])
    G4 = ((NT + 3) // 4) * 4

    # ---- shared (replicated) weight arrays ----
    g = lambda k: np.asarray(inputs[k], f32)
    shared = {'Wembed': g('W_embed'),
              'ident': np.eye(128, dtype=f32),
              'Wr0': g('Wr0'), 'Wr1a': g('Wr1a'), 'Wr1b': g('Wr1b')}
    # layer-0 hu is weight-only (h0 = attrs @ Wembed): precompute the full
    # gathered table on the host, killing the first AllGather.
    h0_full = attrs @ g('W_embed')                       # [N, C]
    huG0 = np.zeros((BG * NL, C), f32)
    huG0[pid] = h0_full @ g('Wup')[0]
    huG0 = huG0.astype(bf16)
    for i in range(L):
        for nm in ('Wpre1', 'Wpre2', 'Wm1', 'Wm2', 'Wup', 'Wmix'):
            shared[f'{nm}_{i}'] = g(nm)[i]
        shared[f'rW1_{i}'] = g('rW1')[i]
        shared[f'rW2_{i}'] = g('rW2')[i]
        shared[f'rW3_{i}'] = g('rW3')[i]
        # rW4 reshaped l-major: [64, l*128 + c]
        shared[f'rW4_{i}'] = g('rW4')[i].reshape(64, C, 4).transpose(0, 2, 1).reshape(64, 4 * C)
        for nm in ('bpre1', 'bpre2', 'bm1', 'bm2'):
            shared[f'{nm}_{i}'] = g(nm)[i].reshape(C, 1)
        for nm in ('rb1', 'rb2', 'rb3'):
            shared[f'{nm}_{i}'] = g(nm)[i].reshape(64, 1)
        kf = np.zeros((KPAD, C), f32)
        kf[:K] = 0.01 * (kdown @ g('WupE')[i])
        shared[f'kfilt_{i}'] = kf
        shared[f'w2T_{i}'] = g('w2')[i].T.copy()             # [C,4] f32
        shared[f'w3T_{i}'] = g('w3')[i].T.copy()

    layouts = _const_layouts(G4)

    # ---- per-core arrays ----
    in_maps = []
    for b in range(BG):
        sl = slice(starts[b], ends[b])
        per = {}
        slot_b = slot[sl]
        at = np.zeros((NEL, NL), f32)
        at[:, slot_b] = attrs[sl].T
        per['attrsT'] = at
        cam = np.zeros((128, NBLK * KPAD), f32)   # atom-major cosd, per block
        sam = np.zeros((128, NBLK * KPAD), f32)
        ckm = np.zeros((KPAD, NL), f32)           # k-major
        skm = np.zeros((KPAD, NL), f32)
        pr, bb = slot_b % 128, slot_b // 128
        cam.reshape(128, NBLK, KPAD)[pr, bb, :K] = cosd[sl]
        sam.reshape(128, NBLK, KPAD)[pr, bb, :K] = sind[sl]
        ckm[:K, slot_b] = cosd[sl].T
        skm[:K, slot_b] = sind[sl].T
        per['cosdam'], per['sindam'] = cam, sam
        per['cosdkm'], per['sindkm'] = ckm, skm

        efp = np.zeros((8, G4 * 128), f32)
        sip = np.zeros((128, NT), np.int32)
        segY = np.zeros((128, NT * 16 * 128), f32)
        emask = gdst == b
        for k in range(NBLK):
            es = np.nonzero(emask & (kblk == k))[0]
            es = es[np.argsort(slot[dst[es]], kind='stable')]
            s = np.arange(len(es))
            tt, p = s // 128, s % 128
            t = O_list[k] + tt
            efp[:, t * 128 + p] = ef[es].T
            sip[p, t] = pid[src[es]]
            a = slot[dst[es]] - k * 128
            base = t * 2048 + a
            for lm in range(16):
                segY[p, base + lm * 128] = Y[es, lm] / AVG_NEI
        per['efTpack'] = efp

        def pack(entries, np_dt):
            width = sum(e[2] for e in entries)
            arr = np.zeros((128, width), np_dt)
            c0 = 0
            for name, rows, cols in entries:
                src_a = per.get(name, shared.get(name))
                arr[0:rows, c0:c0 + cols] = src_a
                c0 += cols
            return arr

        m = {'srcidx': sip, 'segYpack': segY.astype(segy_np), 'huG0': huG0,
             'cbA0': pack(layouts['cbA0'], bf16), 'cbA': pack(layouts['cbA'], bf16),
             'cbB': pack(layouts['cbB'], bf16), 'cf': pack(layouts['cf'], f32)}
        in_maps.append(m)

    e0 = np.zeros(BG, f32)
    ae = attrs @ np.asarray(inputs['atomic_E'], f32)
    for b in range(BG):
        e0[b] = ae[starts[b]:ends[b]].sum()
    return in_maps, T_list, G4, e0


# ---------------------------------------------------------------- device
def build_kernel(T_list, G4):
    import concourse.bass as bass
    import concourse.bacc as bacc
    import concourse.mybir as mybir
    import concourse.tile as tile

    f32 = mybir.dt.float32
    bf16 = mybir.dt.bfloat16
    sdt = mybir.dt.float8e4 if SEGY_FP8 else bf16
    A = mybir.ActivationFunctionType
    NT = int(sum(T_list))
    Tmax = max(T_list)
    O_list = np.concatenate([[0], np.cumsum(T_list)]).astype(int)
    nc = bacc.Bacc("TRN2", target_bir_lowering=False, debug=False, num_devices=BG)

    dins = {}
    def din(name, shape, dt=f32):
        dins[name] = nc.dram_tensor(name, list(shape), dt, kind="ExternalInput").ap()
        return dins[name]

    # load order = SP queue order: the layer-0 critical path first
    layouts = _const_layouts(G4)
    widths = {buf: sum(e[2] for e in entries) for buf, entries in layouts.items()}
    din('srcidx', (128, NT), mybir.dt.int32)
    din('cbA0', (128, widths['cbA0']), bf16)
    din('cf', (128, widths['cf']))
    din('cbB', (128, widths['cbB']), bf16)
    din('cbA', (128, widths['cbA']), bf16)
    segY_d = din('segYpack', (128, NT * 2048), sdt)
    huG0_d = din('huG0', (BG * NL, C), bf16)
    en_out = nc.dram_tensor('en_out', [1, 1], f32, kind="ExternalOutput").ap()

    with tile.TileContext(nc) as tc:
        with (
            tc.tile_pool(name="const", bufs=1) as cp,
            tc.tile_pool(name="work", bufs=2) as wp,
            tc.tile_pool(name="segy", bufs=5) as sy,
            tc.tile_pool(name="big", bufs=1) as bp,
            tc.tile_pool(name="psA", bufs=1, space="PSUM") as psA,
            tc.tile_pool(name="psS", bufs=2, space="PSUM") as psS,
            tc.tile_pool(name="psW", bufs=2, space="PSUM") as psW,
            tc.tile_pool(name="dram", bufs=1, space="DRAM") as dp,
        ):
            sb = {}
            for name in ('srcidx', 'cbA0', 'cf', 'cbB', 'cbA'):
                ap = dins[name]
                t = cp.tile(list(ap.shape), ap.dtype, tag=name)
                nc.sync.dma_start(t[:], ap[:])
                if name == 'srcidx':
                    sb[name] = t
                else:
                    c0 = 0
                    for nm, rows, cols in layouts[name]:
                        sb[nm] = t[0:rows, c0:c0 + cols]
                        c0 += cols

            h = bp.tile([C, NL], bf16, tag="h")
            en = bp.tile([1, 1], f32, tag="en")
            feats_cm = bp.tile([C, NL], bf16, tag="feats_cm")
            hres_am = bp.tile([128, NBLK * 128], bf16, tag="hres_am")
            nc.vector.memset(en[:], 0.0)

            pe = psW.tile([C, NL], f32, tag="pb")
            nc.tensor.matmul(pe[:], sb['Wembed'][:], sb['attrsT'][:], start=True, stop=True)
            nc.scalar.activation(h[:], pe[:], A.Copy)

            hu_dt = mybir.dt.float8e4 if HU_FP8 else bf16
            coll = {}     # layer -> (huL, huG) for layers >= 1
            for i in range(L):
                # ---- gather source: host table (layer 0) or prior AllGather ----
                huG = huG0_d if i == 0 else coll[i][1][:]
                hugs = []
                for k in range(NBLK):
                    lo, hi = int(O_list[k]), int(O_list[k + 1])
                    hg = wp.tile([128, Tmax * 128], bf16 if i == 0 else hu_dt,
                                 tag=f"hug{k % 2}{i}", bufs=1)
                    head = min(4, hi - lo) if k == 0 else hi - lo
                    nc.gpsimd.indirect_dma_start(
                        out=hg[:, 0:head * 128], out_offset=None, in_=huG[:],
                        in_offset=bass.IndirectOffsetOnAxis(
                            ap=sb['srcidx'][:, lo:lo + head], axis=0))
                    if head < hi - lo:
                        nc.gpsimd.indirect_dma_start(
                            out=hg[:, head * 128:(hi - lo) * 128], out_offset=None,
                            in_=huG[:],
                            in_offset=bass.IndirectOffsetOnAxis(
                                ap=sb['srcidx'][:, lo + head:hi], axis=0))
                    hugs.append(hg)
                if i + 1 < L:
                    huL_next = dp.tile([NL, C], hu_dt, tag=f"huL{i + 1}")
                    huG_next = dp.tile([BG * NL, C], hu_dt, tag=f"huG{i + 1}",
                                       addr_space="Shared")
                    hu_am = wp.tile([128, NL], hu_dt, tag="hu_am")
                    coll[i + 1] = (huL_next, huG_next)

                # ---- radial MLP: issued just-in-time, one 4-tile group ahead
                # of the edge loop (the matmul->silu chain is ~3us latency and
                # would serialize as a phase; interleaved it hides behind the
                # per-tile scatter work).
                s3sb = wp.tile([64, G4 * 128], bf16, tag="s3sb")
                radial_next = [0]

                def radial_group(gidx, i=i, s3sb=s3sb):
                    gsl = slice(gidx * 512, (gidx + 1) * 512)
                    pr1 = psS.tile([128, 512], f32, tag="ps")
                    nc.tensor.matmul(pr1[0:64, :], sb[f'rW1_{i}'][:], sb['efTpack'][:, gsl],
                                     start=True, stop=True)
                    s1 = wp.tile([64, 512], bf16, tag="s1")
                    nc.scalar.activation(s1[:], pr1[0:64, :], A.Silu, bias=sb[f'rb1_{i}'][:])
                    pr2 = psS.tile([128, 512], f32, tag="ps")
                    nc.tensor.matmul(pr2[0:64, :], sb[f'rW2_{i}'][:], s1[:], start=True, stop=True)
                    s2 = wp.tile([64, 512], bf16, tag="s1")
                    nc.scalar.activation(s2[:], pr2[0:64, :], A.Silu, bias=sb[f'rb2_{i}'][:])
                    pr3 = psS.tile([128, 512], f32, tag="ps")
                    nc.tensor.matmul(pr3[0:64, :], sb[f'rW3_{i}'][:], s2[:], start=True, stop=True)
                    nc.scalar.activation(s3sb[:, gsl], pr3[0:64, :], A.Silu, bias=sb[f'rb3_{i}'][:])

                def ensure_radial(gwant):
                    while radial_next[0] <= min(gwant, G4 // 4 - 1):
                        radial_group(radial_next[0])
                        radial_next[0] += 1

                # ---- Ewald block (independent of the collective) ----
                p1 = psW.tile([C, NL], f32, tag="pb")
                nc.tensor.matmul(p1[:], sb[f'Wpre1_{i}'][:], h[:], start=True, stop=True)
                t1 = wp.tile([C, NL], bf16, tag="t1")
                nc.scalar.activation(t1[:], p1[:], A.Silu, bias=sb[f'bpre1_{i}'][:])
                p2 = psW.tile([C, NL], f32, tag="pb")
                nc.tensor.matmul(p2[:], sb[f'Wpre2_{i}'][:], t1[:], start=True, stop=True)
                hres = wp.tile([C, NL], bf16, tag="hres")
                nc.vector.tensor_scalar_add(hres[:], p2[:], sb[f'bpre2_{i}'][:])
                nc.vector.tensor_add(hres[:], hres[:], h[:])
                for k in range(NBLK):
                    pt = psS.tile([128, 512], f32, tag="ps")
                    ptb = pt[:].bitcast(bf16)[:, 0:128]
                    nc.tensor.transpose(ptb, hres[:, k * 128:(k + 1) * 128], sb['ident'][:])
                    nc.scalar.activation(hres_am[:, k * 128:(k + 1) * 128], ptb, A.Copy)
                sfk = {}
                for nm, am in (('r', 'cosdam'), ('i', 'sindam')):
                    psf = psS.tile([128, 512], f32, tag="ps")
                    for k in range(NBLK):
                        nc.tensor.matmul(psf[:, 0:128], sb[am][:, k * KPAD:(k + 1) * KPAD],
                                         hres_am[:, k * 128:(k + 1) * 128],
                                         start=(k == 0), stop=(k == NBLK - 1))
                    s = wp.tile([KPAD, C], bf16, tag=f"sfk{nm}")
                    nc.vector.tensor_tensor(s[:], psf[:, 0:128], sb[f'kfilt_{i}'][:],
                                            op=mybir.AluOpType.mult)
                    sfk[nm] = s
                phe = psW.tile([C, NL], f32, tag="pb")
                nc.tensor.matmul(phe[:], sfk['r'][:], sb['cosdkm'][:], start=True, stop=False)
                nc.tensor.matmul(phe[:], sfk['i'][:], sb['sindkm'][:], start=False, stop=True)
                he0 = wp.tile([C, NL], bf16, tag="he0")
                nc.scalar.activation(he0[:], phe[:], A.Copy)
                pm1 = psW.tile([C, NL], f32, tag="pb")
                nc.tensor.matmul(pm1[:], sb[f'Wm1_{i}'][:], he0[:], start=True, stop=True)
                tm = wp.tile([C, NL], bf16, tag="t1")
                nc.scalar.activation(tm[:], pm1[:], A.Silu, bias=sb[f'bm1_{i}'][:])
                pm2 = psW.tile([C, NL], f32, tag="pb")
                nc.tensor.matmul(pm2[:], sb[f'Wm2_{i}'][:], tm[:], start=True, stop=True)
                he2 = wp.tile([C, NL], bf16, tag="he2")
                nc.scalar.activation(he2[:], pm2[:], A.Silu, bias=sb[f'bm2_{i}'][:])
                # layer 0: start the radial chains right after the dense
                # phase; layer >0: PE is idle under the AllGather, so run the
                # whole radial pipeline there.
                ensure_radial(G4 // 4 - 1 if i > 0 else 1)

                # ---- edge loop ----
                def issue_pair(k, tt0, alt, i=i, hugs=hugs):
                    # one 2-tile unit: paired segY DMA, two rW4 matmuls, two mw
                    # products written fp8 into one [128,1024] tile for the
                    # DoubleRow scatter. Second mw alternates DVE/gpsimd.
                    t = int(O_list[k]) + tt0
                    ensure_radial((t + 1) // 4 + 2)
                    sgt = sy.tile([128, 4096], sdt, tag="sg")
                    nc.sync.dma_start(sgt[:], segY_d[:, t * 2048:(t + 2) * 2048])
                    mw = wp.tile([128, 1024], sdt, tag="mw")
                    for j in range(2):
                        pw = psW.tile([C, NL], f32, tag="pb")
                        nc.tensor.matmul(pw[:], s3sb[:, (t + j) * 128:(t + j + 1) * 128],
                                         sb[f'rW4_{i}'][:], start=True, stop=True)
                        nc.vector.tensor_tensor(
                            mw[:, j * 512:(j + 1) * 512].rearrange("p (l c) -> p l c", l=4),
                            pw[:].rearrange("p (l c) -> p l c", l=4),
                            hugs[k][:, (tt0 + j) * 128:(tt0 + j + 1) * 128]
                                .unsqueeze(1).broadcast_to([128, 4, 128]),
                            op=mybir.AluOpType.mult)
                    return mw, sgt

                PAIRS = [(k, tt0) for k in range(NBLK) for tt0 in range(0, int(T_list[k]), 2)]
                LEAD = 1
                fifo = [issue_pair(*PAIRS[j], alt=(j % 2 == 0)) for j in range(LEAD)]
                tidx = [0]

                def next_mw_sgt():
                    j = tidx[0]
                    if j + LEAD < len(PAIRS):
                        fifo.append(issue_pair(*PAIRS[j + LEAD], alt=(j % 2 == 0)))
                    tidx[0] += 1
                    return fifo.pop(0)

                def make_tail(k, i=i, he2=he2):
                    # layer tail for block k: h update, readout, next layer's hu.
                    # Deferred into the next block's tile stream so the product
                    # basis (DVE) overlaps the next block's scatters (PE).
                    def tail():
                        blk = slice(k * 128, (k + 1) * 128)
                        pmx = psW.tile([C, NL], f32, tag="pb")
                        nc.tensor.matmul(pmx[:, 0:128], sb[f'Wmix_{i}'][:],
                                         feats_cm[:, blk], start=True, stop=True)
                        hnk = wp.tile([C, 128], f32, tag="hn")
                        eng = nc.vector if k == NBLK - 1 else nc.gpsimd
                        nc.vector.tensor_add(hnk[:], pmx[:, 0:128], he2[:, blk])
                        eng.tensor_add(hnk[:], hnk[:], h[:, blk])
                        eng.tensor_scalar_mul(h[:, blk], hnk[:], float(SKIP))
                        if i == 0:
                            prd = psS.tile([128, 512], f32, tag="ps")
                            nc.tensor.matmul(prd[0:1, 0:128], sb['Wr0'][:], h[:, blk],
                                             start=True, stop=True)
                            rs = wp.tile([1, 1], f32, tag="rs")
                            nc.vector.reduce_sum(rs[:], prd[0:1, 0:128],
                                                 axis=mybir.AxisListType.X)
                            nc.vector.tensor_add(en[:], en[:], rs[:])
                        else:
                            pra = psS.tile([128, 512], f32, tag="ps")
                            nc.tensor.matmul(pra[0:16, 0:128], sb['Wr1a'][:], h[:, blk],
                                             start=True, stop=True)
                            ta = wp.tile([16, 128], bf16, tag="ta")
                            nc.scalar.activation(ta[:], pra[0:16, 0:128], A.Silu)
                            prb = psS.tile([128, 512], f32, tag="ps")
                            nc.tensor.matmul(prb[0:1, 0:128], sb['Wr1b'][:], ta[:],
                                             start=True, stop=True)
                            rs = wp.tile([1, 1], f32, tag="rs")
                            nc.vector.reduce_sum(rs[:], prb[0:1, 0:128],
                                                 axis=mybir.AxisListType.X)
                            nc.vector.tensor_add(en[:], en[:], rs[:])
                        if i + 1 < L:
                            ph = psW.tile([C, NL], f32, tag="pb")
                            nc.tensor.matmul(ph[:, 0:128], h[:, blk],
                                             sb[f'Wup_{i + 1}'][:], start=True, stop=True)
                            nc.scalar.activation(hu_am[:, blk], ph[:, 0:128], A.Copy)
                            nc.sync.dma_start(huL_next[k * 128:(k + 1) * 128, :],
                                              hu_am[:, blk])
                    return tail

                def make_pb(k, scal, AA, i=i):
                    # product basis for block k (c-major), reading only SBUF.
                    # Deferred into the next block's pair stream so its DVE ops
                    # don't sit between consecutive blocks' mw ops.
                    def pb():
                        AA3 = AA[:].rearrange("c (m a) -> c a m", m=16)
                        inv = wp.tile([128, 512], f32, tag="inv")
                        nc.vector.tensor_copy(inv[:, 0:128], AA3[:, :, 0])
                        nc.vector.reduce_sum(inv[:, 128:256].unsqueeze(2), AA3[:, :, 1:4],
                                             axis=mybir.AxisListType.X)
                        for l in (2, 3):
                            isl = inv[:, l * 128:(l + 1) * 128]
                            m0, wl = L_START[l], L_WIDTH[l]
                            nc.gpsimd.tensor_tensor(isl, AA3[:, :, m0], AA3[:, :, m0 + 1],
                                                    op=mybir.AluOpType.add)
                            for mm in range(m0 + 2, m0 + wl):
                                nc.gpsimd.tensor_tensor(isl, isl, AA3[:, :, mm],
                                                        op=mybir.AluOpType.add)
                        acc = {}
                        for wnm, eng in (('w2T', nc.vector), ('w3T', nc.gpsimd)):
                            t2 = wp.tile([128, 512], f32, tag=f"t2{wnm}")
                            eng.tensor_tensor(
                                t2[:].rearrange("c (l a) -> c l a", l=4),
                                inv[:].rearrange("c (l a) -> c l a", l=4),
                                sb[f'{wnm}_{i}'][:].unsqueeze(2).broadcast_to([128, 4, 128]),
                                op=mybir.AluOpType.mult)
                            ac = wp.tile([128, 128], f32, tag=f"ac{wnm}")
                            if eng is nc.vector:
                                eng.reduce_sum(ac[:].unsqueeze(2),
                                               t2[:].rearrange("c (l a) -> c a l", l=4),
                                               axis=mybir.AxisListType.X)
                            else:
                                t23 = t2[:].rearrange("c (l a) -> c l a", l=4)
                                eng.tensor_tensor(ac[:], t23[:, 0, :], t23[:, 1, :],
                                                  op=mybir.AluOpType.add)
                                eng.tensor_tensor(ac[:], ac[:], t23[:, 2, :],
                                                  op=mybir.AluOpType.add)
                                eng.tensor_tensor(ac[:], ac[:], t23[:, 3, :],
                                                  op=mybir.AluOpType.add)
                            acc[wnm] = ac
                        fe = wp.tile([128, 128], f32, tag="fe")
                        nc.vector.tensor_tensor(fe[:], scal[:], acc['w3T'][:],
                                                op=mybir.AluOpType.mult)
                        nc.vector.tensor_add(fe[:], fe[:], acc['w2T'][:])
                        nc.vector.tensor_tensor(feats_cm[:, k * 128:(k + 1) * 128], fe[:],
                                                scal[:], op=mybir.AluOpType.add)
                    return pb

                pending = []
                for k in range(NBLK):
                    Tk = int(T_list[k])
                    pA1 = psA.tile([128, 1024], f32, tag="pA1")
                    pA2 = psA.tile([128, 1024], f32, tag="pA2")
                    for pp in range(Tk // 2):
                        mw, sgt = next_mw_sgt()
                        mw3 = mw[:].rearrange("p (two x) -> p two x", two=2)
                        sg3 = sgt[:].rearrange("p (two x) -> p two x", two=2)
                        for (l, m0, w) in CHUNKS:
                            pAh, off = (pA1, m0) if m0 < 8 else (pA2, m0 - 8)
                            nc.tensor.matmul(pAh[:, off * 128:(off + w) * 128],
                                             mw3[:, :, l * 128:(l + 1) * 128],
                                             sg3[:, :, m0 * 128:(m0 + w) * 128],
                                             start=(pp == 0), stop=(pp == Tk // 2 - 1),
                                             perf_mode=mybir.MatmulPerfMode.DoubleRow)
                        if pending and pp == min(1, Tk // 2 - 1):
                            pending.pop(0)()            # previous block's PB
                        if pending and pp == min(3, Tk // 2 - 1):
                            pending.pop(0)()            # previous block's tail
                    # free pA early: AA2 alone frees pA2 (whose half is written
                    # first by the next block), then AA1 + scal free pA1.
                    AA = wp.tile([128, 2048], bf16, tag="AA")
                    nc.scalar.activation(AA[:, 1024:2048], pA2[:], A.Square)
                    nc.scalar.activation(AA[:, 0:1024], pA1[:], A.Square)
                    scal = wp.tile([128, 128], bf16, tag="scal")
                    nc.scalar.activation(scal[:], pA1[:, 0:128], A.Copy)
                    pending = [make_pb(k, scal, AA), make_tail(k)]
                for c in pending:
                    c()
                if i + 1 < L:
                    cin, cout = huL_next[:], huG_next[:]
                    if HU_FP8:
                        cin, cout = cin.bitcast(bf16), cout.bitcast(bf16)
                    nc.gpsimd.collective_compute(
                        "AllGather", mybir.AluOpType.bypass,
                        replica_groups=[list(range(BG))],
                        ins=[cin.opt()], outs=[cout.opt()])
            nc.sync.dma_start(en_out[:], en[:])
    nc.compile()
    return nc


def kernel(**inputs):
    from concourse import bass_utils
    in_maps, T_list, G4, e0 = host_prep(inputs)
    key = (tuple(T_list), G4)
    if key not in _CACHE:
        _CACHE[key] = build_kernel(T_list, G4)
    nc = _CACHE[key]
    res = bass_utils.run_bass_kernel_spmd(nc, in_maps, core_ids=list(range(BG)))
    energy = np.zeros(BG, np.float32)
    for b in range(BG):
        energy[b] = res.results[b]['en_out'].reshape(-1)[0] + e0[b]
    return energy


# revision 38
# speedup vs baseline: 1.1343x; 1.0126x over previous
"""MACE+Ewald forward on 8 Trainium2 NeuronCores.

Sharding: graph-per-core (8 graphs, 8 cores). Atoms balanced across 4 blocks
of <=128 slots (padded NL=512 per core); edges assigned to the core/block
owning their dst atom, packed into 128-edge tiles with per-block tile counts.

Key device-side structure per layer:
  1. Layer 0's gather table huG0 = (attrs@Wembed)@Wup_0 is weight-only and
     precomputed on the host, so only ONE AllGather remains (layer 1's hu,
     fp8 payload bitcast to bf16 for transport, Shared output). It is kicked
     from the previous layer's per-block tails and overlapped by the Ewald
     block plus an eager radial-MLP pipeline.
  2. Ewald: structure factors / he MLP, all bf16 matmuls.
  3. Radial MLP issued just-in-time one 4-tile group ahead of the edge loop
     (as a phase its matmul->silu chain latency would serialize).
  4. Edge loop: one batched indirect gather of hu rows per block; edge tiles
     are processed in PAIRS: the per-(channel,l) weights times gathered hu
     (mw, fp8) is the stationary operand and a host-precomputed segY matrix
     (one-hot dst scatter with spherical harmonics Y and 1/avg_nei folded
     in, fp8) is the moving operand of DoubleRow matmuls that contract 256
     edges per pass, c-major output so no transposes are needed afterwards.
     rW4 products are issued via a lead-2 FIFO so the DVE mw latency hides.
  5. Product basis (A^2 contractions) per block split across DVE/gpsimd,
     PSUM freed early through scalar-engine copies; h update, readout and
     the next layer's hu are deferred into the next block's tile stream.

All matmuls run bf16 or fp8 operands with fp32 PSUM accumulation (the
harness tolerance is 2e-2; measured error stays ~2.6e-3). Constants load
as three fused buffers (one DMA each) to cut HWDGE issue latency.
"""

import numpy as np
import ml_dtypes

C = 128
L = 2
NB = 8
NEL = 10
BG = 8
N_ATOMS = 3200
N_EDGES = 51200
R_MAX = 5.0
P_CUT = 5.0
AVG_NEI = 16.0
DELTA_K = 0.2
NKRBF = 128
DP = 8
SKIP = (2.0 + 1.0) ** -0.5
NL = 512            # padded atoms per core
NBLK = NL // 128    # atom blocks per core
KPAD = 128          # padded k-point count (real: 123)
LOFLM = np.repeat(np.arange(4), [1, 3, 5, 7])   # [16]
L_START = [0, 1, 4, 9]
L_WIDTH = [1, 3, 5, 7]
# scatter matmul chunks: (l, first lm, number of lm) with moving-free <= 512
CHUNKS = [(2, 8, 1), (3, 9, 4), (3, 13, 3), (0, 0, 1), (1, 1, 3), (2, 4, 4)]
SEGY_FP8 = True
HU_FP8 = True
SCAT_DR = True   # fp8 DoubleRow scatter (2 edge tiles per PE pass)      # layer>=1 hu AllGather + gather in fp8e4m3

_CACHE = {}


def _const_layouts(G4):
    """Constant packing: 3 fused SBUF-resident buffers loaded with one DMA
    each (HWDGE issue time for ~50 separate loads dominated kernel startup).
    cbA = layer-0 critical path, cbB = the rest, cf = fp32 smalls."""
    bfA0 = [('attrsT', NEL, NL), ('Wembed', NEL, C),
            ('rW1_0', NB, 64), ('rW2_0', 64, 64), ('rW3_0', 64, 64)]
    bfA = [('rW4_0', 64, 4 * C), ('efTpack', 8, G4 * 128)]
    bfB = [('Wpre1_0', C, C), ('Wpre2_0', C, C), ('Wm1_0', C, C), ('Wm2_0', C, C),
           ('Wmix_0', C, C), ('Wup_1', C, C),
           ('cosdam', 128, NBLK * KPAD), ('sindam', 128, NBLK * KPAD),
           ('cosdkm', KPAD, NL), ('sindkm', KPAD, NL),
           ('ident', 128, 128), ('Wr0', C, 1), ('Wr1a', C, 16), ('Wr1b', 16, 1),
           ('Wpre1_1', C, C), ('Wpre2_1', C, C), ('Wm1_1', C, C), ('Wm2_1', C, C),
           ('Wmix_1', C, C),
           ('rW1_1', NB, 64), ('rW2_1', 64, 64), ('rW3_1', 64, 64), ('rW4_1', 64, 4 * C)]
    cf = ([('kfilt_0', KPAD, C), ('kfilt_1', KPAD, C)]
          + [(f'w{j}T_{i}', C, 4) for i in range(L) for j in (2, 3)]
          + [(f'{nm}_{i}', C, 1) for i in range(L) for nm in ('bpre1', 'bpre2', 'bm1', 'bm2')]
          + [(f'{nm}_{i}', 64, 1) for i in range(L) for nm in ('rb1', 'rb2', 'rb3')])
    return {'cbA0': bfA0, 'cbA': bfA, 'cbB': bfB, 'cf': cf}


def unpack_consts(m, G4):
    """Recover named f32 views from a core's fused const buffers (for host_sim)."""
    out = {}
    for buf, entries in _const_layouts(G4).items():
        c0 = 0
        for name, rows, cols in entries:
            out[name] = np.asarray(m[buf][0:rows, c0:c0 + cols], np.float32)
            c0 += cols
    return out


# ---------------------------------------------------------------- host math
def _sph_np(u):
    x, y, z = u[:, 0], u[:, 1], u[:, 2]
    s3, s5, s15 = 3.0 ** 0.5, 5.0 ** 0.5, 15.0 ** 0.5
    c70, c105, c42, c7 = 70.0 ** 0.5 / 4.0, 105.0 ** 0.5, 42.0 ** 0.5 / 4.0, 7.0 ** 0.5 / 2.0
    comps = [np.ones_like(x),
             s3 * x, s3 * y, s3 * z,
             s15 * x * y, s15 * y * z, 0.5 * s5 * (3 * z * z - 1.0), s15 * x * z,
             0.5 * s15 * (x * x - y * y),
             c70 * y * (3 * x * x - y * y), c105 * x * y * z, c42 * y * (5 * z * z - 1.0),
             c7 * z * (5 * z * z - 3.0), c42 * x * (5 * z * z - 1.0),
             0.5 * c105 * z * (x * x - y * y), c70 * x * (x * x - 3 * y * y)]
    return np.stack(comps, axis=-1).astype(np.float32)


def _radial_np(r):
    n = np.arange(1, NB + 1, dtype=np.float32)
    rb = np.sqrt(2.0 / R_MAX) * np.sin(n * np.pi * r[:, None] / R_MAX) / np.maximum(r, 1e-9)[:, None]
    uu = np.clip(r / R_MAX, 0.0, 1.0)
    p = P_CUT
    env = 1.0 - (p + 1.0) * (p + 2.0) / 2.0 * uu ** 5 + p * (p + 2.0) * uu ** 6 - p * (p + 1.0) / 2.0 * uu ** 7
    env = env * (r < R_MAX)
    return (rb * env[:, None]).astype(np.float32)


def host_prep(inputs):
    """Build per-core padded arrays. Returns (in_maps, T_list, G4, e0)."""
    f32 = np.float32
    bf16 = ml_dtypes.bfloat16
    segy_np = ml_dtypes.float8_e4m3 if SEGY_FP8 else bf16
    pos = np.asarray(inputs['positions'], f32)
    attrs = np.asarray(inputs['node_attrs'], f32)
    shifts = np.asarray(inputs['shifts'], f32)
    eidx = np.asarray(inputs['edge_index']).astype(np.int64)
    batch = np.asarray(inputs['batch']).astype(np.int64)
    kgrid = np.asarray(inputs['kgrid'], f32)
    krbf = np.asarray(inputs['krbf'], f32)
    K = kgrid.shape[0]

    # per-graph contiguous atom ranges (batch is sorted)
    starts = np.searchsorted(batch, np.arange(BG))
    ends = np.searchsorted(batch, np.arange(BG), side='right')
    counts = ends - starts
    assert counts.max() <= NL, counts

    # balanced split of each graph's atoms into NBLK blocks of <=128 slots
    slot = np.zeros(N_ATOMS, np.int64)          # padded local slot per atom
    for b in range(BG):
        n = int(counts[b])
        base, rem = divmod(n, NBLK)
        sizes = [base + (k < rem) for k in range(NBLK)]
        assert max(sizes) <= 128
        cum = 0
        for k in range(NBLK):
            j = np.arange(cum, cum + sizes[k])
            slot[starts[b] + j] = k * 128 + (j - cum)
            cum += sizes[k]
    pid = (batch * NL + slot).astype(np.int32)  # padded global id [N]

    # ---- edge geometry (host) ----
    src, dst = eidx[0], eidx[1]
    vec = pos[dst] - pos[src] + shifts
    r = np.linalg.norm(vec.astype(np.float64), axis=1).astype(f32)
    uvec = vec / np.maximum(r, 1e-9)[:, None]
    Y = _sph_np(uvec)                           # [E,16]
    ef = _radial_np(r)                          # [E,8]

    # ---- Ewald geometry (host) ----
    dot = pos @ kgrid.T                         # [N,K]
    sd = np.prod(np.sinc(0.5 * DELTA_K * pos), axis=1).astype(f32)   # [N]
    cosd = (sd[:, None] * np.cos(dot)).astype(f32)
    sind = (sd[:, None] * np.sin(dot)).astype(f32)

    kdown = krbf @ np.asarray(inputs['Wdown'], f32)      # [K,DP]

    # ---- edge -> (core, block) assignment, per-block tile counts ----
    gdst = batch[dst]
    kblk = slot[dst] // 128
    ecount = np.zeros((BG, NBLK), np.int64)
    np.add.at(ecount, (gdst, kblk), 1)
    T_list = [max(1, int(np.ceil(ecount[:, k].max() / 128))) for k in range(NBLK)]
    if SCAT_DR:
        T_list = [t + (t % 2) for t in T_list]
    O_list = np.concatenate([[0], np.cumsum(T_list)]).astype(int)
    NT = int(O_list[-1])
    G4 = ((NT + 3) // 4) * 4

    # ---- shared (replicated) weight arrays ----
    g = lambda k: np.asarray(inputs[k], f32)
    shared = {'Wembed': g('W_embed'),
              'ident': np.eye(128, dtype=f32),
              'Wr0': g('Wr0'), 'Wr1a': g('Wr1a'), 'Wr1b': g('Wr1b')}
    # layer-0 hu is weight-only (h0 = attrs @ Wembed): precompute the full
    # gathered table on the host, killing the first AllGather.
    h0_full = attrs @ g('W_embed')                       # [N, C]
    huG0 = np.zeros((BG * NL, C), f32)
    huG0[pid] = h0_full @ g('Wup')[0]
    huG0 = huG0.astype(bf16)
    for i in range(L):
        for nm in ('Wpre1', 'Wpre2', 'Wm1', 'Wm2', 'Wup', 'Wmix'):
            shared[f'{nm}_{i}'] = g(nm)[i]
        shared[f'rW1_{i}'] = g('rW1')[i]
        shared[f'rW2_{i}'] = g('rW2')[i]
        shared[f'rW3_{i}'] = g('rW3')[i]
        # rW4 reshaped l-major: [64, l*128 + c]
        shared[f'rW4_{i}'] = g('rW4')[i].reshape(64, C, 4).transpose(0, 2, 1).reshape(64, 4 * C)
        for nm in ('bpre1', 'bpre2', 'bm1', 'bm2'):
            shared[f'{nm}_{i}'] = g(nm)[i].reshape(C, 1)
        for nm in ('rb1', 'rb2', 'rb3'):
            shared[f'{nm}_{i}'] = g(nm)[i].reshape(64, 1)
        kf = np.zeros((KPAD, C), f32)
        kf[:K] = 0.01 * (kdown @ g('WupE')[i])
        shared[f'kfilt_{i}'] = kf
        shared[f'w2T_{i}'] = g('w2')[i].T.copy()             # [C,4] f32
        shared[f'w3T_{i}'] = g('w3')[i].T.copy()

    layouts = _const_layouts(G4)

    # ---- per-core arrays ----
    in_maps = []
    for b in range(BG):
        sl = slice(starts[b], ends[b])
        per = {}
        slot_b = slot[sl]
        at = np.zeros((NEL, NL), f32)
        at[:, slot_b] = attrs[sl].T
        per['attrsT'] = at
        cam = np.zeros((128, NBLK * KPAD), f32)   # atom-major cosd, per block
        sam = np.zeros((128, NBLK * KPAD), f32)
        ckm = np.zeros((KPAD, NL), f32)           # k-major
        skm = np.zeros((KPAD, NL), f32)
        pr, bb = slot_b % 128, slot_b // 128
        cam.reshape(128, NBLK, KPAD)[pr, bb, :K] = cosd[sl]
        sam.reshape(128, NBLK, KPAD)[pr, bb, :K] = sind[sl]
        ckm[:K, slot_b] = cosd[sl].T
        skm[:K, slot_b] = sind[sl].T
        per['cosdam'], per['sindam'] = cam, sam
        per['cosdkm'], per['sindkm'] = ckm, skm

        efp = np.zeros((8, G4 * 128), f32)
        sip = np.zeros((128, NT), np.int32)
        segY = np.zeros((128, NT * 16 * 128), f32)
        emask = gdst == b
        for k in range(NBLK):
            es = np.nonzero(emask & (kblk == k))[0]
            es = es[np.argsort(slot[dst[es]], kind='stable')]
            s = np.arange(len(es))
            tt, p = s // 128, s % 128
            t = O_list[k] + tt
            efp[:, t * 128 + p] = ef[es].T
            sip[p, t] = pid[src[es]]
            a = slot[dst[es]] - k * 128
            base = t * 2048 + a
            for lm in range(16):
                segY[p, base + lm * 128] = Y[es, lm] / AVG_NEI
        per['efTpack'] = efp

        def pack(entries, np_dt):
            width = sum(e[2] for e in entries)
            arr = np.zeros((128, width), np_dt)
            c0 = 0
            for name, rows, cols in entries:
                src_a = per.get(name, shared.get(name))
                arr[0:rows, c0:c0 + cols] = src_a
                c0 += cols
            return arr

        m = {'srcidx': sip, 'segYpack': segY.astype(segy_np), 'huG0': huG0,
             'cbA0': pack(layouts['cbA0'], bf16), 'cbA': pack(layouts['cbA'], bf16),
             'cbB': pack(layouts['cbB'], bf16), 'cf': pack(layouts['cf'], f32)}
        in_maps.append(m)

    e0 = np.zeros(BG, f32)
    ae = attrs @ np.asarray(inputs['atomic_E'], f32)
    for b in range(BG):
        e0[b] = ae[starts[b]:ends[b]].sum()
    return in_maps, T_list, G4, e0


# ---------------------------------------------------------------- device
def build_kernel(T_list, G4):
    import concourse.bass as bass
    import concourse.bacc as bacc
    import concourse.mybir as mybir
    import concourse.tile as tile

    f32 = mybir.dt.float32
    bf16 = mybir.dt.bfloat16
    sdt = mybir.dt.float8e4 if SEGY_FP8 else bf16
    A = mybir.ActivationFunctionType
    NT = int(sum(T_list))
    Tmax = max(T_list)
    O_list = np.concatenate([[0], np.cumsum(T_list)]).astype(int)
    nc = bacc.Bacc("TRN2", target_bir_lowering=False, debug=False, num_devices=BG)

    dins = {}
    def din(name, shape, dt=f32):
        dins[name] = nc.dram_tensor(name, list(shape), dt, kind="ExternalInput").ap()
        return dins[name]

    # load order = SP queue order: the layer-0 critical path first
    layouts = _const_layouts(G4)
    widths = {buf: sum(e[2] for e in entries) for buf, entries in layouts.items()}
    din('srcidx', (128, NT), mybir.dt.int32)
    din('cbA0', (128, widths['cbA0']), bf16)
    din('cf', (128, widths['cf']))
    din('cbB', (128, widths['cbB']), bf16)
    din('cbA', (128, widths['cbA']), bf16)
    segY_d = din('segYpack', (128, NT * 2048), sdt)
    huG0_d = din('huG0', (BG * NL, C), bf16)
    en_out = nc.dram_tensor('en_out', [1, 1], f32, kind="ExternalOutput").ap()

    with tile.TileContext(nc) as tc:
        with (
            tc.tile_pool(name="const", bufs=1) as cp,
            tc.tile_pool(name="work", bufs=2) as wp,
            tc.tile_pool(name="segy", bufs=5) as sy,
            tc.tile_pool(name="big", bufs=1) as bp,
            tc.tile_pool(name="psA", bufs=1, space="PSUM") as psA,
            tc.tile_pool(name="psS", bufs=2, space="PSUM") as psS,
            tc.tile_pool(name="psW", bufs=2, space="PSUM") as psW,
            tc.tile_pool(name="dram", bufs=1, space="DRAM") as dp,
        ):
            sb = {}
            for name in ('srcidx', 'cbA0', 'cbB', 'cf', 'cbA'):
                ap = dins[name]
                t = cp.tile(list(ap.shape), ap.dtype, tag=name)
                nc.sync.dma_start(t[:], ap[:])
                if name == 'srcidx':
                    sb[name] = t
                else:
                    c0 = 0
                    for nm, rows, cols in layouts[name]:
                        sb[nm] = t[0:rows, c0:c0 + cols]
                        c0 += cols

            h = bp.tile([C, NL], bf16, tag="h")
            en = bp.tile([1, 1], f32, tag="en")
            feats_cm = bp.tile([C, NL], bf16, tag="feats_cm")
            hres_am = bp.tile([128, NBLK * 128], bf16, tag="hres_am")
            nc.vector.memset(en[:], 0.0)

            pe = psW.tile([C, NL], f32, tag="pb")
            nc.tensor.matmul(pe[:], sb['Wembed'][:], sb['attrsT'][:], start=True, stop=True)
            nc.scalar.activation(h[:], pe[:], A.Copy)

            hu_dt = mybir.dt.float8e4 if HU_FP8 else bf16
            coll = {}     # layer -> (huL, huG) for layers >= 1
            for i in range(L):
                # ---- gather source: host table (layer 0) or prior AllGather ----
                huG = huG0_d if i == 0 else coll[i][1][:]
                hugs = []
                for k in range(NBLK):
                    lo, hi = int(O_list[k]), int(O_list[k + 1])
                    hg = wp.tile([128, Tmax * 128], bf16 if i == 0 else hu_dt,
                                 tag=f"hug{k % 2}{i}", bufs=1)
                    head = min(4, hi - lo) if k == 0 else hi - lo
                    nc.gpsimd.indirect_dma_start(
                        out=hg[:, 0:head * 128], out_offset=None, in_=huG[:],
                        in_offset=bass.IndirectOffsetOnAxis(
                            ap=sb['srcidx'][:, lo:lo + head], axis=0))
                    if head < hi - lo:
                        nc.gpsimd.indirect_dma_start(
                            out=hg[:, head * 128:(hi - lo) * 128], out_offset=None,
                            in_=huG[:],
                            in_offset=bass.IndirectOffsetOnAxis(
                                ap=sb['srcidx'][:, lo + head:hi], axis=0))
                    hugs.append(hg)
                if i + 1 < L:
                    huL_next = dp.tile([NL, C], hu_dt, tag=f"huL{i + 1}")
                    huG_next = dp.tile([BG * NL, C], hu_dt, tag=f"huG{i + 1}",
                                       addr_space="Shared")
                    hu_am = wp.tile([128, NL], hu_dt, tag="hu_am")
                    coll[i + 1] = (huL_next, huG_next)

                # ---- radial MLP: issued just-in-time, one 4-tile group ahead
                # of the edge loop (the matmul->silu chain is ~3us latency and
                # would serialize as a phase; interleaved it hides behind the
                # per-tile scatter work).
                s3sb = wp.tile([64, G4 * 128], bf16, tag="s3sb")
                radial_next = [0]

                def radial_group(gidx, i=i, s3sb=s3sb):
                    gsl = slice(gidx * 512, (gidx + 1) * 512)
                    pr1 = psS.tile([128, 512], f32, tag="ps")
                    nc.tensor.matmul(pr1[0:64, :], sb[f'rW1_{i}'][:], sb['efTpack'][:, gsl],
                                     start=True, stop=True)
                    s1 = wp.tile([64, 512], bf16, tag="s1")
                    nc.scalar.activation(s1[:], pr1[0:64, :], A.Silu, bias=sb[f'rb1_{i}'][:])
                    pr2 = psS.tile([128, 512], f32, tag="ps")
                    nc.tensor.matmul(pr2[0:64, :], sb[f'rW2_{i}'][:], s1[:], start=True, stop=True)
                    s2 = wp.tile([64, 512], bf16, tag="s1")
                    nc.scalar.activation(s2[:], pr2[0:64, :], A.Silu, bias=sb[f'rb2_{i}'][:])
                    pr3 = psS.tile([128, 512], f32, tag="ps")
                    nc.tensor.matmul(pr3[0:64, :], sb[f'rW3_{i}'][:], s2[:], start=True, stop=True)
                    nc.scalar.activation(s3sb[:, gsl], pr3[0:64, :], A.Silu, bias=sb[f'rb3_{i}'][:])

                def ensure_radial(gwant):
                    while radial_next[0] <= min(gwant, G4 // 4 - 1):
                        radial_group(radial_next[0])
                        radial_next[0] += 1

                # ---- Ewald block (independent of the collective) ----
                p1 = psW.tile([C, NL], f32, tag="pb")
                nc.tensor.matmul(p1[:], sb[f'Wpre1_{i}'][:], h[:], start=True, stop=True)
                t1 = wp.tile([C, NL], bf16, tag="t1")
                nc.scalar.activation(t1[:], p1[:], A.Silu, bias=sb[f'bpre1_{i}'][:])
                p2 = psW.tile([C, NL], f32, tag="pb")
                nc.tensor.matmul(p2[:], sb[f'Wpre2_{i}'][:], t1[:], start=True, stop=True)
                hres = wp.tile([C, NL], bf16, tag="hres")
                nc.vector.tensor_scalar_add(hres[:], p2[:], sb[f'bpre2_{i}'][:])
                nc.vector.tensor_add(hres[:], hres[:], h[:])
                for k in range(NBLK):
                    pt = psS.tile([128, 512], f32, tag="ps")
                    ptb = pt[:].bitcast(bf16)[:, 0:128]
                    nc.tensor.transpose(ptb, hres[:, k * 128:(k + 1) * 128], sb['ident'][:])
                    nc.scalar.activation(hres_am[:, k * 128:(k + 1) * 128], ptb, A.Copy)
                sfk = {}
                for nm, am in (('r', 'cosdam'), ('i', 'sindam')):
                    psf = psS.tile([128, 512], f32, tag="ps")
                    for k in range(NBLK):
                        nc.tensor.matmul(psf[:, 0:128], sb[am][:, k * KPAD:(k + 1) * KPAD],
                                         hres_am[:, k * 128:(k + 1) * 128],
                                         start=(k == 0), stop=(k == NBLK - 1))
                    s = wp.tile([KPAD, C], bf16, tag=f"sfk{nm}")
                    nc.vector.tensor_tensor(s[:], psf[:, 0:128], sb[f'kfilt_{i}'][:],
                                            op=mybir.AluOpType.mult)
                    sfk[nm] = s
                phe = psW.tile([C, NL], f32, tag="pb")
                nc.tensor.matmul(phe[:], sfk['r'][:], sb['cosdkm'][:], start=True, stop=False)
                nc.tensor.matmul(phe[:], sfk['i'][:], sb['sindkm'][:], start=False, stop=True)
                he0 = wp.tile([C, NL], bf16, tag="he0")
                nc.scalar.activation(he0[:], phe[:], A.Copy)
                pm1 = psW.tile([C, NL], f32, tag="pb")
                nc.tensor.matmul(pm1[:], sb[f'Wm1_{i}'][:], he0[:], start=True, stop=True)
                tm = wp.tile([C, NL], bf16, tag="t1")
                nc.scalar.activation(tm[:], pm1[:], A.Silu, bias=sb[f'bm1_{i}'][:])
                pm2 = psW.tile([C, NL], f32, tag="pb")
                nc.tensor.matmul(pm2[:], sb[f'Wm2_{i}'][:], tm[:], start=True, stop=True)
                he2 = wp.tile([C, NL], bf16, tag="he2")
                nc.scalar.activation(he2[:], pm2[:], A.Silu, bias=sb[f'bm2_{i}'][:])
                # layer 0: start the radial chains right after the dense
                # phase; layer >0: PE is idle under the AllGather, so run the
                # whole radial pipeline there.
                ensure_radial(G4 // 4 - 1 if i > 0 else 1)

                # ---- edge loop ----
                def issue_pair(k, tt0, alt, i=i, hugs=hugs):
                    # one 2-tile unit: paired segY DMA, two rW4 matmuls, two mw
                    # products written fp8 into one [128,1024] tile for the
                    # DoubleRow scatter. Second mw alternates DVE/gpsimd.
                    t = int(O_list[k]) + tt0
                    ensure_radial((t + 1) // 4 + 2)
                    sgt = sy.tile([128, 4096], sdt, tag="sg")
                    nc.sync.dma_start(sgt[:], segY_d[:, t * 2048:(t + 2) * 2048])
                    mw = wp.tile([128, 1024], sdt, tag="mw", bufs=3)
                    for j in range(2):
                        pw = psW.tile([C, NL], f32, tag="pb")
                        nc.tensor.matmul(pw[:], s3sb[:, (t + j) * 128:(t + j + 1) * 128],
                                         sb[f'rW4_{i}'][:], start=True, stop=True)
                        nc.vector.tensor_tensor(
                            mw[:, j * 512:(j + 1) * 512].rearrange("p (l c) -> p l c", l=4),
                            pw[:].rearrange("p (l c) -> p l c", l=4),
                            hugs[k][:, (tt0 + j) * 128:(tt0 + j + 1) * 128]
                                .unsqueeze(1).broadcast_to([128, 4, 128]),
                            op=mybir.AluOpType.mult)
                    return mw, sgt

                PAIRS = [(k, tt0) for k in range(NBLK) for tt0 in range(0, int(T_list[k]), 2)]
                LEAD = 1
                fifo = [issue_pair(*PAIRS[j], alt=(j % 2 == 0)) for j in range(LEAD)]
                tidx = [0]

                def next_mw_sgt():
                    j = tidx[0]
                    if j + LEAD < len(PAIRS):
                        fifo.append(issue_pair(*PAIRS[j + LEAD], alt=(j % 2 == 0)))
                    tidx[0] += 1
                    return fifo.pop(0)

                def make_tail(k, i=i, he2=he2):
                    # layer tail for block k: h update, readout, next layer's hu.
                    # Deferred into the next block's tile stream so the product
                    # basis (DVE) overlaps the next block's scatters (PE).
                    def tail():
                        blk = slice(k * 128, (k + 1) * 128)
                        pmx = psW.tile([C, NL], f32, tag="pb")
                        nc.tensor.matmul(pmx[:, 0:128], sb[f'Wmix_{i}'][:],
                                         feats_cm[:, blk], start=True, stop=True)
                        hnk = wp.tile([C, 128], f32, tag="hn")
                        eng = nc.vector if k == NBLK - 1 else nc.gpsimd
                        nc.vector.tensor_add(hnk[:], pmx[:, 0:128], he2[:, blk])
                        eng.tensor_add(hnk[:], hnk[:], h[:, blk])
                        eng.tensor_scalar_mul(h[:, blk], hnk[:], float(SKIP))
                        if i == 0:
                            prd = psS.tile([128, 512], f32, tag="ps")
                            nc.tensor.matmul(prd[0:1, 0:128], sb['Wr0'][:], h[:, blk],
                                             start=True, stop=True)
                            rs = wp.tile([1, 1], f32, tag="rs")
                            nc.vector.reduce_sum(rs[:], prd[0:1, 0:128],
                                                 axis=mybir.AxisListType.X)
                            nc.vector.tensor_add(en[:], en[:], rs[:])
                        else:
                            pra = psS.tile([128, 512], f32, tag="ps")
                            nc.tensor.matmul(pra[0:16, 0:128], sb['Wr1a'][:], h[:, blk],
                                             start=True, stop=True)
                            ta = wp.tile([16, 128], bf16, tag="ta")
                            nc.scalar.activation(ta[:], pra[0:16, 0:128], A.Silu)
                            prb = psS.tile([128, 512], f32, tag="ps")
                            nc.tensor.matmul(prb[0:1, 0:128], sb['Wr1b'][:], ta[:],
                                             start=True, stop=True)
                            rs = wp.tile([1, 1], f32, tag="rs")
                            nc.vector.reduce_sum(rs[:], prb[0:1, 0:128],
                                                 axis=mybir.AxisListType.X)
                            nc.vector.tensor_add(en[:], en[:], rs[:])
                        if i + 1 < L:
                            ph = psW.tile([C, NL], f32, tag="pb")
                            nc.tensor.matmul(ph[:, 0:128], h[:, blk],
                                             sb[f'Wup_{i + 1}'][:], start=True, stop=True)
                            nc.scalar.activation(hu_am[:, blk], ph[:, 0:128], A.Copy)
                            nc.sync.dma_start(huL_next[k * 128:(k + 1) * 128, :],
                                              hu_am[:, blk])
                    return tail

                def make_pb(k, scal, AA, i=i):
                    # product basis for block k (c-major), reading only SBUF.
                    # Deferred into the next block's pair stream so its DVE ops
                    # don't sit between consecutive blocks' mw ops.
                    def pb():
                        AA3 = AA[:].rearrange("c (m a) -> c a m", m=16)
                        inv = wp.tile([128, 512], f32, tag="inv")
                        nc.vector.tensor_copy(inv[:, 0:128], AA3[:, :, 0])
                        nc.vector.reduce_sum(inv[:, 128:256].unsqueeze(2), AA3[:, :, 1:4],
                                             axis=mybir.AxisListType.X)
                        for l in (2, 3):
                            isl = inv[:, l * 128:(l + 1) * 128]
                            m0, wl = L_START[l], L_WIDTH[l]
                            nc.gpsimd.tensor_tensor(isl, AA3[:, :, m0], AA3[:, :, m0 + 1],
                                                    op=mybir.AluOpType.add)
                            for mm in range(m0 + 2, m0 + wl):
                                nc.gpsimd.tensor_tensor(isl, isl, AA3[:, :, mm],
                                                        op=mybir.AluOpType.add)
                        acc = {}
                        for wnm, eng in (('w2T', nc.vector), ('w3T', nc.gpsimd)):
                            t2 = wp.tile([128, 512], f32, tag=f"t2{wnm}")
                            eng.tensor_tensor(
                                t2[:].rearrange("c (l a) -> c l a", l=4),
                                inv[:].rearrange("c (l a) -> c l a", l=4),
                                sb[f'{wnm}_{i}'][:].unsqueeze(2).broadcast_to([128, 4, 128]),
                                op=mybir.AluOpType.mult)
                            ac = wp.tile([128, 128], f32, tag=f"ac{wnm}")
                            if eng is nc.vector:
                                eng.reduce_sum(ac[:].unsqueeze(2),
                                               t2[:].rearrange("c (l a) -> c a l", l=4),
                                               axis=mybir.AxisListType.X)
                            else:
                                t23 = t2[:].rearrange("c (l a) -> c l a", l=4)
                                eng.tensor_tensor(ac[:], t23[:, 0, :], t23[:, 1, :],
                                                  op=mybir.AluOpType.add)
                                eng.tensor_tensor(ac[:], ac[:], t23[:, 2, :],
                                                  op=mybir.AluOpType.add)
                                eng.tensor_tensor(ac[:], ac[:], t23[:, 3, :],
                                                  op=mybir.AluOpType.add)
                            acc[wnm] = ac
                        fe = wp.tile([128, 128], f32, tag="fe")
                        nc.vector.tensor_tensor(fe[:], scal[:], acc['w3T'][:],
                                                op=mybir.AluOpType.mult)
                        nc.vector.tensor_add(fe[:], fe[:], acc['w2T'][:])
                        nc.vector.tensor_tensor(feats_cm[:, k * 128:(k + 1) * 128], fe[:],
                                                scal[:], op=mybir.AluOpType.add)
                    return pb

                pending = []
                for k in range(NBLK):
                    Tk = int(T_list[k])
                    pA1 = psA.tile([128, 1024], f32, tag="pA1")
                    pA2 = psA.tile([128, 1024], f32, tag="pA2")
                    for pp in range(Tk // 2):
                        mw, sgt = next_mw_sgt()
                        mw3 = mw[:].rearrange("p (two x) -> p two x", two=2)
                        sg3 = sgt[:].rearrange("p (two x) -> p two x", two=2)
                        for (l, m0, w) in CHUNKS:
                            pAh, off = (pA1, m0) if m0 < 8 else (pA2, m0 - 8)
                            nc.tensor.matmul(pAh[:, off * 128:(off + w) * 128],
                                             mw3[:, :, l * 128:(l + 1) * 128],
                                             sg3[:, :, m0 * 128:(m0 + w) * 128],
                                             start=(pp == 0), stop=(pp == Tk // 2 - 1),
                                             perf_mode=mybir.MatmulPerfMode.DoubleRow)
                        if pending and pp == min(1, Tk // 2 - 1):
                            pending.pop(0)()            # previous block's PB
                        if pending and pp == min(3, Tk // 2 - 1):
                            pending.pop(0)()            # previous block's tail
                    # free pA early: AA2 alone frees pA2 (whose half is written
                    # first by the next block), then AA1 + scal free pA1.
                    AA = wp.tile([128, 2048], bf16, tag="AA")
                    nc.scalar.activation(AA[:, 1024:2048], pA2[:], A.Square)
                    nc.scalar.activation(AA[:, 0:1024], pA1[:], A.Square)
                    scal = wp.tile([128, 128], bf16, tag="scal")
                    nc.scalar.activation(scal[:], pA1[:, 0:128], A.Copy)
                    pending = [make_pb(k, scal, AA), make_tail(k)]
                for c in pending:
                    c()
                if i + 1 < L:
                    cin, cout = huL_next[:], huG_next[:]
                    if HU_FP8:
                        cin, cout = cin.bitcast(bf16), cout.bitcast(bf16)
                    nc.gpsimd.collective_compute(
                        "AllGather", mybir.AluOpType.bypass,
                        replica_groups=[list(range(BG))],
                        ins=[cin.opt()], outs=[cout.opt()])
            nc.sync.dma_start(en_out[:], en[:])
    nc.compile()
    return nc


def kernel(**inputs):
    from concourse import bass_utils
    in_maps, T_list, G4, e0 = host_prep(inputs)
    key = (tuple(T_list), G4)
    if key not in _CACHE:
        _CACHE[key] = build_kernel(T_list, G4)
    nc = _CACHE[key]
    res = bass_utils.run_bass_kernel_spmd(nc, in_maps, core_ids=list(range(BG)))
    energy = np.zeros(BG, np.float32)
    for b in range(BG):
        energy[b] = res.results[b]['en_out'].reshape(-1)[0] + e0[b]
    return energy


# revision 39
# speedup vs baseline: 1.1706x; 1.0320x over previous
"""MACE+Ewald forward on 8 Trainium2 NeuronCores.

Sharding: graph-per-core (8 graphs, 8 cores). Atoms balanced across 4 blocks
of <=128 slots (padded NL=512 per core); edges assigned to the core/block
owning their dst atom, packed into 128-edge tiles with per-block tile counts.

Key device-side structure per layer:
  1. Layer 0's gather table huG0 = (attrs@Wembed)@Wup_0 is weight-only and
     precomputed on the host, so only ONE AllGather remains (layer 1's hu,
     fp8 payload bitcast to bf16 for transport, Shared output). It is kicked
     from the previous layer's per-block tails and overlapped by the Ewald
     block plus an eager radial-MLP pipeline.
  2. Ewald: structure factors / he MLP, all bf16 matmuls.
  3. Radial MLP issued just-in-time one 4-tile group ahead of the edge loop
     (as a phase its matmul->silu chain latency would serialize).
  4. Edge loop: one batched indirect gather of hu rows per block; edge tiles
     are processed in PAIRS: the per-(channel,l) weights times gathered hu
     (mw, fp8) is the stationary operand and a host-precomputed segY matrix
     (one-hot dst scatter with spherical harmonics Y and 1/avg_nei folded
     in, fp8) is the moving operand of DoubleRow matmuls that contract 256
     edges per pass, c-major output so no transposes are needed afterwards.
     rW4 products are issued via a lead-2 FIFO so the DVE mw latency hides.
  5. Product basis (A^2 contractions) per block split across DVE/gpsimd,
     PSUM freed early through scalar-engine copies; h update, readout and
     the next layer's hu are deferred into the next block's tile stream.

All matmuls run bf16 or fp8 operands with fp32 PSUM accumulation (the
harness tolerance is 2e-2; measured error stays ~2.6e-3). Constants load
as three fused buffers (one DMA each) to cut HWDGE issue latency.
"""

import numpy as np
import ml_dtypes

C = 128
L = 2
NB = 8
NEL = 10
BG = 8
N_ATOMS = 3200
N_EDGES = 51200
R_MAX = 5.0
P_CUT = 5.0
AVG_NEI = 16.0
DELTA_K = 0.2
NKRBF = 128
DP = 8
SKIP = (2.0 + 1.0) ** -0.5
NL = 512            # padded atoms per core
NBLK = NL // 128    # atom blocks per core
KPAD = 128          # padded k-point count (real: 123)
LOFLM = np.repeat(np.arange(4), [1, 3, 5, 7])   # [16]
L_START = [0, 1, 4, 9]
L_WIDTH = [1, 3, 5, 7]
# scatter matmul chunks: (l, first lm, number of lm) with moving-free <= 512
CHUNKS = [(2, 8, 1), (3, 9, 4), (3, 13, 3), (0, 0, 1), (1, 1, 3), (2, 4, 4)]
SEGY_FP8 = True
HU_FP8 = True
SCAT_DR = True   # fp8 DoubleRow scatter (2 edge tiles per PE pass)      # layer>=1 hu AllGather + gather in fp8e4m3

_CACHE = {}


def _const_layouts(G4):
    """Constant packing: 3 fused SBUF-resident buffers loaded with one DMA
    each (HWDGE issue time for ~50 separate loads dominated kernel startup).
    cbA = layer-0 critical path, cbB = the rest, cf = fp32 smalls."""
    bfA0 = [('attrsT', NEL, NL), ('Wembed', NEL, C),
            ('rW1_0', NB, 64), ('rW2_0', 64, 64), ('rW3_0', 64, 64)]
    bfA = [('rW4_0', 64, 4 * C), ('efTpack', 8, G4 * 128)]
    bfB = [('Wpre1_0', C, C), ('Wpre2_0', C, C), ('Wm1_0', C, C), ('Wm2_0', C, C),
           ('Wmix_0', C, C), ('Wup_1', C, C),
           ('cosdam', 128, NBLK * KPAD), ('sindam', 128, NBLK * KPAD),
           ('cosdkm', KPAD, NL), ('sindkm', KPAD, NL),
           ('ident', 128, 128), ('Wr0', C, 1), ('Wr1a', C, 16), ('Wr1b', 16, 1),
           ('Wpre1_1', C, C), ('Wpre2_1', C, C), ('Wm1_1', C, C), ('Wm2_1', C, C),
           ('Wmix_1', C, C),
           ('rW1_1', NB, 64), ('rW2_1', 64, 64), ('rW3_1', 64, 64), ('rW4_1', 64, 4 * C)]
    cf = ([('kfilt_0', KPAD, C), ('kfilt_1', KPAD, C)]
          + [(f'w{j}T_{i}', C, 4) for i in range(L) for j in (2, 3)]
          + [(f'{nm}_{i}', C, 1) for i in range(L) for nm in ('bpre1', 'bpre2', 'bm1', 'bm2')]
          + [(f'{nm}_{i}', 64, 1) for i in range(L) for nm in ('rb1', 'rb2', 'rb3')])
    return {'cbA0': bfA0, 'cbA': bfA, 'cbB': bfB, 'cf': cf}


def unpack_consts(m, G4):
    """Recover named f32 views from a core's fused const buffers (for host_sim)."""
    out = {}
    for buf, entries in _const_layouts(G4).items():
        c0 = 0
        for name, rows, cols in entries:
            out[name] = np.asarray(m[buf][0:rows, c0:c0 + cols], np.float32)
            c0 += cols
    return out


# ---------------------------------------------------------------- host math
def _sph_np(u):
    x, y, z = u[:, 0], u[:, 1], u[:, 2]
    s3, s5, s15 = 3.0 ** 0.5, 5.0 ** 0.5, 15.0 ** 0.5
    c70, c105, c42, c7 = 70.0 ** 0.5 / 4.0, 105.0 ** 0.5, 42.0 ** 0.5 / 4.0, 7.0 ** 0.5 / 2.0
    comps = [np.ones_like(x),
             s3 * x, s3 * y, s3 * z,
             s15 * x * y, s15 * y * z, 0.5 * s5 * (3 * z * z - 1.0), s15 * x * z,
             0.5 * s15 * (x * x - y * y),
             c70 * y * (3 * x * x - y * y), c105 * x * y * z, c42 * y * (5 * z * z - 1.0),
             c7 * z * (5 * z * z - 3.0), c42 * x * (5 * z * z - 1.0),
             0.5 * c105 * z * (x * x - y * y), c70 * x * (x * x - 3 * y * y)]
    return np.stack(comps, axis=-1).astype(np.float32)


def _radial_np(r):
    n = np.arange(1, NB + 1, dtype=np.float32)
    rb = np.sqrt(2.0 / R_MAX) * np.sin(n * np.pi * r[:, None] / R_MAX) / np.maximum(r, 1e-9)[:, None]
    uu = np.clip(r / R_MAX, 0.0, 1.0)
    p = P_CUT
    env = 1.0 - (p + 1.0) * (p + 2.0) / 2.0 * uu ** 5 + p * (p + 2.0) * uu ** 6 - p * (p + 1.0) / 2.0 * uu ** 7
    env = env * (r < R_MAX)
    return (rb * env[:, None]).astype(np.float32)


def host_prep(inputs):
    """Build per-core padded arrays. Returns (in_maps, T_list, G4, e0)."""
    f32 = np.float32
    bf16 = ml_dtypes.bfloat16
    segy_np = ml_dtypes.float8_e4m3 if SEGY_FP8 else bf16
    pos = np.asarray(inputs['positions'], f32)
    attrs = np.asarray(inputs['node_attrs'], f32)
    shifts = np.asarray(inputs['shifts'], f32)
    eidx = np.asarray(inputs['edge_index']).astype(np.int64)
    batch = np.asarray(inputs['batch']).astype(np.int64)
    kgrid = np.asarray(inputs['kgrid'], f32)
    krbf = np.asarray(inputs['krbf'], f32)
    K = kgrid.shape[0]

    # per-graph contiguous atom ranges (batch is sorted)
    starts = np.searchsorted(batch, np.arange(BG))
    ends = np.searchsorted(batch, np.arange(BG), side='right')
    counts = ends - starts
    assert counts.max() <= NL, counts

    # balanced split of each graph's atoms into NBLK blocks of <=128 slots
    slot = np.zeros(N_ATOMS, np.int64)          # padded local slot per atom
    for b in range(BG):
        n = int(counts[b])
        base, rem = divmod(n, NBLK)
        sizes = [base + (k < rem) for k in range(NBLK)]
        assert max(sizes) <= 128
        cum = 0
        for k in range(NBLK):
            j = np.arange(cum, cum + sizes[k])
            slot[starts[b] + j] = k * 128 + (j - cum)
            cum += sizes[k]
    pid = (batch * NL + slot).astype(np.int32)  # padded global id [N]

    # ---- edge geometry (host) ----
    src, dst = eidx[0], eidx[1]
    vec = pos[dst] - pos[src] + shifts
    r = np.linalg.norm(vec.astype(np.float64), axis=1).astype(f32)
    uvec = vec / np.maximum(r, 1e-9)[:, None]
    Y = _sph_np(uvec)                           # [E,16]
    ef = _radial_np(r)                          # [E,8]

    # ---- Ewald geometry (host) ----
    dot = pos @ kgrid.T                         # [N,K]
    sd = np.prod(np.sinc(0.5 * DELTA_K * pos), axis=1).astype(f32)   # [N]
    cosd = (sd[:, None] * np.cos(dot)).astype(f32)
    sind = (sd[:, None] * np.sin(dot)).astype(f32)

    kdown = krbf @ np.asarray(inputs['Wdown'], f32)      # [K,DP]

    # ---- edge -> (core, block) assignment, per-block tile counts ----
    gdst = batch[dst]
    kblk = slot[dst] // 128
    ecount = np.zeros((BG, NBLK), np.int64)
    np.add.at(ecount, (gdst, kblk), 1)
    T_list = [max(1, int(np.ceil(ecount[:, k].max() / 128))) for k in range(NBLK)]
    if SCAT_DR:
        T_list = [t + (t % 2) for t in T_list]
    O_list = np.concatenate([[0], np.cumsum(T_list)]).astype(int)
    NT = int(O_list[-1])
    G4 = ((NT + 3) // 4) * 4

    # ---- shared (replicated) weight arrays ----
    g = lambda k: np.asarray(inputs[k], f32)
    shared = {'Wembed': g('W_embed'),
              'ident': np.eye(128, dtype=f32),
              'Wr0': g('Wr0'), 'Wr1a': g('Wr1a'), 'Wr1b': g('Wr1b')}
    # layer-0 hu is weight-only (h0 = attrs @ Wembed): precompute the full
    # gathered table on the host, killing the first AllGather.
    h0_full = attrs @ g('W_embed')                       # [N, C]
    huG0 = np.zeros((BG * NL, C), f32)
    huG0[pid] = h0_full @ g('Wup')[0]
    huG0 = huG0.astype(bf16)
    for i in range(L):
        for nm in ('Wpre1', 'Wpre2', 'Wm1', 'Wm2', 'Wup', 'Wmix'):
            shared[f'{nm}_{i}'] = g(nm)[i]
        shared[f'rW1_{i}'] = g('rW1')[i]
        shared[f'rW2_{i}'] = g('rW2')[i]
        shared[f'rW3_{i}'] = g('rW3')[i]
        # rW4 reshaped l-major: [64, l*128 + c]
        shared[f'rW4_{i}'] = g('rW4')[i].reshape(64, C, 4).transpose(0, 2, 1).reshape(64, 4 * C)
        for nm in ('bpre1', 'bpre2', 'bm1', 'bm2'):
            shared[f'{nm}_{i}'] = g(nm)[i].reshape(C, 1)
        for nm in ('rb1', 'rb2', 'rb3'):
            shared[f'{nm}_{i}'] = g(nm)[i].reshape(64, 1)
        kf = np.zeros((KPAD, C), f32)
        kf[:K] = 0.01 * (kdown @ g('WupE')[i])
        shared[f'kfilt_{i}'] = kf
        shared[f'w2T_{i}'] = g('w2')[i].T.copy()             # [C,4] f32
        shared[f'w3T_{i}'] = g('w3')[i].T.copy()

    layouts = _const_layouts(G4)

    # ---- per-core arrays ----
    in_maps = []
    for b in range(BG):
        sl = slice(starts[b], ends[b])
        per = {}
        slot_b = slot[sl]
        at = np.zeros((NEL, NL), f32)
        at[:, slot_b] = attrs[sl].T
        per['attrsT'] = at
        cam = np.zeros((128, NBLK * KPAD), f32)   # atom-major cosd, per block
        sam = np.zeros((128, NBLK * KPAD), f32)
        ckm = np.zeros((KPAD, NL), f32)           # k-major
        skm = np.zeros((KPAD, NL), f32)
        pr, bb = slot_b % 128, slot_b // 128
        cam.reshape(128, NBLK, KPAD)[pr, bb, :K] = cosd[sl]
        sam.reshape(128, NBLK, KPAD)[pr, bb, :K] = sind[sl]
        ckm[:K, slot_b] = cosd[sl].T
        skm[:K, slot_b] = sind[sl].T
        per['cosdam'], per['sindam'] = cam, sam
        per['cosdkm'], per['sindkm'] = ckm, skm

        efp = np.zeros((8, G4 * 128), f32)
        sip = np.zeros((128, NT), np.int32)
        segY = np.zeros((128, NT * 16 * 128), f32)
        emask = gdst == b
        for k in range(NBLK):
            es = np.nonzero(emask & (kblk == k))[0]
            es = es[np.argsort(slot[dst[es]], kind='stable')]
            s = np.arange(len(es))
            tt, p = s // 128, s % 128
            t = O_list[k] + tt
            efp[:, t * 128 + p] = ef[es].T
            sip[p, t] = pid[src[es]]
            a = slot[dst[es]] - k * 128
            base = t * 2048 + a
            for lm in range(16):
                segY[p, base + lm * 128] = Y[es, lm] / AVG_NEI
        per['efTpack'] = efp

        def pack(entries, np_dt):
            width = sum(e[2] for e in entries)
            arr = np.zeros((128, width), np_dt)
            c0 = 0
            for name, rows, cols in entries:
                src_a = per.get(name, shared.get(name))
                arr[0:rows, c0:c0 + cols] = src_a
                c0 += cols
            return arr

        m = {'srcidx': sip, 'segYpack': segY.astype(segy_np), 'huG0': huG0,
             'cbA0': pack(layouts['cbA0'], bf16), 'cbA': pack(layouts['cbA'], bf16),
             'cbB': pack(layouts['cbB'], bf16), 'cf': pack(layouts['cf'], f32)}
        in_maps.append(m)

    e0 = np.zeros(BG, f32)
    ae = attrs @ np.asarray(inputs['atomic_E'], f32)
    for b in range(BG):
        e0[b] = ae[starts[b]:ends[b]].sum()
    return in_maps, T_list, G4, e0


# ---------------------------------------------------------------- device
def build_kernel(T_list, G4):
    import concourse.bass as bass
    import concourse.bacc as bacc
    import concourse.mybir as mybir
    import concourse.tile as tile

    f32 = mybir.dt.float32
    bf16 = mybir.dt.bfloat16
    sdt = mybir.dt.float8e4 if SEGY_FP8 else bf16
    A = mybir.ActivationFunctionType
    NT = int(sum(T_list))
    Tmax = max(T_list)
    O_list = np.concatenate([[0], np.cumsum(T_list)]).astype(int)
    nc = bacc.Bacc("TRN2", target_bir_lowering=False, debug=False, num_devices=BG)

    dins = {}
    def din(name, shape, dt=f32):
        dins[name] = nc.dram_tensor(name, list(shape), dt, kind="ExternalInput").ap()
        return dins[name]

    # load order = SP queue order: the layer-0 critical path first
    layouts = _const_layouts(G4)
    widths = {buf: sum(e[2] for e in entries) for buf, entries in layouts.items()}
    din('srcidx', (128, NT), mybir.dt.int32)
    din('cbA0', (128, widths['cbA0']), bf16)
    din('cf', (128, widths['cf']))
    din('cbB', (128, widths['cbB']), bf16)
    din('cbA', (128, widths['cbA']), bf16)
    segY_d = din('segYpack', (128, NT * 2048), sdt)
    huG0_d = din('huG0', (BG * NL, C), bf16)
    en_out = nc.dram_tensor('en_out', [1, 1], f32, kind="ExternalOutput").ap()

    with tile.TileContext(nc) as tc:
        with (
            tc.tile_pool(name="const", bufs=1) as cp,
            tc.tile_pool(name="work", bufs=2) as wp,
            tc.tile_pool(name="segy", bufs=5) as sy,
            tc.tile_pool(name="big", bufs=1) as bp,
            tc.tile_pool(name="psA", bufs=1, space="PSUM") as psA,
            tc.tile_pool(name="psS", bufs=2, space="PSUM") as psS,
            tc.tile_pool(name="psW", bufs=2, space="PSUM") as psW,
            tc.tile_pool(name="dram", bufs=1, space="DRAM") as dp,
        ):
            sb = {}
            for name in ('srcidx', 'cbA0', 'cbB', 'cf', 'cbA'):
                ap = dins[name]
                t = cp.tile(list(ap.shape), ap.dtype, tag=name)
                nc.sync.dma_start(t[:], ap[:])
                if name == 'srcidx':
                    sb[name] = t
                else:
                    c0 = 0
                    for nm, rows, cols in layouts[name]:
                        sb[nm] = t[0:rows, c0:c0 + cols]
                        c0 += cols

            h = bp.tile([C, NL], bf16, tag="h")
            en = bp.tile([1, 1], f32, tag="en")
            feats_cm = bp.tile([C, NL], bf16, tag="feats_cm")
            hres_am = bp.tile([128, NBLK * 128], bf16, tag="hres_am")
            nc.vector.memset(en[:], 0.0)

            pe = psW.tile([C, NL], f32, tag="pb")
            nc.tensor.matmul(pe[:], sb['Wembed'][:], sb['attrsT'][:], start=True, stop=True)
            nc.scalar.activation(h[:], pe[:], A.Copy)

            hu_dt = mybir.dt.float8e4 if HU_FP8 else bf16
            coll = {}     # layer -> (huL, huG) for layers >= 1
            for i in range(L):
                # ---- gather source: host table (layer 0) or prior AllGather ----
                huG = huG0_d if i == 0 else coll[i][1][:]
                hugs = []
                for k in range(NBLK):
                    lo, hi = int(O_list[k]), int(O_list[k + 1])
                    hg = wp.tile([128, Tmax * 128], bf16 if i == 0 else hu_dt,
                                 tag=f"hug{k % 2}{i}", bufs=1)
                    head = min(4, hi - lo) if k == 0 else hi - lo
                    nc.gpsimd.indirect_dma_start(
                        out=hg[:, 0:head * 128], out_offset=None, in_=huG[:],
                        in_offset=bass.IndirectOffsetOnAxis(
                            ap=sb['srcidx'][:, lo:lo + head], axis=0))
                    if head < hi - lo:
                        nc.gpsimd.indirect_dma_start(
                            out=hg[:, head * 128:(hi - lo) * 128], out_offset=None,
                            in_=huG[:],
                            in_offset=bass.IndirectOffsetOnAxis(
                                ap=sb['srcidx'][:, lo + head:hi], axis=0))
                    hugs.append(hg)
                if i + 1 < L:
                    huL_next = dp.tile([NL, C], hu_dt, tag=f"huL{i + 1}")
                    huG_next = dp.tile([BG * NL, C], hu_dt, tag=f"huG{i + 1}",
                                       addr_space="Shared")
                    hu_am = wp.tile([128, NL], hu_dt, tag="hu_am")
                    coll[i + 1] = (huL_next, huG_next)

                # ---- radial MLP: issued just-in-time, one 4-tile group ahead
                # of the edge loop (the matmul->silu chain is ~3us latency and
                # would serialize as a phase; interleaved it hides behind the
                # per-tile scatter work).
                s3sb = wp.tile([64, G4 * 128], bf16, tag="s3sb")
                radial_next = [0]

                def radial_group(gidx, i=i, s3sb=s3sb):
                    gsl = slice(gidx * 512, (gidx + 1) * 512)
                    pr1 = psS.tile([128, 512], f32, tag="ps")
                    nc.tensor.matmul(pr1[0:64, :], sb[f'rW1_{i}'][:], sb['efTpack'][:, gsl],
                                     start=True, stop=True)
                    s1 = wp.tile([64, 512], bf16, tag="s1")
                    nc.scalar.activation(s1[:], pr1[0:64, :], A.Silu, bias=sb[f'rb1_{i}'][:])
                    pr2 = psS.tile([128, 512], f32, tag="ps")
                    nc.tensor.matmul(pr2[0:64, :], sb[f'rW2_{i}'][:], s1[:], start=True, stop=True)
                    s2 = wp.tile([64, 512], bf16, tag="s1")
                    nc.scalar.activation(s2[:], pr2[0:64, :], A.Silu, bias=sb[f'rb2_{i}'][:])
                    pr3 = psS.tile([128, 512], f32, tag="ps")
                    nc.tensor.matmul(pr3[0:64, :], sb[f'rW3_{i}'][:], s2[:], start=True, stop=True)
                    nc.scalar.activation(s3sb[:, gsl], pr3[0:64, :], A.Silu, bias=sb[f'rb3_{i}'][:])

                def ensure_radial(gwant):
                    while radial_next[0] <= min(gwant, G4 // 4 - 1):
                        radial_group(radial_next[0])
                        radial_next[0] += 1

                # ---- Ewald block (independent of the collective) ----
                p1 = psW.tile([C, NL], f32, tag="pb")
                nc.tensor.matmul(p1[:], sb[f'Wpre1_{i}'][:], h[:], start=True, stop=True)
                t1 = wp.tile([C, NL], bf16, tag="t1")
                nc.scalar.activation(t1[:], p1[:], A.Silu, bias=sb[f'bpre1_{i}'][:])
                p2 = psW.tile([C, NL], f32, tag="pb")
                nc.tensor.matmul(p2[:], sb[f'Wpre2_{i}'][:], t1[:], start=True, stop=True)
                hres = wp.tile([C, NL], bf16, tag="hres")
                nc.vector.tensor_scalar_add(hres[:], p2[:], sb[f'bpre2_{i}'][:])
                nc.vector.tensor_add(hres[:], hres[:], h[:])
                for k in range(NBLK):
                    pt = psS.tile([128, 512], f32, tag="ps")
                    ptb = pt[:].bitcast(bf16)[:, 0:128]
                    nc.tensor.transpose(ptb, hres[:, k * 128:(k + 1) * 128], sb['ident'][:])
                    nc.scalar.activation(hres_am[:, k * 128:(k + 1) * 128], ptb, A.Copy)
                sfk = {}
                for nm, am in (('r', 'cosdam'), ('i', 'sindam')):
                    psf = psS.tile([128, 512], f32, tag="ps")
                    for k in range(NBLK):
                        nc.tensor.matmul(psf[:, 0:128], sb[am][:, k * KPAD:(k + 1) * KPAD],
                                         hres_am[:, k * 128:(k + 1) * 128],
                                         start=(k == 0), stop=(k == NBLK - 1))
                    s = wp.tile([KPAD, C], bf16, tag=f"sfk{nm}")
                    nc.vector.tensor_tensor(s[:], psf[:, 0:128], sb[f'kfilt_{i}'][:],
                                            op=mybir.AluOpType.mult)
                    sfk[nm] = s
                phe = psW.tile([C, NL], f32, tag="pb")
                nc.tensor.matmul(phe[:], sfk['r'][:], sb['cosdkm'][:], start=True, stop=False)
                nc.tensor.matmul(phe[:], sfk['i'][:], sb['sindkm'][:], start=False, stop=True)
                he0 = wp.tile([C, NL], bf16, tag="he0")
                nc.scalar.activation(he0[:], phe[:], A.Copy)
                pm1 = psW.tile([C, NL], f32, tag="pb")
                nc.tensor.matmul(pm1[:], sb[f'Wm1_{i}'][:], he0[:], start=True, stop=True)
                tm = wp.tile([C, NL], bf16, tag="t1")
                nc.scalar.activation(tm[:], pm1[:], A.Silu, bias=sb[f'bm1_{i}'][:])
                pm2 = psW.tile([C, NL], f32, tag="pb")
                nc.tensor.matmul(pm2[:], sb[f'Wm2_{i}'][:], tm[:], start=True, stop=True)
                he2 = wp.tile([C, NL], bf16, tag="he2")
                nc.scalar.activation(he2[:], pm2[:], A.Silu, bias=sb[f'bm2_{i}'][:])
                # layer 0: start the radial chains right after the dense
                # phase; layer >0: PE is idle under the AllGather, so run the
                # whole radial pipeline there.
                ensure_radial(G4 // 4 - 1 if i > 0 else 1)

                # ---- edge loop ----
                def issue_pair(k, tt0, alt, i=i, hugs=hugs):
                    # one 2-tile unit: paired segY DMA, two rW4 matmuls, two mw
                    # products written fp8 into one [128,1024] tile for the
                    # DoubleRow scatter. Second mw alternates DVE/gpsimd.
                    t = int(O_list[k]) + tt0
                    ensure_radial((t + 1) // 4 + 2)
                    sgt = sy.tile([128, 4096], sdt, tag="sg")
                    nc.sync.dma_start(sgt[:], segY_d[:, t * 2048:(t + 2) * 2048])
                    mw = wp.tile([128, 1024], sdt, tag="mw", bufs=3)
                    for j in range(2):
                        pw = psW.tile([C, NL], f32, tag="pb")
                        nc.tensor.matmul(pw[:], s3sb[:, (t + j) * 128:(t + j + 1) * 128],
                                         sb[f'rW4_{i}'][:], start=True, stop=True)
                        nc.vector.tensor_tensor(
                            mw[:, j * 512:(j + 1) * 512].rearrange("p (l c) -> p l c", l=4),
                            pw[:].rearrange("p (l c) -> p l c", l=4),
                            hugs[k][:, (tt0 + j) * 128:(tt0 + j + 1) * 128]
                                .unsqueeze(1).broadcast_to([128, 4, 128]),
                            op=mybir.AluOpType.mult)
                    return mw, sgt

                PAIRS = [(k, tt0) for k in range(NBLK) for tt0 in range(0, int(T_list[k]), 2)]
                LEAD = 1
                fifo = [issue_pair(*PAIRS[j], alt=(j % 2 == 0)) for j in range(LEAD)]
                tidx = [0]

                def next_mw_sgt():
                    j = tidx[0]
                    if j + LEAD < len(PAIRS):
                        fifo.append(issue_pair(*PAIRS[j + LEAD], alt=(j % 2 == 0)))
                    tidx[0] += 1
                    return fifo.pop(0)

                def make_tail(k, i=i, he2=he2):
                    # layer tail for block k: h update, readout, next layer's hu.
                    # Deferred into the next block's tile stream so the product
                    # basis (DVE) overlaps the next block's scatters (PE).
                    def tail():
                        blk = slice(k * 128, (k + 1) * 128)
                        pmx = psW.tile([C, NL], f32, tag="pb")
                        nc.tensor.matmul(pmx[:, 0:128], sb[f'Wmix_{i}'][:],
                                         feats_cm[:, blk], start=True, stop=True)
                        hnk = wp.tile([C, 128], f32, tag="hn")
                        eng = nc.vector if k == NBLK - 1 else nc.gpsimd
                        nc.vector.tensor_add(hnk[:], pmx[:, 0:128], he2[:, blk])
                        eng.tensor_add(hnk[:], hnk[:], h[:, blk])
                        eng.tensor_scalar_mul(h[:, blk], hnk[:], float(SKIP))
                        if i == 0:
                            prd = psS.tile([128, 512], f32, tag="ps")
                            nc.tensor.matmul(prd[0:1, 0:128], sb['Wr0'][:], h[:, blk],
                                             start=True, stop=True)
                            rs = wp.tile([1, 1], f32, tag="rs")
                            nc.vector.reduce_sum(rs[:], prd[0:1, 0:128],
                                                 axis=mybir.AxisListType.X)
                            nc.vector.tensor_add(en[:], en[:], rs[:])
                        else:
                            pra = psS.tile([128, 512], f32, tag="ps")
                            nc.tensor.matmul(pra[0:16, 0:128], sb['Wr1a'][:], h[:, blk],
                                             start=True, stop=True)
                            ta = wp.tile([16, 128], bf16, tag="ta")
                            nc.scalar.activation(ta[:], pra[0:16, 0:128], A.Silu)
                            prb = psS.tile([128, 512], f32, tag="ps")
                            nc.tensor.matmul(prb[0:1, 0:128], sb['Wr1b'][:], ta[:],
                                             start=True, stop=True)
                            rs = wp.tile([1, 1], f32, tag="rs")
                            nc.vector.reduce_sum(rs[:], prb[0:1, 0:128],
                                                 axis=mybir.AxisListType.X)
                            nc.vector.tensor_add(en[:], en[:], rs[:])
                        if i + 1 < L:
                            ph = psW.tile([C, NL], f32, tag="pb")
                            nc.tensor.matmul(ph[:, 0:128], h[:, blk],
                                             sb[f'Wup_{i + 1}'][:], start=True, stop=True)
                            nc.scalar.activation(hu_am[:, blk], ph[:, 0:128], A.Copy)
                            nc.sync.dma_start(huL_next[k * 128:(k + 1) * 128, :],
                                              hu_am[:, blk])
                    return tail

                def make_pb(k, scal, AA, i=i):
                    # product basis for block k (c-major), reading only SBUF.
                    # Deferred into the next block's pair stream so its DVE ops
                    # don't sit between consecutive blocks' mw ops.
                    def pb():
                        AA3 = AA[:].rearrange("c (m a) -> c a m", m=16)
                        inv = wp.tile([128, 512], f32, tag="inv")
                        nc.vector.tensor_copy(inv[:, 0:128], AA3[:, :, 0])
                        nc.vector.reduce_sum(inv[:, 128:256].unsqueeze(2), AA3[:, :, 1:4],
                                             axis=mybir.AxisListType.X)
                        for l in (2, 3):
                            isl = inv[:, l * 128:(l + 1) * 128]
                            m0, wl = L_START[l], L_WIDTH[l]
                            nc.gpsimd.tensor_tensor(isl, AA3[:, :, m0], AA3[:, :, m0 + 1],
                                                    op=mybir.AluOpType.add)
                            for mm in range(m0 + 2, m0 + wl):
                                nc.gpsimd.tensor_tensor(isl, isl, AA3[:, :, mm],
                                                        op=mybir.AluOpType.add)
                        acc = {}
                        for wnm, eng in (('w2T', nc.vector), ('w3T', nc.gpsimd)):
                            t2 = wp.tile([128, 512], f32, tag=f"t2{wnm}")
                            eng.tensor_tensor(
                                t2[:].rearrange("c (l a) -> c l a", l=4),
                                inv[:].rearrange("c (l a) -> c l a", l=4),
                                sb[f'{wnm}_{i}'][:].unsqueeze(2).broadcast_to([128, 4, 128]),
                                op=mybir.AluOpType.mult)
                            ac = wp.tile([128, 128], f32, tag=f"ac{wnm}")
                            if eng is nc.vector:
                                eng.reduce_sum(ac[:].unsqueeze(2),
                                               t2[:].rearrange("c (l a) -> c a l", l=4),
                                               axis=mybir.AxisListType.X)
                            else:
                                t23 = t2[:].rearrange("c (l a) -> c l a", l=4)
                                eng.tensor_tensor(ac[:], t23[:, 0, :], t23[:, 1, :],
                                                  op=mybir.AluOpType.add)
                                eng.tensor_tensor(ac[:], ac[:], t23[:, 2, :],
                                                  op=mybir.AluOpType.add)
                                eng.tensor_tensor(ac[:], ac[:], t23[:, 3, :],
                                                  op=mybir.AluOpType.add)
                            acc[wnm] = ac
                        fe = wp.tile([128, 128], f32, tag="fe")
                        nc.vector.tensor_tensor(fe[:], scal[:], acc['w3T'][:],
                                                op=mybir.AluOpType.mult)
                        nc.vector.tensor_add(fe[:], fe[:], acc['w2T'][:])
                        nc.vector.tensor_tensor(feats_cm[:, k * 128:(k + 1) * 128], fe[:],
                                                scal[:], op=mybir.AluOpType.add)
                    return pb

                pending = []
                for k in range(NBLK):
                    Tk = int(T_list[k])
                    pA1 = psA.tile([128, 1024], f32, tag="pA1")
                    pA2 = psA.tile([128, 1024], f32, tag="pA2")
                    for pp in range(Tk // 2):
                        mw, sgt = next_mw_sgt()
                        mw3 = mw[:].rearrange("p (two x) -> p two x", two=2)
                        sg3 = sgt[:].rearrange("p (two x) -> p two x", two=2)
                        for (l, m0, w) in CHUNKS:
                            pAh, off = (pA1, m0) if m0 < 8 else (pA2, m0 - 8)
                            nc.tensor.matmul(pAh[:, off * 128:(off + w) * 128],
                                             mw3[:, :, l * 128:(l + 1) * 128],
                                             sg3[:, :, m0 * 128:(m0 + w) * 128],
                                             start=(pp == 0), stop=(pp == Tk // 2 - 1),
                                             perf_mode=mybir.MatmulPerfMode.DoubleRow)
                        if pending and pp == min(2, Tk // 2 - 1):
                            pending.pop(0)()            # previous block's PB
                        if pending and pp == min(4, Tk // 2 - 1):
                            pending.pop(0)()            # previous block's tail
                    # free pA early: AA2 alone frees pA2 (whose half is written
                    # first by the next block), then AA1 + scal free pA1.
                    AA = wp.tile([128, 2048], bf16, tag="AA")
                    nc.scalar.activation(AA[:, 1024:2048], pA2[:], A.Square)
                    nc.scalar.activation(AA[:, 0:1024], pA1[:], A.Square)
                    scal = wp.tile([128, 128], bf16, tag="scal")
                    nc.scalar.activation(scal[:], pA1[:, 0:128], A.Copy)
                    pending = [make_pb(k, scal, AA), make_tail(k)]
                for c in pending:
                    c()
                if i + 1 < L:
                    cin, cout = huL_next[:], huG_next[:]
                    if HU_FP8:
                        cin, cout = cin.bitcast(bf16), cout.bitcast(bf16)
                    nc.gpsimd.collective_compute(
                        "AllGather", mybir.AluOpType.bypass,
                        replica_groups=[list(range(BG))],
                        ins=[cin.opt()], outs=[cout.opt()])
            nc.sync.dma_start(en_out[:], en[:])
    nc.compile()
    return nc


def kernel(**inputs):
    from concourse import bass_utils
    in_maps, T_list, G4, e0 = host_prep(inputs)
    key = (tuple(T_list), G4)
    if key not in _CACHE:
        _CACHE[key] = build_kernel(T_list, G4)
    nc = _CACHE[key]
    res = bass_utils.run_bass_kernel_spmd(nc, in_maps, core_ids=list(range(BG)))
    energy = np.zeros(BG, np.float32)
    for b in range(BG):
        energy[b] = res.results[b]['en_out'].reshape(-1)[0] + e0[b]
    return energy
